# revision 1
# baseline (speedup 1.0000x reference)
"""GAT layer (PyG GATConv eval, 2 heads x 128, self-loops, ELU) on 8 trn2 cores.

Strategy (dst-sharded, per core):
  Phase A: per 128-node tile of full h: PE-transpose -> matmul with Wa4
           (=W.att contractions) -> write a_src to DRAM table TA[N,64] (256B rows).
  Phase A-bis: same on this core's dst shard -> a_dst resident in SBUF.
  Phase B: edges grouped by dst block (128 dsts), chunks of 128 edges.
           dma_gather of h rows (512B) + TA rows (256B) by src id (int16
           lo/hi table split).  Per chunk: dst one-hot masks via iota+is_equal,
           ex = exp(leakyrelu(a_src+a_dst)), GT_h[k,d] += (ex_h*Hg)^T M (PE),
           segsum[d,h] += M^T ex (PE).  Per block: U_h = (GT_h)^T W_h via PE,
           normalize by segsum, +bias, ELU, write out rows.
Softmax max-subtraction is skipped (shift-invariant; logits are O(10) so fp32
exp cannot overflow).
"""
import math
from contextlib import ExitStack

import numpy as np

HEADS = 2
C = 128
IN = 128
N = 50000
NC_CORES = 8
SH = N // NC_CORES            # 6250 dst nodes per core
NBLK = math.ceil(SH / 128)    # 49 dst blocks per core
SHP = NBLK * 128              # padded shard rows 6272
NTILE = math.ceil(N / 128)    # 391 tiles of full h
LO = 32768                    # int16 gather index split
GRP = 4                       # dst blocks per dma_gather call group
NEG_SLOPE = 0.2

_CACHE = {}


# ----------------------------------------------------------------- host prep
def _prep(edge_index):
    src = np.concatenate([edge_index[0], np.arange(N, dtype=np.int64)])
    dst = np.concatenate([edge_index[1], np.arange(N, dtype=np.int64)])
    src = src.astype(np.int64)
    core = dst // SH
    blk = (dst % SH) // 128
    dloc = (dst % SH) % 128
    half = (src >= LO).astype(np.int64)          # 0 = lo, 1 = hi

    key = (core * NBLK + blk) * 2 + half
    order = np.argsort(key, kind="stable")
    key_s = key[order]
    src_s = src[order]
    dloc_s = dloc[order]

    ngroups = NC_CORES * NBLK * 2
    sizes = np.bincount(key_s, minlength=ngroups)
    starts = np.concatenate([[0], np.cumsum(sizes)[:-1]])
    rank = np.arange(len(key_s)) - starts[key_s]

    lo_sizes = sizes.reshape(NC_CORES, NBLK, 2)[:, :, 0]
    hi_sizes = sizes.reshape(NC_CORES, NBLK, 2)[:, :, 1]
    K_LO = int(math.ceil(lo_sizes.max() / 128))
    K_HI = int(math.ceil(hi_sizes.max() / 128))
    K = K_LO + K_HI

    half_s = key_s % 2
    slot = rank + half_s * (K_LO * 128)          # slot within block [0, K*128)

    core_s = key_s // (2 * NBLK)
    blk_s = (key_s // 2) % NBLK

    # int16 gather index per slot (dummy 0 for padding), dst_local (999 pad)
    idx16 = np.zeros((NC_CORES, NBLK, K * 128), dtype=np.int16)
    dstl = np.full((NC_CORES, NBLK, K * 128), 999.0, dtype=np.float32)
    idxv = np.where(half_s == 0, src_s, src_s - LO).astype(np.int16)
    idx16[core_s, blk_s, slot] = idxv
    dstl[core_s, blk_s, slot] = dloc_s.astype(np.float32)

    # wrapped-16 gather index layout per block: w[b, p, col] = idx[col*16 + p%16]
    sl = idx16.reshape(NC_CORES, NBLK, K * 8, 16)       # [., ., col, p16]
    w_lo = np.ascontiguousarray(
        np.broadcast_to(
            sl[:, :, : K_LO * 8].transpose(0, 1, 3, 2)[:, :, None, :, :],
            (NC_CORES, NBLK, 8, 16, K_LO * 8),
        ).reshape(NC_CORES, NBLK, 128, K_LO * 8)
    )
    w_hi = np.ascontiguousarray(
        np.broadcast_to(
            sl[:, :, K_LO * 8 :].transpose(0, 1, 3, 2)[:, :, None, :, :],
            (NC_CORES, NBLK, 8, 16, K_HI * 8),
        ).reshape(NC_CORES, NBLK, 128, K_HI * 8)
    )

    # dstl layouts: [b, p, j] (slot s = j*128+p); uint16 variant for the
    # per-block a_dst table lookup (pads point at entry 127)
    d3 = dstl.reshape(NC_CORES, NBLK, K, 128)           # [., ., j, p]
    dstl_pj = np.ascontiguousarray(d3.transpose(0, 1, 3, 2))  # [., ., p, j]
    dstl_jp = np.ascontiguousarray(d3)                  # [., ., j, p]
    return K_LO, K_HI, w_lo, w_hi, dstl_pj, dstl_jp


# ------------------------------------------------------------ device program
def _build(K_LO, K_HI, phase="full"):
    import concourse.bacc as bacc
    import concourse.bass as bass
    import concourse.mybir as mybir
    import concourse.tile as tile
    from concourse.masks import make_identity

    dt = mybir.dt
    op = mybir.AluOpType
    act = mybir.ActivationFunctionType
    K = K_LO + K_HI
    P = 128

    nc = bacc.Bacc("TRN2", target_bir_lowering=False, debug=False,
                   num_devices=NC_CORES)
    h = nc.dram_tensor("h", [N, IN], dt.float32, kind="ExternalInput")
    h_sh = nc.dram_tensor("h_sh", [SHP, IN], dt.float32, kind="ExternalInput")
    w_in = nc.dram_tensor("w_in", [IN, HEADS * C], dt.float32, kind="ExternalInput")
    asrc_in = nc.dram_tensor("asrc_in", [HEADS, C], dt.float32, kind="ExternalInput")
    adst_in = nc.dram_tensor("adst_in", [HEADS, C], dt.float32, kind="ExternalInput")
    bias_in = nc.dram_tensor("bias_in", [1, HEADS * C], dt.float32, kind="ExternalInput")
    wlo_in = nc.dram_tensor("wlo", [NBLK, P, K_LO * 8], dt.int16, kind="ExternalInput")
    whi_in = nc.dram_tensor("whi", [NBLK, P, K_HI * 8], dt.int16, kind="ExternalInput")
    dpj_in = nc.dram_tensor("dpj", [NBLK * P, K], dt.float32, kind="ExternalInput")
    djp_in = nc.dram_tensor("djp", [NBLK, K * P], dt.float32, kind="ExternalInput")
    ta = nc.dram_tensor("ta", [NTILE * P, 64], dt.float32)
    out_t = nc.dram_tensor("out", [SHP, HEADS * C], dt.float32, kind="ExternalOutput")

    hap = h.ap()
    STAGE = 17  # phase-A tiles staged per TA write burst (391 = 23*17)

    with tile.TileContext(nc) as tc, ExitStack() as ctx:
        const = ctx.enter_context(tc.tile_pool(name="const", bufs=1))
        ctxA = ctx.enter_context(ExitStack())
        sbA = ctxA.enter_context(tc.tile_pool(name="sbA", bufs=3))
        stg = ctxA.enter_context(tc.tile_pool(name="stg", bufs=2))

        # ---- constants
        ident = const.tile([P, P], dt.float32)
        make_identity(nc, ident[:])
        iota_row = const.tile([P, P], dt.float32)
        nc.gpsimd.iota(iota_row[:], pattern=[[1, P]], base=0, channel_multiplier=0,
                       allow_small_or_imprecise_dtypes=True)
        iota_col4 = const.tile([P, 512], dt.float32)
        nc.gpsimd.iota(iota_col4[:], pattern=[[0, 512]], base=0, channel_multiplier=1,
                       allow_small_or_imprecise_dtypes=True)
        w_sb = const.tile([P, HEADS * C], dt.float32)
        nc.sync.dma_start(w_sb[:], w_in.ap()[:, :])

        ones_col = const.tile([P, 1], dt.float32)
        nc.gpsimd.memset(ones_col[:], 1.0)
        ones1 = const.tile([1, P], dt.float32)
        nc.gpsimd.memset(ones1[:], 1.0)
        bias_bc = const.tile([P, HEADS * C], dt.float32)
        nc.sync.dma_start(bias_bc[:], bass.AP(bias_in, 0, [[0, P], [1, HEADS * C]]))

        # Wa4[k, 0:2] = sum_c W[k, h*C+c]*att_src[h, c];  cols 2:4 for att_dst
        wa4 = const.tile([P, 4], dt.float32)
        tmp_pool = ctxA.enter_context(tc.tile_pool(name="watmp", bufs=2))
        for hd in range(HEADS):
            for j, attt in enumerate((asrc_in, adst_in)):
                abc = tmp_pool.tile([P, C], dt.float32, tag="abc")
                nc.sync.dma_start(abc[:], bass.AP(attt, hd * C, [[0, P], [1, C]]))
                t = tmp_pool.tile([P, C], dt.float32, tag="t")
                nc.vector.tensor_tensor(
                    out=t[:], in0=w_sb[:, hd * C:(hd + 1) * C],
                    in1=abc[:], op=op.mult)
                nc.vector.tensor_reduce(
                    out=wa4[:, 2 * j + hd:2 * j + hd + 1], in_=t[:],
                    axis=mybir.AxisListType.X, op=op.add)

        psA = ctxA.enter_context(tc.tile_pool(name="psA", bufs=2, space="PSUM"))
        psA2 = ctxA.enter_context(tc.tile_pool(name="psA2", bufs=2, space="PSUM"))

        # ---- phase A: a_src table for all N (+ phase A-bis shard a_dst)
        adst_sb = const.tile([P, NBLK, 2], dt.float32)

        def attn_tile(src_ap, nrows):
            """load [nrows,128] h rows -> return psum [128,4] a-values tile."""
            ht = sbA.tile([P, IN], dt.float32, tag="ht")
            nc.sync.dma_start(ht[:nrows, :], src_ap)
            tp = psA.tile([P, P], dt.float32, tag="tp", space="PSUM")
            nc.tensor.transpose(out=tp[:], in_=ht[:], identity=ident[:])
            hT = sbA.tile([P, P], dt.float32, tag="hT")
            nc.scalar.copy(out=hT[:], in_=tp[:])
            a4 = psA2.tile([P, 4], dt.float32, tag="a4", space="PSUM")
            nc.tensor.matmul(out=a4[:], lhsT=hT[:], rhs=wa4[:], start=True, stop=True)
            return a4

        for t0 in range(0, NTILE, STAGE):
            nst = min(STAGE, NTILE - t0)
            st = stg.tile([P, STAGE, 4], dt.float32, tag="st")
            for g in range(nst):
                ti = t0 + g
                nrows = min(P, N - ti * P)
                a4 = attn_tile(hap[ti * P:ti * P + nrows, :], nrows)
                nc.scalar.copy(out=st[:, g, :], in_=a4[:])
            # burst write to TA rows [t0*128, (t0+nst)*128), cols 0:4
            out_ap = bass.AP(ta, t0 * P * 64, [[64, P], [P * 64, nst], [1, 4]])
            nc.gpsimd.dma_start(out_ap, st[:, :nst, :])

        for b in range(NBLK):
            a4 = attn_tile(h_sh.ap()[b * P:(b + 1) * P, :], P)
            nc.scalar.copy(out=adst_sb[:, b, 0:2], in_=a4[:, 2:4])

        ctxA.close()  # free phase-A SBUF + PSUM pools before phase B

        # ---- phase B
        gh = ctx.enter_context(tc.tile_pool(name="gh", bufs=2))
        gt = ctx.enter_context(tc.tile_pool(name="gt", bufs=2))
        gi = ctx.enter_context(tc.tile_pool(name="gi", bufs=2))
        mk = ctx.enter_context(tc.tile_pool(name="mk", bufs=3))
        sm = ctx.enter_context(tc.tile_pool(name="sm", bufs=3))
        fin = ctx.enter_context(tc.tile_pool(name="fin", bufs=2))
        psGT = ctx.enter_context(tc.tile_pool(name="psGT", bufs=2, space="PSUM"))
        psSS = ctx.enter_context(tc.tile_pool(name="psSS", bufs=1, space="PSUM"))
        psAD = ctx.enter_context(tc.tile_pool(name="psAD", bufs=1, space="PSUM"))
        psB = ctx.enter_context(tc.tile_pool(name="psB", bufs=1, space="PSUM"))
        psU = ctx.enter_context(tc.tile_pool(name="psU", bufs=1, space="PSUM"))

        taap = ta.ap()
        blk_range = [] if phase == "A" else list(range(0, NBLK, GRP))
        for g0 in blk_range:
            ng = min(GRP, NBLK - g0)
            ilo = gi.tile([P, GRP * K_LO * 8], dt.int16, tag="ilo")
            nc.sync.dma_start(
                ilo[:, : ng * K_LO * 8],
                bass.AP(wlo_in, g0 * P * K_LO * 8,
                        [[K_LO * 8, P], [P * K_LO * 8, ng], [1, K_LO * 8]]))
            ihi = gi.tile([P, GRP * K_HI * 8], dt.int16, tag="ihi")
            nc.sync.dma_start(
                ihi[:, : ng * K_HI * 8],
                bass.AP(whi_in, g0 * P * K_HI * 8,
                        [[K_HI * 8, P], [P * K_HI * 8, ng], [1, K_HI * 8]]))

            hg_lo = gh.tile([P, GRP * K_LO, IN], dt.float32, tag="hglo")
            nc.gpsimd.dma_gather(
                out_ap=hg_lo[:, : ng * K_LO, :], in_ap=hap[0:LO, :],
                idxs_ap=ilo[:, : ng * K_LO * 8], num_idxs=ng * K_LO * P,
                num_idxs_reg=ng * K_LO * P, elem_size=IN, single_packet=False)
            hg_hi = gh.tile([P, GRP * K_HI, IN], dt.float32, tag="hghi")
            nc.gpsimd.dma_gather(
                out_ap=hg_hi[:, : ng * K_HI, :], in_ap=hap[LO:N, :],
                idxs_ap=ihi[:, : ng * K_HI * 8], num_idxs=ng * K_HI * P,
                num_idxs_reg=ng * K_HI * P, elem_size=IN, single_packet=False)
            ta_lo = gt.tile([P, GRP * K_LO, 64], dt.float32, tag="talo")
            nc.gpsimd.dma_gather(
                out_ap=ta_lo[:, : ng * K_LO, :], in_ap=taap[0:LO, :],
                idxs_ap=ilo[:, : ng * K_LO * 8], num_idxs=ng * K_LO * P,
                num_idxs_reg=ng * K_LO * P, elem_size=64, single_packet=False)
            ta_hi = gt.tile([P, GRP * K_HI, 64], dt.float32, tag="tahi")
            nc.gpsimd.dma_gather(
                out_ap=ta_hi[:, : ng * K_HI, :], in_ap=taap[LO:NTILE * P, :],
                idxs_ap=ihi[:, : ng * K_HI * 8], num_idxs=ng * K_HI * P,
                num_idxs_reg=ng * K_HI * P, elem_size=64, single_packet=False)

            if phase == "gather":
                ob0 = fin.tile([P, HEADS * C], dt.float32, tag="ob")
                nc.vector.tensor_copy(out=ob0[:, 0:IN], in_=hg_lo[:, 0, :])
                nc.vector.tensor_copy(out=ob0[:, IN:IN + 64], in_=ta_lo[:, 0, :])
                nc.vector.tensor_copy(out=ob0[:, IN + 64:IN + 128],
                                      in_=ta_hi[:, 0, :])
                nc.sync.dma_start(out_t.ap()[(g0 // GRP) * P:(g0 // GRP + 1) * P, :], ob0[:])
                continue
            for bg in range(ng):
                b = g0 + bg
                dpj = sm.tile([P, K], dt.float32, tag="dpj")
                nc.sync.dma_start(dpj[:], dpj_in.ap()[b * P:(b + 1) * P, :])
                djp = sm.tile([1, K * P], dt.float32, tag="djp")
                nc.sync.dma_start(djp[:], djp_in.ap()[b:b + 1, :])

                # a_dst per edge slot via transposed one-hot matmuls,
                # 4 chunks per broadcast round
                adp = psAD.tile([P, K, 2], dt.float32, tag="adp", space="PSUM")
                for j0 in range(0, K, 4):
                    nb = min(4, K - j0)
                    bc = psB.tile([P, 512], dt.float32, tag="bc", space="PSUM")
                    nc.tensor.matmul(out=bc[:, : nb * P], lhsT=ones1[:],
                                     rhs=djp[:, j0 * P:(j0 + nb) * P],
                                     start=True, stop=True)
                    mt4 = mk.tile([P, 512], dt.float32, tag="mt4")
                    nc.vector.tensor_tensor(out=mt4[:, : nb * P],
                                            in0=iota_col4[:, : nb * P],
                                            in1=bc[:, : nb * P], op=op.is_equal)
                    for jj in range(nb):
                        nc.tensor.matmul(out=adp[:, j0 + jj, :],
                                         lhsT=mt4[:, jj * P:(jj + 1) * P],
                                         rhs=adst_sb[:, b, :],
                                         start=True, stop=True)

                # logits -> ex for every slot of the block, batched wide ops
                tsum = sm.tile([P, K, 2], dt.float32, tag="tsum")
                nc.vector.tensor_tensor(
                    out=tsum[:, :K_LO, :],
                    in0=ta_lo[:, bg * K_LO:(bg + 1) * K_LO, 0:2],
                    in1=adp[:, :K_LO, :], op=op.add)
                nc.vector.tensor_tensor(
                    out=tsum[:, K_LO:, :],
                    in0=ta_hi[:, bg * K_HI:(bg + 1) * K_HI, 0:2],
                    in1=adp[:, K_LO:, :], op=op.add)
                u02 = sm.tile([P, K, 2], dt.float32, tag="u02")
                nc.vector.tensor_scalar(out=u02[:], in0=tsum[:], scalar1=NEG_SLOPE,
                                        scalar2=None, op0=op.mult)
                lr = sm.tile([P, K, 2], dt.float32, tag="lr")
                nc.vector.tensor_tensor(out=lr[:], in0=tsum[:], in1=u02[:],
                                        op=op.max)
                ex = sm.tile([P, K, 2], dt.float32, tag="ex")
                nc.scalar.activation(out=ex[:], in_=lr[:], func=act.Exp)

                gtt = psGT.tile([P, 2 * P], dt.float32, tag="gt", space="PSUM")
                ss0 = psSS.tile([P, 1], dt.float32, tag="ss0", space="PSUM")
                ss1 = psSS.tile([P, 1], dt.float32, tag="ss1", space="PSUM")

                for j in range(K):
                    if j < K_LO:
                        hgc = hg_lo[:, bg * K_LO + j, :]
                    else:
                        hgc = hg_hi[:, bg * K_HI + (j - K_LO), :]
                    st_ = j == 0
                    sp = j == K - 1
                    exm = mk.tile([P, 2 * P], dt.float32, tag="exm")
                    for hd, sstile in ((0, ss0), (1, ss1)):
                        nc.vector.tensor_scalar(
                            out=exm[:, hd * P:(hd + 1) * P], in0=iota_row[:],
                            scalar1=dpj[:, j:j + 1],
                            scalar2=ex[:, j, hd:hd + 1],
                            op0=op.is_equal, op1=op.mult)
                        nc.tensor.matmul(out=sstile[:],
                                         lhsT=exm[:, hd * P:(hd + 1) * P],
                                         rhs=ones_col[:], start=st_, stop=sp)
                    nc.tensor.matmul(out=gtt[:], lhsT=hgc, rhs=exm[:],
                                     start=st_, stop=sp)

                # ---- finalize block b
                rec = fin.tile([P, 2], dt.float32, tag="rec")
                nc.vector.reciprocal(out=rec[:, 0:1], in_=ss0[:])
                nc.vector.reciprocal(out=rec[:, 1:2], in_=ss1[:])
                ob = fin.tile([P, HEADS * C], dt.float32, tag="ob")
                for hd in range(HEADS):
                    gs = fin.tile([P, P], dt.float32, tag="gs")
                    nc.scalar.copy(out=gs[:], in_=gtt[:, hd * P:(hd + 1) * P])
                    u = psU.tile([P, P], dt.float32, tag="u", space="PSUM")
                    nc.tensor.matmul(out=u[:], lhsT=gs[:],
                                     rhs=w_sb[:, hd * C:(hd + 1) * C],
                                     start=True, stop=True)
                    o = fin.tile([P, C], dt.float32, tag="o")
                    nc.vector.tensor_scalar(
                        out=o[:], in0=u[:], scalar1=rec[:, hd:hd + 1],
                        scalar2=None, op0=op.mult)
                    o2 = fin.tile([P, C], dt.float32, tag="o2")
                    nc.vector.tensor_tensor(
                        out=o2[:], in0=o[:],
                        in1=bias_bc[:, hd * C:(hd + 1) * C], op=op.add)
                    a1 = fin.tile([P, C], dt.float32, tag="a1")
                    nc.vector.tensor_scalar(out=a1[:], in0=o2[:], scalar1=0.0,
                                            scalar2=None, op0=op.min)
                    e1 = fin.tile([P, C], dt.float32, tag="e1")
                    nc.scalar.activation(out=e1[:], in_=a1[:], func=act.Exp)
                    a3 = fin.tile([P, C], dt.float32, tag="a3")
                    nc.vector.tensor_scalar(out=a3[:], in0=o2[:], scalar1=0.0,
                                            scalar2=-1.0, op0=op.max, op1=op.add)
                    nc.vector.tensor_tensor(
                        out=ob[:, hd * C:(hd + 1) * C], in0=a3[:], in1=e1[:],
                        op=op.add)
                nc.sync.dma_start(out_t.ap()[b * P:(b + 1) * P, :], ob[:])

    nc.compile()
    return nc


def _get_program(K_LO, K_HI):
    key = (K_LO, K_HI)
    if key not in _CACHE:
        _CACHE[key] = _build(K_LO, K_HI)
    return _CACHE[key]


# ------------------------------------------------------------------- kernel
def kernel(h_node, edge_index, W, att_src, att_dst, bias):
    from concourse.bass_utils import run_bass_kernel_spmd

    h_node = np.asarray(h_node, dtype=np.float32)
    W = np.asarray(W, dtype=np.float32)
    att_src = np.asarray(att_src, dtype=np.float32)
    att_dst = np.asarray(att_dst, dtype=np.float32)
    bias = np.asarray(bias, dtype=np.float32).reshape(1, HEADS * C)

    K_LO, K_HI, w_lo, w_hi, dstl_pj, dstl_jp = _prep(np.asarray(edge_index))
    nc = _get_program(K_LO, K_HI)

    in_maps = []
    for c in range(NC_CORES):
        hs = np.zeros((SHP, IN), dtype=np.float32)
        hs[:SH] = h_node[c * SH:(c + 1) * SH]
        in_maps.append({
            "h": h_node, "h_sh": hs, "w_in": W, "asrc_in": att_src,
            "adst_in": att_dst, "bias_in": bias,
            "wlo": w_lo[c], "whi": w_hi[c],
            "dpj": dstl_pj[c].reshape(NBLK * 128, K_LO + K_HI),
            "djp": dstl_jp[c].reshape(NBLK, (K_LO + K_HI) * 128),
        })
    res = run_bass_kernel_spmd(nc, in_maps, core_ids=list(range(NC_CORES)))
    out = np.concatenate([res.results[c]["out"][:SH] for c in range(NC_CORES)], axis=0)
    return out



# revision 2
# speedup vs baseline: 1.0477x; 1.0477x over previous
"""GAT layer (PyG GATConv eval, 2 heads x 128, self-loops, ELU) on 8 trn2 cores.

v2 design (dst-block sharded, rank-dealt, bf16 datapath):
  - ht table [50048, 256] bf16 in DRAM: cols 0:128 = bf16(h) (host-uploaded),
    cols 128:136 = a_src/a_dst logits as bf16 hi/lo pairs (device-computed in
    phase A).  One 512B-row dma_gather per edge fetches h AND the src logits.
  - Global dst blocks (128 nodes) are dealt to (core, position) slots by edge
    count rank so per-position chunk counts are uniform across cores (SPMD).
  - Edges sorted by (core, pos, src<32768, dst_local); per (pos, half) padded
    to 128-slot chunks.  Self loops ride the edge stream.
  - Per-slot a_dst via "staircase" matmul: SM[d, slot] = (slot >= first slot of
    dst d's run), adp = SM^T @ delta(a_dst) reconstructs a_dst[dst(slot)]
    exactly (fp16 hi/lo deltas).  No one-hot broadcast machinery.
  - exm one-hot masks in bf16 (4x DVE mode); gtt/ss/U matmuls in bf16.
  - Finalize: normalize on Act engine, ELU via exp/min/max identity.
"""
import math
from contextlib import ExitStack

import numpy as np
import ml_dtypes

BF16 = ml_dtypes.bfloat16
FP16 = np.float16

HEADS = 2
C = 128
IN = 128
N = 50000
NC_CORES = 8
NTILE = math.ceil(N / 128)        # 391 tiles / global blocks
NROWS = NTILE * 128               # 50048 table rows
POS = math.ceil(NTILE / NC_CORES)  # 49 positions per core
LO = 32768                        # int16 gather index split
GRP = 4                           # positions per dma_gather call
RND = 16                          # max chunks per staircase round
NEG_SLOPE = 0.2
STAGE = 16                        # phase-A tiles per group

_CACHE = {}


# ----------------------------------------------------------------- host prep
def _prep(edge_index):
    src = np.concatenate([edge_index[0], np.arange(N)]).astype(np.int64)
    dst = np.concatenate([edge_index[1], np.arange(N)]).astype(np.int64)
    g = dst // 128
    dloc = dst % 128
    half = (src >= LO).astype(np.int64)

    sizes_g = np.bincount(g, minlength=NTILE)
    order_g = np.argsort(-sizes_g, kind="stable")
    gmap = np.full((NC_CORES, POS), -1, dtype=np.int64)
    for j in range(POS):
        for c in range(NC_CORES):
            r = NC_CORES * j + c
            if r < NTILE:
                gmap[c, j] = order_g[r]
    core_of = np.zeros(NTILE, dtype=np.int64)
    pos_of = np.zeros(NTILE, dtype=np.int64)
    for c in range(NC_CORES):
        for j in range(POS):
            gg = gmap[c, j]
            if gg >= 0:
                core_of[gg] = c
                pos_of[gg] = j

    ecore = core_of[g]
    epos = pos_of[g]
    key = ((ecore * POS + epos) * 2 + half) * 128 + dloc
    order = np.argsort(key, kind="stable")
    src_s = src[order]
    dloc_s = dloc[order]
    ecore_s = ecore[order]
    epos_s = epos[order]
    half_s = half[order]

    cnt = np.zeros((NC_CORES, POS, 2), dtype=np.int64)
    np.add.at(cnt, (ecore_s, epos_s, half_s), 1)
    K_LO = np.ceil(cnt[:, :, 0].max(axis=0) / 128).astype(int)  # [POS]
    K_HI = np.ceil(cnt[:, :, 1].max(axis=0) / 128).astype(int)
    K_ALL = K_LO + K_HI
    SUM_LO = int(K_LO.sum())
    SUM_HI = int(K_HI.sum())
    SUM_K = int(K_ALL.sum())
    # rounds per (pos, half)
    R_LO = [math.ceil(k / RND) if k else 0 for k in K_LO]
    R_HI = [math.ceil(k / RND) if k else 0 for k in K_HI]
    SUM_R = int(sum(R_LO) + sum(R_HI))

    # group starts (of edges) per (core, pos, half)
    starts = np.zeros(NC_CORES * POS * 2 + 1, dtype=np.int64)
    np.cumsum(np.bincount(
        (ecore_s * POS + epos_s) * 2 + half_s,
        minlength=NC_CORES * POS * 2), out=starts[1:])

    # per-core tables
    wlo = np.zeros((NC_CORES, 128, SUM_LO * 8), dtype=np.int16)
    whi = np.zeros((NC_CORES, 128, SUM_HI * 8), dtype=np.int16)
    dpj = np.full((NC_CORES, 128, SUM_K), 999.0, dtype=np.float32)
    bnd = np.zeros((NC_CORES, 128, max(SUM_R, 1)), dtype=np.float32)
    bglo = np.zeros((NC_CORES, 128, POS * 8), dtype=np.int16)
    bghi = np.zeros((NC_CORES, 128, POS * 8), dtype=np.int16)
    bgmask = np.zeros((NC_CORES, 128, POS * 2), dtype=np.uint8)

    def wrap16(idx):
        """idx [n] (n % 128 == 0) -> wrapped [128, n // 16] int16."""
        n = len(idx)
        sl = idx.reshape(n // 16, 16).T            # [16, n/16]
        return np.broadcast_to(sl[None, :, :], (8, 16, n // 16)).reshape(
            128, n // 16).astype(np.int16)

    for c in range(NC_CORES):
        off_lo = 0
        off_hi = 0
        off_k = 0
        off_r = 0
        for j in range(POS):
            gg = gmap[c, j]
            for h in range(2):
                K = int((K_LO if h == 0 else K_HI)[j])
                nt = K * 128
                if gg >= 0:
                    s0 = starts[(c * POS + j) * 2 + h]
                    s1 = starts[(c * POS + j) * 2 + h + 1]
                    srcs = src_s[s0:s1]
                    dls = dloc_s[s0:s1]
                else:
                    srcs = np.zeros(0, dtype=np.int64)
                    dls = np.zeros(0, dtype=np.int64)
                n = len(srcs)
                assert n <= nt
                idx = np.zeros(nt, dtype=np.int64)
                idx[:n] = srcs - (LO if h == 1 else 0)
                w = wrap16(idx)
                # slot i -> (chunk i//128, partition i%128)
                dv = np.full(nt, 999.0, dtype=np.float32)
                dv[:n] = dls
                dcol = dv.reshape(K, 128).T if K else np.zeros((128, 0), np.float32)
                # staircase boundaries per round
                first = np.searchsorted(dls, np.arange(128), side="left")  # [128]
                R = math.ceil(K / RND) if K else 0
                for r in range(R):
                    lo_c = r * RND * 128
                    ln = min(RND * 128, nt - lo_c)
                    b = np.clip(first - lo_c, 0, ln).astype(np.float32)
                    bnd[c, :, off_r + r] = b
                if h == 0:
                    wlo[c, :, off_lo * 8:(off_lo + K) * 8] = w
                    off_lo += K
                else:
                    whi[c, :, off_hi * 8:(off_hi + K) * 8] = w
                    off_hi += K
                dpj[c, :, off_k:off_k + K] = dcol
                off_k += K
                off_r += R
            # block gather (a_dst per block)
            if gg >= 0:
                nodes = 128 * gg + np.arange(128)
                if gg < 256:
                    bglo[c, :, j * 8:(j + 1) * 8] = wrap16(nodes)
                    bgmask[c, :, j * 2:(j + 1) * 2] = 1
                else:
                    bghi[c, :, j * 8:(j + 1) * 8] = wrap16(nodes - LO)
    return dict(gmap=gmap, K_LO=K_LO, K_HI=K_HI, R_LO=R_LO, R_HI=R_HI,
                SUM_LO=SUM_LO, SUM_HI=SUM_HI, SUM_K=SUM_K, SUM_R=SUM_R,
                wlo=wlo, whi=whi, dpj=dpj, bnd=bnd,
                bglo=bglo, bghi=bghi, bgmask=bgmask)


# ------------------------------------------------------------ device program
def _build(K_LO, K_HI, R_LO, R_HI):
    import concourse.bacc as bacc
    import concourse.bass as bass
    import concourse.mybir as mybir
    import concourse.tile as tile
    from concourse.masks import make_identity

    dt = mybir.dt
    op = mybir.AluOpType
    act = mybir.ActivationFunctionType
    P = 128
    SUM_LO = int(sum(K_LO))
    SUM_HI = int(sum(K_HI))
    SUM_K = SUM_LO + SUM_HI
    SUM_R = int(sum(R_LO) + sum(R_HI))
    KMAX = int(max(K_LO[j] + K_HI[j] for j in range(POS)))
    # per-group gather widths
    g_lo = [int(sum(K_LO[g:min(g + GRP, POS)])) for g in range(0, POS, GRP)]
    g_hi = [int(sum(K_HI[g:min(g + GRP, POS)])) for g in range(0, POS, GRP)]
    GLOMAX = max(g_lo)
    GHIMAX = max(g_hi)

    nc = bacc.Bacc("TRN2", target_bir_lowering=False, debug=False,
                   num_devices=NC_CORES)
    htab = nc.dram_tensor("htab", [128 * NTILE, 128], dt.bfloat16,
                          kind="ExternalInput")       # p-major bf16 h
    th = nc.dram_tensor("th", [NROWS, 256], dt.bfloat16,
                        kind="ExternalInput")         # node-major gather table
    w_in = nc.dram_tensor("w_in", [IN, HEADS * C], dt.float32, kind="ExternalInput")
    asrc_in = nc.dram_tensor("asrc_in", [HEADS, C], dt.float32, kind="ExternalInput")
    adst_in = nc.dram_tensor("adst_in", [HEADS, C], dt.float32, kind="ExternalInput")
    bias_in = nc.dram_tensor("bias_in", [1, HEADS * C], dt.float32, kind="ExternalInput")
    wlo_in = nc.dram_tensor("wlo", [128, SUM_LO * 8], dt.int16, kind="ExternalInput")
    whi_in = nc.dram_tensor("whi", [128, SUM_HI * 8], dt.int16, kind="ExternalInput")
    dpj_in = nc.dram_tensor("dpj", [128, SUM_K], dt.float32, kind="ExternalInput")
    bnd_in = nc.dram_tensor("bnd", [128, max(SUM_R, 1)], dt.float32, kind="ExternalInput")
    bglo_in = nc.dram_tensor("bglo", [128, POS * 8], dt.int16, kind="ExternalInput")
    bghi_in = nc.dram_tensor("bghi", [128, POS * 8], dt.int16, kind="ExternalInput")
    bgm_in = nc.dram_tensor("bgm", [128, POS * 2], dt.uint8, kind="ExternalInput")
    out_t = nc.dram_tensor("out", [POS * 128, HEADS * C], dt.float32,
                           kind="ExternalOutput")

    with tile.TileContext(nc) as tc, ExitStack() as ctx:
        const = ctx.enter_context(tc.tile_pool(name="const", bufs=1))

        # ---- constants
        ident_bf = const.tile([P, P], dt.bfloat16)
        make_identity(nc, ident_bf[:])
        iota_row = const.tile([P, P], dt.bfloat16)
        nc.gpsimd.iota(iota_row[:], pattern=[[1, P]], base=0, channel_multiplier=0,
                       allow_small_or_imprecise_dtypes=True)
        iota2k = const.tile([P, RND * 128], dt.float16)
        nc.gpsimd.iota(iota2k[:], pattern=[[1, RND * 128]], base=0,
                       channel_multiplier=0, allow_small_or_imprecise_dtypes=True)
        ones_bf = const.tile([P, 1], dt.bfloat16)
        nc.gpsimd.memset(ones_bf[:], 1.0)
        iota_cp1 = const.tile([P, 1], dt.float32)
        nc.gpsimd.iota(iota_cp1[:], pattern=[[0, 1]], base=1, channel_multiplier=1,
                       allow_small_or_imprecise_dtypes=True)
        shiftmat = const.tile([P, P], dt.bfloat16)
        nc.vector.tensor_scalar(out=shiftmat[:], in0=iota_row[:],
                                scalar1=iota_cp1[:], scalar2=None, op0=op.is_equal)
        w_sb = const.tile([P, HEADS * C], dt.float32)
        nc.sync.dma_start(w_sb[:], w_in.ap()[:, :])
        w_bf = const.tile([P, HEADS * C], dt.bfloat16)
        nc.vector.tensor_scalar(out=w_bf[:], in0=w_sb[:], scalar1=0.0,
                                scalar2=None, op0=op.add)
        bias_bf = const.tile([P, HEADS * C], dt.bfloat16)
        bias_f32 = const.tile([P, HEADS * C], dt.float32)
        nc.sync.dma_start(bias_f32[:], bass.AP(bias_in, 0, [[0, P], [1, HEADS * C]]))
        nc.vector.tensor_scalar(out=bias_bf[:], in0=bias_f32[:], scalar1=0.0,
                                scalar2=None, op0=op.add)

        # wa4[k, i] = sum_c W[k, h*C+c]*att[h, c]; cols: as0 as1 ad0 ad1
        wa4 = const.tile([P, 4], dt.float32)
        wa4hl = const.tile([P, 8], dt.bfloat16)   # [hi0..hi3, lo0..lo3]
        with tc.tile_pool(name="watmp", bufs=2) as tmp_pool:
            for jat, attt in enumerate((asrc_in, adst_in)):
                for hd in range(HEADS):
                    abc = tmp_pool.tile([P, C], dt.float32, tag="abc")
                    nc.sync.dma_start(abc[:], bass.AP(attt, hd * C, [[0, P], [1, C]]))
                    t = tmp_pool.tile([P, C], dt.float32, tag="t")
                    nc.vector.tensor_tensor(
                        out=t[:], in0=w_sb[:, hd * C:(hd + 1) * C],
                        in1=abc[:], op=op.mult)
                    nc.vector.tensor_reduce(
                        out=wa4[:, 2 * jat + hd:2 * jat + hd + 1], in_=t[:],
                        axis=mybir.AxisListType.X, op=op.add)
            nc.vector.tensor_scalar(out=wa4hl[:, 0:4], in0=wa4[:], scalar1=0.0,
                                    scalar2=None, op0=op.add)
            hic = tmp_pool.tile([P, 4], dt.float32, tag="hic")
            nc.vector.tensor_scalar(out=hic[:], in0=wa4hl[:, 0:4], scalar1=0.0,
                                    scalar2=None, op0=op.add)
            lo32 = tmp_pool.tile([P, 4], dt.float32, tag="lo32")
            nc.vector.tensor_tensor(out=lo32[:], in0=wa4[:], in1=hic[:],
                                    op=op.subtract)
            nc.vector.tensor_scalar(out=wa4hl[:, 4:8], in0=lo32[:], scalar1=0.0,
                                    scalar2=None, op0=op.add)

        # ---- phase A: write a_src/a_dst hi/lo into th[:, 128:136]
        ctxA = ExitStack()
        sbA = ctxA.enter_context(tc.tile_pool(name="sbA", bufs=2))
        psT = ctxA.enter_context(tc.tile_pool(name="psT", bufs=2, space="PSUM"))
        psA8 = ctxA.enter_context(tc.tile_pool(name="psA8", bufs=2, space="PSUM"))
        stgA = ctxA.enter_context(tc.tile_pool(name="stgA", bufs=2))

        for t0 in range(0, NTILE, STAGE):
            nst = min(STAGE, NTILE - t0)
            htile = sbA.tile([P, STAGE, 128], dt.bfloat16, tag="htile")
            nc.sync.dma_start(
                htile[:, :nst, :],
                bass.AP(htab, t0 * 128, [[NTILE * 128, P], [128, nst], [1, 128]]))
            tp = psT.tile([P, STAGE * 128], dt.bfloat16, tag="tp", space="PSUM")
            for gi in range(nst):
                nc.tensor.transpose(out=tp[:, gi * 128:(gi + 1) * 128],
                                    in_=htile[:, gi, :], identity=ident_bf[:])
            hT = sbA.tile([P, STAGE * 128], dt.bfloat16, tag="hT")
            nc.vector.tensor_scalar(out=hT[:, :nst * 128], in0=tp[:, :nst * 128],
                                    scalar1=0.0, scalar2=None, op0=op.add)
            a8 = psA8.tile([P, STAGE, 8], dt.float32, tag="a8", space="PSUM")
            for gi in range(nst):
                nc.tensor.matmul(out=a8[:, gi, :],
                                 lhsT=hT[:, gi * 128:(gi + 1) * 128],
                                 rhs=wa4hl[:], start=True, stop=True)
            a8s = stgA.tile([P, STAGE, 8], dt.float32, tag="a8s")
            nc.vector.tensor_scalar(out=a8s[:, :nst, :], in0=a8[:, :nst, :],
                                    scalar1=0.0, scalar2=None, op0=op.add)
            a4g = stgA.tile([P, STAGE, 4], dt.float32, tag="a4g")
            nc.vector.tensor_tensor(out=a4g[:, :nst, :], in0=a8s[:, :nst, 0:4],
                                    in1=a8s[:, :nst, 4:8], op=op.add)
            # th cols 128:136 = [as0h as1h ad0h ad1h | as0l as1l ad0l ad1l]
            st = stgA.tile([P, STAGE, 8], dt.bfloat16, tag="st")
            nc.vector.tensor_scalar(out=st[:, :nst, 0:4], in0=a4g[:, :nst, :],
                                    scalar1=0.0, scalar2=None, op0=op.add)
            hic4 = stgA.tile([P, STAGE, 4], dt.float32, tag="hic4")
            nc.vector.tensor_scalar(out=hic4[:, :nst, :], in0=st[:, :nst, 0:4],
                                    scalar1=0.0, scalar2=None, op0=op.add)
            lo32t = stgA.tile([P, STAGE, 4], dt.float32, tag="lo32t")
            nc.vector.tensor_tensor(out=lo32t[:, :nst, :], in0=a4g[:, :nst, :],
                                    in1=hic4[:, :nst, :], op=op.subtract)
            nc.vector.tensor_scalar(out=st[:, :nst, 4:8], in0=lo32t[:, :nst, :],
                                    scalar1=0.0, scalar2=None, op0=op.add)
            nc.scalar.dma_start(
                bass.AP(th, (128 * t0) * 256 + 128,
                        [[256, P], [128 * 256, nst], [1, 8]]),
                st[:, :nst, :])
        ctxA.close()

        # ---- block gather: a_dst hi/lo per (pos, dst_local) + fp16 deltas
        bgp = ExitStack()
        bgpool = bgp.enter_context(tc.tile_pool(name="bgpool", bufs=1))
        lo_ap = bass.AP(th, 0, [[256, LO], [1, 256]])
        hi_ap = bass.AP(th, LO * 256, [[256, NROWS - LO], [1, 256]])
        bgidx = bgpool.tile([P, POS * 8], dt.int16, tag="bgidx")
        nc.sync.dma_start(bgidx[:], bglo_in.ap()[:, :])
        bgidx2 = bgpool.tile([P, POS * 8], dt.int16, tag="bgidx2")
        nc.sync.dma_start(bgidx2[:], bghi_in.ap()[:, :])
        bgA = bgpool.tile([P, POS, 256], dt.bfloat16, tag="bgA")
        nc.gpsimd.dma_gather(
            out_ap=bgA[:], in_ap=lo_ap, idxs_ap=bgidx[:],
            num_idxs=POS * 128, num_idxs_reg=POS * 128, elem_size=256,
            single_packet=False)
        bgB = bgpool.tile([P, POS, 256], dt.bfloat16, tag="bgB")
        nc.gpsimd.dma_gather(
            out_ap=bgB[:], in_ap=hi_ap, idxs_ap=bgidx2[:],
            num_idxs=POS * 128, num_idxs_reg=POS * 128, elem_size=256,
            single_packet=False)
        bgm = bgpool.tile([P, POS, 2], dt.uint8, tag="bgm")
        nc.sync.dma_start(bgm[:], bgm_in.ap()[:, :])
        # a_dst hi/lo per block: th cols 130:132 (hi), 134:136 (lo); A/B merge
        ad4 = bgpool.tile([P, POS, 4], dt.bfloat16, tag="ad4")
        nc.vector.tensor_copy(out=ad4[:, :, 0:2], in_=bgB[:, :, 130:132])
        nc.vector.tensor_copy(out=ad4[:, :, 2:4], in_=bgB[:, :, 134:136])
        nc.vector.copy_predicated(out=ad4[:, :, 0:2], mask=bgm[:],
                                  data=bgA[:, :, 130:132])
        nc.vector.copy_predicated(out=ad4[:, :, 2:4], mask=bgm[:],
                                  data=bgA[:, :, 134:136])
        ad4f = bgpool.tile([P, POS, 4], dt.float32, tag="ad4f")
        nc.vector.tensor_scalar(out=ad4f[:], in0=ad4[:], scalar1=0.0,
                                scalar2=None, op0=op.add)
        adf = bgpool.tile([P, POS, 2], dt.float32, tag="adf")
        nc.vector.tensor_tensor(out=adf[:], in0=ad4f[:, :, 0:2],
                                in1=ad4f[:, :, 2:4], op=op.add)
        # shifted[d] = a_dst[d-1] via shift-matrix matmul (exact in fp32 psum)
        psBG = bgp.enter_context(tc.tile_pool(name="psBG", bufs=1, space="PSUM"))
        sh4 = psBG.tile([P, POS, 4], dt.float32, tag="sh4", space="PSUM")
        nc.tensor.matmul(out=sh4[:], lhsT=shiftmat[:], rhs=ad4[:],
                         start=True, stop=True)
        sh4s = bgpool.tile([P, POS, 4], dt.float32, tag="sh4s")
        nc.vector.tensor_scalar(out=sh4s[:], in0=sh4[:], scalar1=0.0,
                                scalar2=None, op0=op.add)
        shf = bgpool.tile([P, POS, 2], dt.float32, tag="shf")
        nc.vector.tensor_tensor(out=shf[:], in0=sh4s[:, :, 0:2],
                                in1=sh4s[:, :, 2:4], op=op.add)
        dlt = bgpool.tile([P, POS, 2], dt.float32, tag="dlt")
        nc.vector.tensor_tensor(out=dlt[:], in0=adf[:], in1=shf[:],
                                op=op.subtract)
        delta4 = const.tile([P, POS, 4], dt.float16)
        nc.vector.tensor_scalar(out=delta4[:, :, 0:2], in0=dlt[:],
                                scalar1=0.0, scalar2=None, op0=op.add)
        dhc = bgpool.tile([P, POS, 2], dt.float32, tag="dhc")
        nc.vector.tensor_scalar(out=dhc[:], in0=delta4[:, :, 0:2],
                                scalar1=0.0, scalar2=None, op0=op.add)
        dlo = bgpool.tile([P, POS, 2], dt.float32, tag="dlo")
        nc.vector.tensor_tensor(out=dlo[:], in0=dlt[:], in1=dhc[:],
                                op=op.subtract)
        nc.vector.tensor_scalar(out=delta4[:, :, 2:4], in0=dlo[:],
                                scalar1=0.0, scalar2=None, op0=op.add)
        bgp.close()

        # ---- phase B preloads
        wlo_sb = const.tile([P, SUM_LO * 8], dt.int16)
        nc.sync.dma_start(wlo_sb[:], wlo_in.ap()[:, :])
        whi_sb = const.tile([P, SUM_HI * 8], dt.int16)
        nc.sync.dma_start(whi_sb[:], whi_in.ap()[:, :])
        dpj_sb = const.tile([P, SUM_K], dt.float32)
        nc.sync.dma_start(dpj_sb[:], dpj_in.ap()[:, :])
        bnd_sb = const.tile([P, max(SUM_R, 1)], dt.float32)
        nc.sync.dma_start(bnd_sb[:], bnd_in.ap()[:, :])

        gh = ctx.enter_context(tc.tile_pool(name="gh", bufs=2))
        smp = ctx.enter_context(tc.tile_pool(name="smp", bufs=2))
        exp_ = ctx.enter_context(tc.tile_pool(name="exp", bufs=3))
        tsp = ctx.enter_context(tc.tile_pool(name="tsp", bufs=2))
        fin = ctx.enter_context(tc.tile_pool(name="fin", bufs=2))
        psGT = ctx.enter_context(tc.tile_pool(name="psGT", bufs=2, space="PSUM"))
        psSS = ctx.enter_context(tc.tile_pool(name="psSS", bufs=1, space="PSUM"))
        psAD = ctx.enter_context(tc.tile_pool(name="psAD", bufs=2, space="PSUM"))
        psU = ctx.enter_context(tc.tile_pool(name="psU", bufs=1, space="PSUM"))

        off_lo = [int(sum(K_LO[:j])) for j in range(POS + 1)]
        off_hi = [int(sum(K_HI[:j])) for j in range(POS + 1)]
        off_k = [int(sum(K_LO[:j]) + sum(K_HI[:j])) for j in range(POS + 1)]
        off_r = [0]
        for j in range(POS):
            off_r.append(off_r[-1] + R_LO[j] + R_HI[j])

        for g0 in range(0, POS, GRP):
            ng = min(GRP, POS - g0)
            slo = off_lo[g0 + ng] - off_lo[g0]
            shi = off_hi[g0 + ng] - off_hi[g0]
            ghlo = gh.tile([P, GLOMAX, 256], dt.bfloat16, tag="ghlo")
            nc.gpsimd.dma_gather(
                out_ap=ghlo[:, :slo, :], in_ap=lo_ap,
                idxs_ap=wlo_sb[:, off_lo[g0] * 8:(off_lo[g0] + slo) * 8],
                num_idxs=slo * 128, num_idxs_reg=slo * 128, elem_size=256,
                single_packet=False)
            ghhi = gh.tile([P, GHIMAX, 256], dt.bfloat16, tag="ghhi")
            nc.gpsimd.dma_gather(
                out_ap=ghhi[:, :shi, :], in_ap=hi_ap,
                idxs_ap=whi_sb[:, off_hi[g0] * 8:(off_hi[g0] + shi) * 8],
                num_idxs=shi * 128, num_idxs_reg=shi * 128, elem_size=256,
                single_packet=False)

            for j in range(g0, g0 + ng):
                KL = int(K_LO[j])
                KH = int(K_HI[j])
                K = KL + KH
                lbase = off_lo[j] - off_lo[g0]   # chunk offset inside ghlo
                hbase = off_hi[j] - off_hi[g0]

                # --- staircase a_dst per slot
                adp = psAD.tile([P, KMAX, 4], dt.float32, tag="adp", space="PSUM")
                rcol = off_r[j]
                for h, KHF, base in ((0, KL, 0), (1, KH, KL)):
                    R = math.ceil(KHF / RND) if KHF else 0
                    for r in range(R):
                        c0 = r * RND
                        nch = min(RND, KHF - c0)
                        sm = smp.tile([P, RND * 128], dt.float16, tag="sm")
                        nc.vector.tensor_scalar(
                            out=sm[:, :nch * 128], in0=iota2k[:, :nch * 128],
                            scalar1=bnd_sb[:, rcol:rcol + 1], scalar2=None,
                            op0=op.is_ge)
                        for jj in range(nch):
                            nc.tensor.matmul(
                                out=adp[:, base + c0 + jj, :],
                                lhsT=sm[:, jj * 128:(jj + 1) * 128],
                                rhs=delta4[:, j, :], start=True, stop=True)
                        rcol += 1

                # --- logits -> ex
                ash = tsp.tile([P, KMAX, 2], dt.float32, tag="ash")
                asl = tsp.tile([P, KMAX, 2], dt.float32, tag="asl")
                if KL:
                    nc.vector.tensor_scalar(
                        out=ash[:, :KL, :], in0=ghlo[:, lbase:lbase + KL, 128:130],
                        scalar1=0.0, scalar2=None, op0=op.add)
                    nc.vector.tensor_scalar(
                        out=asl[:, :KL, :], in0=ghlo[:, lbase:lbase + KL, 132:134],
                        scalar1=0.0, scalar2=None, op0=op.add)
                if KH:
                    nc.vector.tensor_scalar(
                        out=ash[:, KL:K, :], in0=ghhi[:, hbase:hbase + KH, 128:130],
                        scalar1=0.0, scalar2=None, op0=op.add)
                    nc.vector.tensor_scalar(
                        out=asl[:, KL:K, :], in0=ghhi[:, hbase:hbase + KH, 132:134],
                        scalar1=0.0, scalar2=None, op0=op.add)
                tsa = tsp.tile([P, KMAX, 2], dt.float32, tag="tsa")
                nc.vector.tensor_tensor(out=tsa[:, :K, :], in0=ash[:, :K, :],
                                        in1=asl[:, :K, :], op=op.add)
                t1 = tsp.tile([P, KMAX, 2], dt.float32, tag="t1")
                nc.vector.tensor_tensor(out=t1[:, :K, :], in0=tsa[:, :K, :],
                                        in1=adp[:, :K, 0:2], op=op.add)
                tsum = tsp.tile([P, KMAX, 2], dt.float32, tag="tsum")
                nc.vector.tensor_tensor(out=tsum[:, :K, :], in0=t1[:, :K, :],
                                        in1=adp[:, :K, 2:4], op=op.add)
                u02 = tsp.tile([P, KMAX, 2], dt.float32, tag="u02")
                nc.vector.tensor_scalar(out=u02[:, :K, :], in0=tsum[:, :K, :],
                                        scalar1=NEG_SLOPE, scalar2=None,
                                        op0=op.mult)
                lrt = tsp.tile([P, KMAX, 2], dt.float32, tag="lrt")
                nc.vector.tensor_tensor(out=lrt[:, :K, :], in0=tsum[:, :K, :],
                                        in1=u02[:, :K, :], op=op.max)
                ex = tsp.tile([P, KMAX, 2], dt.float32, tag="ex")
                nc.scalar.activation(out=ex[:, :K, :], in_=lrt[:, :K, :],
                                     func=act.Exp)

                # --- chunks: exm masks + gtt/ss accumulation
                gtt = psGT.tile([P, HEADS * C], dt.float32, tag="gtt", space="PSUM")
                ss0 = psSS.tile([P, 1], dt.float32, tag="ss0", space="PSUM")
                ss1 = psSS.tile([P, 1], dt.float32, tag="ss1", space="PSUM")
                for jc in range(K):
                    if jc < KL:
                        hgc = ghlo[:, lbase + jc, 0:128]
                    else:
                        hgc = ghhi[:, hbase + (jc - KL), 0:128]
                    st_ = jc == 0
                    sp_ = jc == K - 1
                    exm = exp_.tile([P, 2 * P], dt.bfloat16, tag="exm")
                    for hd in range(HEADS):
                        nc.vector.tensor_scalar(
                            out=exm[:, hd * P:(hd + 1) * P], in0=iota_row[:],
                            scalar1=dpj_sb[:, off_k[j] + jc:off_k[j] + jc + 1],
                            scalar2=ex[:, jc, hd:hd + 1],
                            op0=op.is_equal, op1=op.mult)
                    nc.tensor.matmul(out=gtt[:], lhsT=hgc, rhs=exm[:],
                                     start=st_, stop=sp_)
                    nc.tensor.matmul(out=ss0[:], lhsT=exm[:, 0:P],
                                     rhs=ones_bf[:], start=st_, stop=sp_)
                    nc.tensor.matmul(out=ss1[:], lhsT=exm[:, P:2 * P],
                                     rhs=ones_bf[:], start=st_, stop=sp_)

                # --- finalize position j
                rec = fin.tile([P, 2], dt.float32, tag="rec")
                nc.vector.reciprocal(out=rec[:, 0:1], in_=ss0[:])
                nc.vector.reciprocal(out=rec[:, 1:2], in_=ss1[:])
                gs = fin.tile([P, HEADS * C], dt.bfloat16, tag="gs")
                nc.scalar.copy(out=gs[:], in_=gtt[:])
                ot = fin.tile([P, HEADS * C], dt.bfloat16, tag="ot")
                for hd in range(HEADS):
                    u = psU.tile([P, C], dt.float32, tag="u", space="PSUM")
                    nc.tensor.matmul(out=u[:],
                                     lhsT=gs[:, hd * P:(hd + 1) * P],
                                     rhs=w_bf[:, hd * C:(hd + 1) * C],
                                     start=True, stop=True)
                    nc.scalar.mul(out=ot[:, hd * C:(hd + 1) * C],
                                  in_=u[:],
                                  mul=rec[:, hd:hd + 1])
                zt = fin.tile([P, HEADS * C], dt.bfloat16, tag="zt")
                nc.gpsimd.tensor_tensor(out=zt[:], in0=ot[:], in1=bias_bf[:],
                                        op=op.add)
                et = fin.tile([P, HEADS * C], dt.bfloat16, tag="et")
                nc.scalar.activation(out=et[:], in_=zt[:], func=act.Exp)
                mt = fin.tile([P, HEADS * C], dt.bfloat16, tag="mt")
                nc.vector.tensor_scalar(out=mt[:], in0=et[:], scalar1=1.0,
                                        scalar2=-1.0, op0=op.min, op1=op.add)
                rt = fin.tile([P, HEADS * C], dt.bfloat16, tag="rt")
                nc.vector.tensor_scalar(out=rt[:], in0=zt[:], scalar1=0.0,
                                        scalar2=None, op0=op.max)
                ob = fin.tile([P, HEADS * C], dt.bfloat16, tag="ob")
                nc.gpsimd.tensor_tensor(out=ob[:], in0=mt[:], in1=rt[:],
                                        op=op.add)
                obf = fin.tile([P, HEADS * C], dt.float32, tag="obf")
                nc.scalar.copy(out=obf[:], in_=ob[:])
                nc.sync.dma_start(out_t.ap()[j * P:(j + 1) * P, :], obf[:])

    nc.compile()
    return nc


def _get_program(K_LO, K_HI, R_LO, R_HI):
    key = (tuple(K_LO), tuple(K_HI))
    if key not in _CACHE:
        _CACHE[key] = _build(K_LO, K_HI, R_LO, R_HI)
    return _CACHE[key]


# ------------------------------------------------------------------- kernel
def kernel(h_node, edge_index, W, att_src, att_dst, bias):
    from concourse.bass_utils import run_bass_kernel_spmd

    h_node = np.asarray(h_node, dtype=np.float32)
    W = np.asarray(W, dtype=np.float32)
    att_src = np.asarray(att_src, dtype=np.float32)
    att_dst = np.asarray(att_dst, dtype=np.float32)
    bias = np.asarray(bias, dtype=np.float32).reshape(1, HEADS * C)

    pr = _prep(np.asarray(edge_index))
    nc = _get_program(pr["K_LO"], pr["K_HI"], pr["R_LO"], pr["R_HI"])

    hb = np.zeros((NROWS, 128), dtype=BF16)
    hb[:N] = h_node.astype(BF16)
    # p-major layout: row p*NTILE + t = node 128*t + p
    htab = np.ascontiguousarray(
        hb.reshape(NTILE, 128, 128).transpose(1, 0, 2)).reshape(128 * NTILE, 128)
    thh = np.zeros((NROWS, 256), dtype=BF16)
    thh[:, 0:128] = hb

    in_maps = []
    for c in range(NC_CORES):
        in_maps.append({
            "htab": htab, "th": thh, "w_in": W, "asrc_in": att_src,
            "adst_in": att_dst, "bias_in": bias,
            "wlo": pr["wlo"][c], "whi": pr["whi"][c], "dpj": pr["dpj"][c],
            "bnd": pr["bnd"][c], "bglo": pr["bglo"][c], "bghi": pr["bghi"][c],
            "bgm": pr["bgmask"][c],
        })
    res = run_bass_kernel_spmd(nc, in_maps, core_ids=list(range(NC_CORES)))
    out = np.zeros((N, HEADS * C), dtype=np.float32)
    gmap = pr["gmap"]
    for c in range(NC_CORES):
        o = res.results[c]["out"]
        for j in range(POS):
            gg = gmap[c, j]
            if gg < 0:
                continue
            lo_n = 128 * gg
            hi_n = min(128 * (gg + 1), N)
            out[lo_n:hi_n] = o[j * 128:j * 128 + (hi_n - lo_n)]
    return out


# revision 3
# speedup vs baseline: 1.0696x; 1.0209x over previous
"""GAT layer (PyG GATConv eval, 2 heads x 128, self-loops, ELU) on 8 trn2 cores.

v2 design (dst-block sharded, rank-dealt, bf16 datapath):
  - ht table [50048, 256] bf16 in DRAM: cols 0:128 = bf16(h) (host-uploaded),
    cols 128:136 = a_src/a_dst logits as bf16 hi/lo pairs (device-computed in
    phase A).  One 512B-row dma_gather per edge fetches h AND the src logits.
  - Global dst blocks (128 nodes) are dealt to (core, position) slots by edge
    count rank so per-position chunk counts are uniform across cores (SPMD).
  - Edges sorted by (core, pos, src<32768, dst_local); per (pos, half) padded
    to 128-slot chunks.  Self loops ride the edge stream.
  - Per-slot a_dst via "staircase" matmul: SM[d, slot] = (slot >= first slot of
    dst d's run), adp = SM^T @ delta(a_dst) reconstructs a_dst[dst(slot)]
    exactly (fp16 hi/lo deltas).  No one-hot broadcast machinery.
  - exm one-hot masks in bf16 (4x DVE mode); gtt/ss/U matmuls in bf16.
  - Finalize: normalize on Act engine, ELU via exp/min/max identity.
"""
import math
from contextlib import ExitStack

import numpy as np
import ml_dtypes

BF16 = ml_dtypes.bfloat16
FP16 = np.float16

HEADS = 2
C = 128
IN = 128
N = 50000
NC_CORES = 8
NTILE = math.ceil(N / 128)        # 391 tiles / global blocks
NROWS = NTILE * 128               # 50048 table rows
POS = math.ceil(NTILE / NC_CORES)  # 49 positions per core
LO = 32768                        # int16 gather index split
GRP = 4                           # positions per dma_gather call
RND = 16                          # max chunks per staircase round
NEG_SLOPE = 0.2
STAGE = 16                        # phase-A tiles per group

_CACHE = {}


# ----------------------------------------------------------------- host prep
def _prep(edge_index):
    src = np.concatenate([edge_index[0], np.arange(N)]).astype(np.int64)
    dst = np.concatenate([edge_index[1], np.arange(N)]).astype(np.int64)
    g = dst // 128
    dloc = dst % 128
    half = (src >= LO).astype(np.int64)

    sizes_g = np.bincount(g, minlength=NTILE)
    order_g = np.argsort(-sizes_g, kind="stable")
    gmap = np.full((NC_CORES, POS), -1, dtype=np.int64)
    for j in range(POS):
        for c in range(NC_CORES):
            r = NC_CORES * j + c
            if r < NTILE:
                gmap[c, j] = order_g[r]
    core_of = np.zeros(NTILE, dtype=np.int64)
    pos_of = np.zeros(NTILE, dtype=np.int64)
    for c in range(NC_CORES):
        for j in range(POS):
            gg = gmap[c, j]
            if gg >= 0:
                core_of[gg] = c
                pos_of[gg] = j

    ecore = core_of[g]
    epos = pos_of[g]
    key = ((ecore * POS + epos) * 2 + half) * 128 + dloc
    order = np.argsort(key, kind="stable")
    src_s = src[order]
    dloc_s = dloc[order]
    ecore_s = ecore[order]
    epos_s = epos[order]
    half_s = half[order]

    cnt = np.zeros((NC_CORES, POS, 2), dtype=np.int64)
    np.add.at(cnt, (ecore_s, epos_s, half_s), 1)
    K_LO = np.ceil(cnt[:, :, 0].max(axis=0) / 128).astype(int)  # [POS]
    K_HI = np.ceil(cnt[:, :, 1].max(axis=0) / 128).astype(int)
    K_ALL = K_LO + K_HI
    SUM_LO = int(K_LO.sum())
    SUM_HI = int(K_HI.sum())
    SUM_K = int(K_ALL.sum())
    # rounds per (pos, half)
    R_LO = [math.ceil(k / RND) if k else 0 for k in K_LO]
    R_HI = [math.ceil(k / RND) if k else 0 for k in K_HI]
    SUM_R = int(sum(R_LO) + sum(R_HI))

    # group starts (of edges) per (core, pos, half)
    starts = np.zeros(NC_CORES * POS * 2 + 1, dtype=np.int64)
    np.cumsum(np.bincount(
        (ecore_s * POS + epos_s) * 2 + half_s,
        minlength=NC_CORES * POS * 2), out=starts[1:])

    # per-core tables
    wlo = np.zeros((NC_CORES, 128, SUM_LO * 8), dtype=np.int16)
    whi = np.zeros((NC_CORES, 128, SUM_HI * 8), dtype=np.int16)
    dpj = np.full((NC_CORES, 128, SUM_K), 999.0, dtype=np.float32)
    bnd = np.zeros((NC_CORES, 128, max(SUM_R, 1)), dtype=np.float32)
    bglo = np.zeros((NC_CORES, 128, POS * 8), dtype=np.int16)
    bghi = np.zeros((NC_CORES, 128, POS * 8), dtype=np.int16)
    bgmask = np.zeros((NC_CORES, 128, POS * 2), dtype=np.uint8)

    def wrap16(idx):
        """idx [n] (n % 128 == 0) -> wrapped [128, n // 16] int16."""
        n = len(idx)
        sl = idx.reshape(n // 16, 16).T            # [16, n/16]
        return np.broadcast_to(sl[None, :, :], (8, 16, n // 16)).reshape(
            128, n // 16).astype(np.int16)

    for c in range(NC_CORES):
        off_lo = 0
        off_hi = 0
        off_k = 0
        off_r = 0
        for j in range(POS):
            gg = gmap[c, j]
            for h in range(2):
                K = int((K_LO if h == 0 else K_HI)[j])
                nt = K * 128
                if gg >= 0:
                    s0 = starts[(c * POS + j) * 2 + h]
                    s1 = starts[(c * POS + j) * 2 + h + 1]
                    srcs = src_s[s0:s1]
                    dls = dloc_s[s0:s1]
                else:
                    srcs = np.zeros(0, dtype=np.int64)
                    dls = np.zeros(0, dtype=np.int64)
                n = len(srcs)
                assert n <= nt
                idx = np.zeros(nt, dtype=np.int64)
                idx[:n] = srcs - (LO if h == 1 else 0)
                w = wrap16(idx)
                # slot i -> (chunk i//128, partition i%128)
                dv = np.full(nt, 999.0, dtype=np.float32)
                dv[:n] = dls
                dcol = dv.reshape(K, 128).T if K else np.zeros((128, 0), np.float32)
                # staircase boundaries per round
                first = np.searchsorted(dls, np.arange(128), side="left")  # [128]
                R = math.ceil(K / RND) if K else 0
                for r in range(R):
                    lo_c = r * RND * 128
                    ln = min(RND * 128, nt - lo_c)
                    b = np.clip(first - lo_c, 0, ln).astype(np.float32)
                    bnd[c, :, off_r + r] = b
                if h == 0:
                    wlo[c, :, off_lo * 8:(off_lo + K) * 8] = w
                    off_lo += K
                else:
                    whi[c, :, off_hi * 8:(off_hi + K) * 8] = w
                    off_hi += K
                dpj[c, :, off_k:off_k + K] = dcol
                off_k += K
                off_r += R
            # block gather (a_dst per block)
            if gg >= 0:
                nodes = 128 * gg + np.arange(128)
                if gg < 256:
                    bglo[c, :, j * 8:(j + 1) * 8] = wrap16(nodes)
                    bgmask[c, :, j * 2:(j + 1) * 2] = 1
                else:
                    bghi[c, :, j * 8:(j + 1) * 8] = wrap16(nodes - LO)
    return dict(gmap=gmap, K_LO=K_LO, K_HI=K_HI, R_LO=R_LO, R_HI=R_HI,
                SUM_LO=SUM_LO, SUM_HI=SUM_HI, SUM_K=SUM_K, SUM_R=SUM_R,
                wlo=wlo, whi=whi, dpj=dpj, bnd=bnd,
                bglo=bglo, bghi=bghi, bgmask=bgmask)


# ------------------------------------------------------------ device program
def _build(K_LO, K_HI, R_LO, R_HI):
    import concourse.bacc as bacc
    import concourse.bass as bass
    import concourse.mybir as mybir
    import concourse.tile as tile
    from concourse.masks import make_identity

    dt = mybir.dt
    op = mybir.AluOpType
    act = mybir.ActivationFunctionType
    P = 128
    SUM_LO = int(sum(K_LO))
    SUM_HI = int(sum(K_HI))
    SUM_K = SUM_LO + SUM_HI
    SUM_R = int(sum(R_LO) + sum(R_HI))
    KMAX = int(max(K_LO[j] + K_HI[j] for j in range(POS)))
    # gather groups: 4-wide, with a small tail so the last gather's compute
    # doesn't leave a long serial epilogue
    GRPS = []
    rem = POS
    while rem > 5:
        GRPS.append(GRP)
        rem -= GRP
    while rem > 0:
        GRPS.append(min(2, rem) if rem > 1 else 1)
        rem -= GRPS[-1]
    g_starts = [int(sum(GRPS[:i])) for i in range(len(GRPS))]
    GLOMAX = max(int(sum(K_LO[g0:g0 + ng])) for g0, ng in zip(g_starts, GRPS))
    GHIMAX = max(int(sum(K_HI[g0:g0 + ng])) for g0, ng in zip(g_starts, GRPS))

    nc = bacc.Bacc("TRN2", target_bir_lowering=False, debug=False,
                   num_devices=NC_CORES)
    htab = nc.dram_tensor("htab", [128 * NTILE, 128], dt.bfloat16,
                          kind="ExternalInput")       # p-major bf16 h
    th = nc.dram_tensor("th", [NROWS, 256], dt.bfloat16,
                        kind="ExternalInput")         # node-major gather table
    w_in = nc.dram_tensor("w_in", [IN, HEADS * C], dt.float32, kind="ExternalInput")
    asrc_in = nc.dram_tensor("asrc_in", [HEADS, C], dt.float32, kind="ExternalInput")
    adst_in = nc.dram_tensor("adst_in", [HEADS, C], dt.float32, kind="ExternalInput")
    bias_in = nc.dram_tensor("bias_in", [1, HEADS * C], dt.float32, kind="ExternalInput")
    wlo_in = nc.dram_tensor("wlo", [128, SUM_LO * 8], dt.int16, kind="ExternalInput")
    whi_in = nc.dram_tensor("whi", [128, SUM_HI * 8], dt.int16, kind="ExternalInput")
    dpj_in = nc.dram_tensor("dpj", [128, SUM_K], dt.float32, kind="ExternalInput")
    bnd_in = nc.dram_tensor("bnd", [128, max(SUM_R, 1)], dt.float32, kind="ExternalInput")
    bglo_in = nc.dram_tensor("bglo", [128, POS * 8], dt.int16, kind="ExternalInput")
    bghi_in = nc.dram_tensor("bghi", [128, POS * 8], dt.int16, kind="ExternalInput")
    bgm_in = nc.dram_tensor("bgm", [128, POS * 2], dt.uint8, kind="ExternalInput")
    out_t = nc.dram_tensor("out", [POS * 128, HEADS * C], dt.float32,
                           kind="ExternalOutput")

    with tile.TileContext(nc) as tc, ExitStack() as ctx:
        const = ctx.enter_context(tc.tile_pool(name="const", bufs=1))

        # ---- constants
        ident_bf = const.tile([P, P], dt.bfloat16)
        make_identity(nc, ident_bf[:])
        iota_row = const.tile([P, P], dt.bfloat16)
        nc.gpsimd.iota(iota_row[:], pattern=[[1, P]], base=0, channel_multiplier=0,
                       allow_small_or_imprecise_dtypes=True)
        iota2k = const.tile([P, RND * 128], dt.float16)
        nc.gpsimd.iota(iota2k[:], pattern=[[1, RND * 128]], base=0,
                       channel_multiplier=0, allow_small_or_imprecise_dtypes=True)
        ones_bf = const.tile([P, 1], dt.bfloat16)
        nc.gpsimd.memset(ones_bf[:], 1.0)
        iota_cp1 = const.tile([P, 1], dt.float32)
        nc.gpsimd.iota(iota_cp1[:], pattern=[[0, 1]], base=1, channel_multiplier=1,
                       allow_small_or_imprecise_dtypes=True)
        shiftmat = const.tile([P, P], dt.float32)
        nc.vector.tensor_scalar(out=shiftmat[:], in0=iota_row[:],
                                scalar1=iota_cp1[:], scalar2=None, op0=op.is_equal)
        w_sb = const.tile([P, HEADS * C], dt.float32)
        nc.sync.dma_start(w_sb[:], w_in.ap()[:, :])
        w_bf = const.tile([P, HEADS * C], dt.bfloat16)
        nc.vector.tensor_scalar(out=w_bf[:], in0=w_sb[:], scalar1=0.0,
                                scalar2=None, op0=op.add)
        bias_bf = const.tile([P, HEADS * C], dt.bfloat16)
        bias_f32 = const.tile([P, HEADS * C], dt.float32)
        nc.sync.dma_start(bias_f32[:], bass.AP(bias_in, 0, [[0, P], [1, HEADS * C]]))
        nc.vector.tensor_scalar(out=bias_bf[:], in0=bias_f32[:], scalar1=0.0,
                                scalar2=None, op0=op.add)

        # wa4[k, i] = sum_c W[k, h*C+c]*att[h, c]; cols: as0 as1 ad0 ad1
        wa4 = const.tile([P, 4], dt.float32)
        wa4hl = const.tile([P, 8], dt.bfloat16)   # [hi0..hi3, lo0..lo3]
        with tc.tile_pool(name="watmp", bufs=2) as tmp_pool:
            for jat, attt in enumerate((asrc_in, adst_in)):
                for hd in range(HEADS):
                    abc = tmp_pool.tile([P, C], dt.float32, tag="abc")
                    nc.sync.dma_start(abc[:], bass.AP(attt, hd * C, [[0, P], [1, C]]))
                    t = tmp_pool.tile([P, C], dt.float32, tag="t")
                    nc.vector.tensor_tensor(
                        out=t[:], in0=w_sb[:, hd * C:(hd + 1) * C],
                        in1=abc[:], op=op.mult)
                    nc.vector.tensor_reduce(
                        out=wa4[:, 2 * jat + hd:2 * jat + hd + 1], in_=t[:],
                        axis=mybir.AxisListType.X, op=op.add)
            nc.vector.tensor_scalar(out=wa4hl[:, 0:4], in0=wa4[:], scalar1=0.0,
                                    scalar2=None, op0=op.add)
            hic = tmp_pool.tile([P, 4], dt.float32, tag="hic")
            nc.vector.tensor_scalar(out=hic[:], in0=wa4hl[:, 0:4], scalar1=0.0,
                                    scalar2=None, op0=op.add)
            lo32 = tmp_pool.tile([P, 4], dt.float32, tag="lo32")
            nc.vector.tensor_tensor(out=lo32[:], in0=wa4[:], in1=hic[:],
                                    op=op.subtract)
            nc.vector.tensor_scalar(out=wa4hl[:, 4:8], in0=lo32[:], scalar1=0.0,
                                    scalar2=None, op0=op.add)

        # ---- phase A: write a_src/a_dst hi/lo into th[:, 128:136]
        ctxA = ExitStack()
        sbA = ctxA.enter_context(tc.tile_pool(name="sbA", bufs=2))
        psT = ctxA.enter_context(tc.tile_pool(name="psT", bufs=2, space="PSUM"))
        psA8 = ctxA.enter_context(tc.tile_pool(name="psA8", bufs=2, space="PSUM"))
        stgA = ctxA.enter_context(tc.tile_pool(name="stgA", bufs=2))

        for t0 in range(0, NTILE, STAGE):
            nst = min(STAGE, NTILE - t0)
            htile = sbA.tile([P, STAGE, 128], dt.bfloat16, tag="htile")
            nc.sync.dma_start(
                htile[:, :nst, :],
                bass.AP(htab, t0 * 128, [[NTILE * 128, P], [128, nst], [1, 128]]))
            tp = psT.tile([P, STAGE * 128], dt.bfloat16, tag="tp", space="PSUM")
            for gi in range(nst):
                nc.tensor.transpose(out=tp[:, gi * 128:(gi + 1) * 128],
                                    in_=htile[:, gi, :], identity=ident_bf[:])
            hT = sbA.tile([P, STAGE * 128], dt.bfloat16, tag="hT")
            nc.vector.tensor_scalar(out=hT[:, :nst * 128], in0=tp[:, :nst * 128],
                                    scalar1=0.0, scalar2=None, op0=op.add)
            a8 = psA8.tile([P, STAGE, 8], dt.float32, tag="a8", space="PSUM")
            for gi in range(nst):
                nc.tensor.matmul(out=a8[:, gi, :],
                                 lhsT=hT[:, gi * 128:(gi + 1) * 128],
                                 rhs=wa4hl[:], start=True, stop=True)
            a8s = stgA.tile([P, STAGE, 8], dt.float32, tag="a8s")
            nc.vector.tensor_scalar(out=a8s[:, :nst, :], in0=a8[:, :nst, :],
                                    scalar1=0.0, scalar2=None, op0=op.add)
            a4g = stgA.tile([P, STAGE, 4], dt.float32, tag="a4g")
            nc.vector.tensor_tensor(out=a4g[:, :nst, :], in0=a8s[:, :nst, 0:4],
                                    in1=a8s[:, :nst, 4:8], op=op.add)
            # th cols 128:136 hold raw fp32 bits of [as0 as1 ad0 ad1]
            nc.scalar.dma_start(
                bass.AP(th, (128 * t0) * 256 + 128,
                        [[256, P], [128 * 256, nst], [1, 8]]),
                a4g[:, :nst, :].bitcast(dt.bfloat16))
        ctxA.close()

        # ---- block gather: a_dst hi/lo per (pos, dst_local) + fp16 deltas
        bgp = ExitStack()
        bgpool = bgp.enter_context(tc.tile_pool(name="bgpool", bufs=1))
        lo_ap = bass.AP(th, 0, [[256, LO], [1, 256]])
        hi_ap = bass.AP(th, LO * 256, [[256, NROWS - LO], [1, 256]])
        bgidx = bgpool.tile([P, POS * 8], dt.int16, tag="bgidx")
        nc.sync.dma_start(bgidx[:], bglo_in.ap()[:, :])
        bgidx2 = bgpool.tile([P, POS * 8], dt.int16, tag="bgidx2")
        nc.sync.dma_start(bgidx2[:], bghi_in.ap()[:, :])
        bgA = bgpool.tile([P, POS, 256], dt.bfloat16, tag="bgA")
        nc.gpsimd.dma_gather(
            out_ap=bgA[:], in_ap=lo_ap, idxs_ap=bgidx[:],
            num_idxs=POS * 128, num_idxs_reg=POS * 128, elem_size=256,
            single_packet=False)
        bgB = bgpool.tile([P, POS, 256], dt.bfloat16, tag="bgB")
        nc.gpsimd.dma_gather(
            out_ap=bgB[:], in_ap=hi_ap, idxs_ap=bgidx2[:],
            num_idxs=POS * 128, num_idxs_reg=POS * 128, elem_size=256,
            single_packet=False)
        bgm = bgpool.tile([P, POS, 2], dt.uint8, tag="bgm")
        nc.sync.dma_start(bgm[:], bgm_in.ap()[:, :])
        # a_dst per block: th cols 132:136 hold fp32 bits of [ad0 ad1]; A/B merge
        adf = bgpool.tile([P, POS, 2], dt.float32, tag="adf")
        nc.vector.tensor_copy(out=adf[:], in_=bgB[:, :, 132:136].bitcast(dt.float32))
        nc.vector.copy_predicated(out=adf[:], mask=bgm[:],
                                  data=bgA[:, :, 132:136].bitcast(dt.float32))
        # shifted[d] = a_dst[d-1] via shift-matrix matmul (exact in fp32 psum)
        psBG = bgp.enter_context(tc.tile_pool(name="psBG", bufs=1, space="PSUM"))
        sh4 = psBG.tile([P, POS, 2], dt.float32, tag="sh4", space="PSUM")
        nc.tensor.matmul(out=sh4[:], lhsT=shiftmat[:], rhs=adf[:],
                         start=True, stop=True)
        shf = bgpool.tile([P, POS, 2], dt.float32, tag="shf")
        nc.vector.tensor_scalar(out=shf[:], in0=sh4[:], scalar1=0.0,
                                scalar2=None, op0=op.add)
        dlt = bgpool.tile([P, POS, 2], dt.float32, tag="dlt")
        nc.vector.tensor_tensor(out=dlt[:], in0=adf[:], in1=shf[:],
                                op=op.subtract)
        delta4 = const.tile([P, POS, 4], dt.float16)
        nc.vector.tensor_scalar(out=delta4[:, :, 0:2], in0=dlt[:],
                                scalar1=0.0, scalar2=None, op0=op.add)
        dhc = bgpool.tile([P, POS, 2], dt.float32, tag="dhc")
        nc.vector.tensor_scalar(out=dhc[:], in0=delta4[:, :, 0:2],
                                scalar1=0.0, scalar2=None, op0=op.add)
        dlo = bgpool.tile([P, POS, 2], dt.float32, tag="dlo")
        nc.vector.tensor_tensor(out=dlo[:], in0=dlt[:], in1=dhc[:],
                                op=op.subtract)
        nc.vector.tensor_scalar(out=delta4[:, :, 2:4], in0=dlo[:],
                                scalar1=0.0, scalar2=None, op0=op.add)
        bgp.close()

        # ---- phase B preloads
        wlo_sb = const.tile([P, SUM_LO * 8], dt.int16)
        nc.sync.dma_start(wlo_sb[:], wlo_in.ap()[:, :])
        whi_sb = const.tile([P, SUM_HI * 8], dt.int16)
        nc.sync.dma_start(whi_sb[:], whi_in.ap()[:, :])
        dpj_sb = const.tile([P, SUM_K], dt.float32)
        nc.sync.dma_start(dpj_sb[:], dpj_in.ap()[:, :])
        bnd_sb = const.tile([P, max(SUM_R, 1)], dt.float32)
        nc.sync.dma_start(bnd_sb[:], bnd_in.ap()[:, :])

        gh = ctx.enter_context(tc.tile_pool(name="gh", bufs=2))
        smp = ctx.enter_context(tc.tile_pool(name="smp", bufs=2))
        exp_ = ctx.enter_context(tc.tile_pool(name="exp", bufs=3))
        tsp = ctx.enter_context(tc.tile_pool(name="tsp", bufs=2))
        fin = ctx.enter_context(tc.tile_pool(name="fin", bufs=2))
        psGT = ctx.enter_context(tc.tile_pool(name="psGT", bufs=2, space="PSUM"))
        psSS = ctx.enter_context(tc.tile_pool(name="psSS", bufs=1, space="PSUM"))
        psAD = ctx.enter_context(tc.tile_pool(name="psAD", bufs=2, space="PSUM"))
        psU = ctx.enter_context(tc.tile_pool(name="psU", bufs=1, space="PSUM"))

        off_lo = [int(sum(K_LO[:j])) for j in range(POS + 1)]
        off_hi = [int(sum(K_HI[:j])) for j in range(POS + 1)]
        off_k = [int(sum(K_LO[:j]) + sum(K_HI[:j])) for j in range(POS + 1)]
        off_r = [0]
        for j in range(POS):
            off_r.append(off_r[-1] + R_LO[j] + R_HI[j])

        for g0, ng in zip(g_starts, GRPS):
            slo = off_lo[g0 + ng] - off_lo[g0]
            shi = off_hi[g0 + ng] - off_hi[g0]
            ghlo = gh.tile([P, GLOMAX, 256], dt.bfloat16, tag="ghlo")
            nc.gpsimd.dma_gather(
                out_ap=ghlo[:, :slo, :], in_ap=lo_ap,
                idxs_ap=wlo_sb[:, off_lo[g0] * 8:(off_lo[g0] + slo) * 8],
                num_idxs=slo * 128, num_idxs_reg=slo * 128, elem_size=256,
                single_packet=False)
            ghhi = gh.tile([P, GHIMAX, 256], dt.bfloat16, tag="ghhi")
            nc.gpsimd.dma_gather(
                out_ap=ghhi[:, :shi, :], in_ap=hi_ap,
                idxs_ap=whi_sb[:, off_hi[g0] * 8:(off_hi[g0] + shi) * 8],
                num_idxs=shi * 128, num_idxs_reg=shi * 128, elem_size=256,
                single_packet=False)

            for j in range(g0, g0 + ng):
                KL = int(K_LO[j])
                KH = int(K_HI[j])
                K = KL + KH
                lbase = off_lo[j] - off_lo[g0]   # chunk offset inside ghlo
                hbase = off_hi[j] - off_hi[g0]

                # --- staircase a_dst per slot
                adp = psAD.tile([P, KMAX, 4], dt.float32, tag="adp", space="PSUM")
                rcol = off_r[j]
                for h, KHF, base in ((0, KL, 0), (1, KH, KL)):
                    R = math.ceil(KHF / RND) if KHF else 0
                    for r in range(R):
                        c0 = r * RND
                        nch = min(RND, KHF - c0)
                        sm = smp.tile([P, RND * 128], dt.float16, tag="sm")
                        nc.vector.tensor_scalar(
                            out=sm[:, :nch * 128], in0=iota2k[:, :nch * 128],
                            scalar1=bnd_sb[:, rcol:rcol + 1], scalar2=None,
                            op0=op.is_ge)
                        for jj in range(nch):
                            nc.tensor.matmul(
                                out=adp[:, base + c0 + jj, :],
                                lhsT=sm[:, jj * 128:(jj + 1) * 128],
                                rhs=delta4[:, j, :], start=True, stop=True)
                        rcol += 1

                # --- logits -> ex  (th cols 128:132 = fp32 bits of [as0 as1])
                t1 = tsp.tile([P, KMAX, 2], dt.float32, tag="t1")
                if KL:
                    nc.vector.tensor_tensor(
                        out=t1[:, :KL, :],
                        in0=ghlo[:, lbase:lbase + KL, 128:132].bitcast(dt.float32),
                        in1=adp[:, :KL, 0:2], op=op.add)
                if KH:
                    nc.vector.tensor_tensor(
                        out=t1[:, KL:K, :],
                        in0=ghhi[:, hbase:hbase + KH, 128:132].bitcast(dt.float32),
                        in1=adp[:, KL:K, 0:2], op=op.add)
                tsum = tsp.tile([P, KMAX, 2], dt.float32, tag="tsum")
                nc.vector.tensor_tensor(out=tsum[:, :K, :], in0=t1[:, :K, :],
                                        in1=adp[:, :K, 2:4], op=op.add)
                u02 = tsp.tile([P, KMAX, 2], dt.float32, tag="u02")
                nc.vector.tensor_scalar(out=u02[:, :K, :], in0=tsum[:, :K, :],
                                        scalar1=NEG_SLOPE, scalar2=None,
                                        op0=op.mult)
                lrt = tsp.tile([P, KMAX, 2], dt.float32, tag="lrt")
                nc.vector.tensor_tensor(out=lrt[:, :K, :], in0=tsum[:, :K, :],
                                        in1=u02[:, :K, :], op=op.max)
                ex = tsp.tile([P, KMAX, 2], dt.float32, tag="ex")
                nc.scalar.activation(out=ex[:, :K, :], in_=lrt[:, :K, :],
                                     func=act.Exp)

                # --- chunks: exm masks + gtt/ss accumulation
                gtt = psGT.tile([P, HEADS * C], dt.float32, tag="gtt", space="PSUM")
                ss0 = psSS.tile([P, 1], dt.float32, tag="ss0", space="PSUM")
                ss1 = psSS.tile([P, 1], dt.float32, tag="ss1", space="PSUM")
                for jc in range(K):
                    if jc < KL:
                        hgc = ghlo[:, lbase + jc, 0:128]
                    else:
                        hgc = ghhi[:, hbase + (jc - KL), 0:128]
                    st_ = jc == 0
                    sp_ = jc == K - 1
                    exm = exp_.tile([P, 2 * P], dt.bfloat16, tag="exm")
                    for hd in range(HEADS):
                        nc.vector.tensor_scalar(
                            out=exm[:, hd * P:(hd + 1) * P], in0=iota_row[:],
                            scalar1=dpj_sb[:, off_k[j] + jc:off_k[j] + jc + 1],
                            scalar2=ex[:, jc, hd:hd + 1],
                            op0=op.is_equal, op1=op.mult)
                    nc.tensor.matmul(out=gtt[:], lhsT=hgc, rhs=exm[:],
                                     start=st_, stop=sp_)
                    nc.tensor.matmul(out=ss0[:], lhsT=exm[:, 0:P],
                                     rhs=ones_bf[:], start=st_, stop=sp_)
                    nc.tensor.matmul(out=ss1[:], lhsT=exm[:, P:2 * P],
                                     rhs=ones_bf[:], start=st_, stop=sp_)

                # --- finalize position j
                rec = fin.tile([P, 2], dt.float32, tag="rec")
                nc.vector.reciprocal(out=rec[:, 0:1], in_=ss0[:])
                nc.vector.reciprocal(out=rec[:, 1:2], in_=ss1[:])
                gs = fin.tile([P, HEADS * C], dt.bfloat16, tag="gs")
                nc.scalar.copy(out=gs[:], in_=gtt[:])
                ot = fin.tile([P, HEADS * C], dt.bfloat16, tag="ot")
                for hd in range(HEADS):
                    u = psU.tile([P, C], dt.float32, tag="u", space="PSUM")
                    nc.tensor.matmul(out=u[:],
                                     lhsT=gs[:, hd * P:(hd + 1) * P],
                                     rhs=w_bf[:, hd * C:(hd + 1) * C],
                                     start=True, stop=True)
                    nc.scalar.mul(out=ot[:, hd * C:(hd + 1) * C],
                                  in_=u[:],
                                  mul=rec[:, hd:hd + 1])
                zt = fin.tile([P, HEADS * C], dt.bfloat16, tag="zt")
                nc.gpsimd.tensor_tensor(out=zt[:], in0=ot[:], in1=bias_bf[:],
                                        op=op.add)
                et = fin.tile([P, HEADS * C], dt.bfloat16, tag="et")
                nc.scalar.activation(out=et[:], in_=zt[:], func=act.Exp)
                mt = fin.tile([P, HEADS * C], dt.bfloat16, tag="mt")
                nc.vector.tensor_scalar(out=mt[:], in0=et[:], scalar1=1.0,
                                        scalar2=-1.0, op0=op.min, op1=op.add)
                rt = fin.tile([P, HEADS * C], dt.bfloat16, tag="rt")
                nc.vector.tensor_scalar(out=rt[:], in0=zt[:], scalar1=0.0,
                                        scalar2=None, op0=op.max)
                ob = fin.tile([P, HEADS * C], dt.bfloat16, tag="ob")
                nc.gpsimd.tensor_tensor(out=ob[:], in0=mt[:], in1=rt[:],
                                        op=op.add)
                obf = fin.tile([P, HEADS * C], dt.float32, tag="obf")
                nc.scalar.copy(out=obf[:], in_=ob[:])
                nc.sync.dma_start(out_t.ap()[j * P:(j + 1) * P, :], obf[:])

    nc.compile()
    return nc


def _get_program(K_LO, K_HI, R_LO, R_HI):
    key = (tuple(K_LO), tuple(K_HI))
    if key not in _CACHE:
        _CACHE[key] = _build(K_LO, K_HI, R_LO, R_HI)
    return _CACHE[key]


# ------------------------------------------------------------------- kernel
def kernel(h_node, edge_index, W, att_src, att_dst, bias):
    from concourse.bass_utils import run_bass_kernel_spmd

    h_node = np.asarray(h_node, dtype=np.float32)
    W = np.asarray(W, dtype=np.float32)
    att_src = np.asarray(att_src, dtype=np.float32)
    att_dst = np.asarray(att_dst, dtype=np.float32)
    bias = np.asarray(bias, dtype=np.float32).reshape(1, HEADS * C)

    pr = _prep(np.asarray(edge_index))
    nc = _get_program(pr["K_LO"], pr["K_HI"], pr["R_LO"], pr["R_HI"])

    hb = np.zeros((NROWS, 128), dtype=BF16)
    hb[:N] = h_node.astype(BF16)
    # p-major layout: row p*NTILE + t = node 128*t + p
    htab = np.ascontiguousarray(
        hb.reshape(NTILE, 128, 128).transpose(1, 0, 2)).reshape(128 * NTILE, 128)
    thh = np.zeros((NROWS, 256), dtype=BF16)
    thh[:, 0:128] = hb

    in_maps = []
    for c in range(NC_CORES):
        in_maps.append({
            "htab": htab, "th": thh, "w_in": W, "asrc_in": att_src,
            "adst_in": att_dst, "bias_in": bias,
            "wlo": pr["wlo"][c], "whi": pr["whi"][c], "dpj": pr["dpj"][c],
            "bnd": pr["bnd"][c], "bglo": pr["bglo"][c], "bghi": pr["bghi"][c],
            "bgm": pr["bgmask"][c],
        })
    res = run_bass_kernel_spmd(nc, in_maps, core_ids=list(range(NC_CORES)))
    out = np.zeros((N, HEADS * C), dtype=np.float32)
    gmap = pr["gmap"]
    for c in range(NC_CORES):
        o = res.results[c]["out"]
        for j in range(POS):
            gg = gmap[c, j]
            if gg < 0:
                continue
            lo_n = 128 * gg
            hi_n = min(128 * (gg + 1), N)
            out[lo_n:hi_n] = o[j * 128:j * 128 + (hi_n - lo_n)]
    return out


# revision 5
# speedup vs baseline: 1.0800x; 1.0098x over previous
"""GAT layer (PyG GATConv eval, 2 heads x 128, self-loops, ELU) on 8 trn2 cores.

v2 design (dst-block sharded, rank-dealt, bf16 datapath):
  - ht table [50048, 256] bf16 in DRAM: cols 0:128 = bf16(h) (host-uploaded),
    cols 128:136 = a_src/a_dst logits as bf16 hi/lo pairs (device-computed in
    phase A).  One 512B-row dma_gather per edge fetches h AND the src logits.
  - Global dst blocks (128 nodes) are dealt to (core, position) slots by edge
    count rank so per-position chunk counts are uniform across cores (SPMD).
  - Edges sorted by (core, pos, src<32768, dst_local); per (pos, half) padded
    to 128-slot chunks.  Self loops ride the edge stream.
  - Per-slot a_dst via "staircase" matmul: SM[d, slot] = (slot >= first slot of
    dst d's run), adp = SM^T @ delta(a_dst) reconstructs a_dst[dst(slot)]
    exactly (fp16 hi/lo deltas).  No one-hot broadcast machinery.
  - exm one-hot masks in bf16 (4x DVE mode); gtt/ss/U matmuls in bf16.
  - Finalize: normalize on Act engine, ELU via exp/min/max identity.
"""
import math
from contextlib import ExitStack

import numpy as np
import ml_dtypes

BF16 = ml_dtypes.bfloat16
FP16 = np.float16

HEADS = 2
C = 128
IN = 128
N = 50000
NC_CORES = 8
NTILE = math.ceil(N / 128)        # 391 tiles / global blocks
NROWS = NTILE * 128               # 50048 table rows
POS = math.ceil(NTILE / NC_CORES)  # 49 positions per core
LO = 32768                        # int16 gather index split
GRP = 4                           # positions per dma_gather call
RND = 16                          # max chunks per staircase round
NEG_SLOPE = 0.2
STAGE = 16                        # phase-A tiles per group

_CACHE = {}


# ----------------------------------------------------------------- host prep
def _prep(edge_index):
    src = np.concatenate([edge_index[0], np.arange(N)]).astype(np.int64)
    dst = np.concatenate([edge_index[1], np.arange(N)]).astype(np.int64)
    g = dst // 128
    dloc = dst % 128
    half = (src >= LO).astype(np.int64)

    sizes_g = np.bincount(g, minlength=NTILE)
    order_g = np.argsort(-sizes_g, kind="stable")
    gmap = np.full((NC_CORES, POS), -1, dtype=np.int64)
    for j in range(POS):
        for c in range(NC_CORES):
            r = NC_CORES * j + c
            if r < NTILE:
                gmap[c, j] = order_g[r]
    core_of = np.zeros(NTILE, dtype=np.int64)
    pos_of = np.zeros(NTILE, dtype=np.int64)
    for c in range(NC_CORES):
        for j in range(POS):
            gg = gmap[c, j]
            if gg >= 0:
                core_of[gg] = c
                pos_of[gg] = j

    ecore = core_of[g]
    epos = pos_of[g]
    key = ((ecore * POS + epos) * 2 + half) * 128 + dloc
    order = np.argsort(key, kind="stable")
    src_s = src[order]
    dloc_s = dloc[order]
    ecore_s = ecore[order]
    epos_s = epos[order]
    half_s = half[order]

    cnt = np.zeros((NC_CORES, POS, 2), dtype=np.int64)
    np.add.at(cnt, (ecore_s, epos_s, half_s), 1)
    K_LO = np.ceil(cnt[:, :, 0].max(axis=0) / 128).astype(int)  # [POS]
    K_HI = np.ceil(cnt[:, :, 1].max(axis=0) / 128).astype(int)
    K_ALL = K_LO + K_HI
    SUM_LO = int(K_LO.sum())
    SUM_HI = int(K_HI.sum())
    SUM_K = int(K_ALL.sum())
    # rounds per (pos, half)
    R_LO = [math.ceil(k / RND) if k else 0 for k in K_LO]
    R_HI = [math.ceil(k / RND) if k else 0 for k in K_HI]
    SUM_R = int(sum(R_LO) + sum(R_HI))

    # group starts (of edges) per (core, pos, half)
    starts = np.zeros(NC_CORES * POS * 2 + 1, dtype=np.int64)
    np.cumsum(np.bincount(
        (ecore_s * POS + epos_s) * 2 + half_s,
        minlength=NC_CORES * POS * 2), out=starts[1:])

    # per-core tables
    wlo = np.zeros((NC_CORES, 128, SUM_LO * 8), dtype=np.int16)
    whi = np.zeros((NC_CORES, 128, SUM_HI * 8), dtype=np.int16)
    dpj = np.full((NC_CORES, 128, SUM_K), 999.0, dtype=np.float32)
    bnd = np.zeros((NC_CORES, 128, max(SUM_R, 1)), dtype=np.float32)
    bglo = np.zeros((NC_CORES, 128, POS * 8), dtype=np.int16)
    bghi = np.zeros((NC_CORES, 128, POS * 8), dtype=np.int16)
    bgmask = np.zeros((NC_CORES, 128, POS * 2), dtype=np.uint8)

    def wrap16(idx):
        """idx [n] (n % 128 == 0) -> wrapped [128, n // 16] int16."""
        n = len(idx)
        sl = idx.reshape(n // 16, 16).T            # [16, n/16]
        return np.broadcast_to(sl[None, :, :], (8, 16, n // 16)).reshape(
            128, n // 16).astype(np.int16)

    for c in range(NC_CORES):
        off_lo = 0
        off_hi = 0
        off_k = 0
        off_r = 0
        for j in range(POS):
            gg = gmap[c, j]
            for h in range(2):
                K = int((K_LO if h == 0 else K_HI)[j])
                nt = K * 128
                if gg >= 0:
                    s0 = starts[(c * POS + j) * 2 + h]
                    s1 = starts[(c * POS + j) * 2 + h + 1]
                    srcs = src_s[s0:s1]
                    dls = dloc_s[s0:s1]
                else:
                    srcs = np.zeros(0, dtype=np.int64)
                    dls = np.zeros(0, dtype=np.int64)
                n = len(srcs)
                assert n <= nt
                idx = np.zeros(nt, dtype=np.int64)
                idx[:n] = srcs - (LO if h == 1 else 0)
                w = wrap16(idx)
                # slot i -> (chunk i//128, partition i%128)
                dv = np.full(nt, 999.0, dtype=np.float32)
                dv[:n] = dls
                dcol = dv.reshape(K, 128).T if K else np.zeros((128, 0), np.float32)
                # staircase boundaries per round
                first = np.searchsorted(dls, np.arange(128), side="left")  # [128]
                R = math.ceil(K / RND) if K else 0
                for r in range(R):
                    lo_c = r * RND * 128
                    ln = min(RND * 128, nt - lo_c)
                    b = np.clip(first - lo_c, 0, ln).astype(np.float32)
                    bnd[c, :, off_r + r] = b
                if h == 0:
                    wlo[c, :, off_lo * 8:(off_lo + K) * 8] = w
                    off_lo += K
                else:
                    whi[c, :, off_hi * 8:(off_hi + K) * 8] = w
                    off_hi += K
                dpj[c, :, off_k:off_k + K] = dcol
                off_k += K
                off_r += R
            # block gather (a_dst per block)
            if gg >= 0:
                nodes = 128 * gg + np.arange(128)
                if gg < 256:
                    bglo[c, :, j * 8:(j + 1) * 8] = wrap16(nodes)
                    bgmask[c, :, j * 2:(j + 1) * 2] = 1
                else:
                    bghi[c, :, j * 8:(j + 1) * 8] = wrap16(nodes - LO)
    return dict(gmap=gmap, K_LO=K_LO, K_HI=K_HI, R_LO=R_LO, R_HI=R_HI,
                SUM_LO=SUM_LO, SUM_HI=SUM_HI, SUM_K=SUM_K, SUM_R=SUM_R,
                wlo=wlo, whi=whi, dpj=dpj, bnd=bnd,
                bglo=bglo, bghi=bghi, bgmask=bgmask)


# ------------------------------------------------------------ device program
def _build(K_LO, K_HI, R_LO, R_HI):
    import concourse.bacc as bacc
    import concourse.bass as bass
    import concourse.mybir as mybir
    import concourse.tile as tile
    from concourse.masks import make_identity

    dt = mybir.dt
    op = mybir.AluOpType
    act = mybir.ActivationFunctionType
    P = 128
    SUM_LO = int(sum(K_LO))
    SUM_HI = int(sum(K_HI))
    SUM_K = SUM_LO + SUM_HI
    SUM_R = int(sum(R_LO) + sum(R_HI))
    KMAX = int(max(K_LO[j] + K_HI[j] for j in range(POS)))
    # gather groups: 4-wide, with a small tail so the last gather's compute
    # doesn't leave a long serial epilogue
    GRPS = []
    rem = POS
    while rem > 5:
        GRPS.append(GRP)
        rem -= GRP
    while rem > 0:
        GRPS.append(min(2, rem) if rem > 1 else 1)
        rem -= GRPS[-1]
    g_starts = [int(sum(GRPS[:i])) for i in range(len(GRPS))]
    GLOMAX = max(int(sum(K_LO[g0:g0 + ng])) for g0, ng in zip(g_starts, GRPS))
    GHIMAX = max(int(sum(K_HI[g0:g0 + ng])) for g0, ng in zip(g_starts, GRPS))

    nc = bacc.Bacc("TRN2", target_bir_lowering=False, debug=False,
                   num_devices=NC_CORES)
    htab = nc.dram_tensor("htab", [128 * NTILE, 128], dt.bfloat16,
                          kind="ExternalInput")       # p-major bf16 h
    th = nc.dram_tensor("th", [NROWS, 256], dt.bfloat16,
                        kind="ExternalInput")         # node-major gather table
    w_in = nc.dram_tensor("w_in", [IN, HEADS * C], dt.float32, kind="ExternalInput")
    asrc_in = nc.dram_tensor("asrc_in", [HEADS, C], dt.float32, kind="ExternalInput")
    adst_in = nc.dram_tensor("adst_in", [HEADS, C], dt.float32, kind="ExternalInput")
    bias_in = nc.dram_tensor("bias_in", [1, HEADS * C], dt.float32, kind="ExternalInput")
    wlo_in = nc.dram_tensor("wlo", [128, SUM_LO * 8], dt.int16, kind="ExternalInput")
    whi_in = nc.dram_tensor("whi", [128, SUM_HI * 8], dt.int16, kind="ExternalInput")
    dpj_in = nc.dram_tensor("dpj", [128, SUM_K], dt.float32, kind="ExternalInput")
    bnd_in = nc.dram_tensor("bnd", [128, max(SUM_R, 1)], dt.float32, kind="ExternalInput")
    bglo_in = nc.dram_tensor("bglo", [128, POS * 8], dt.int16, kind="ExternalInput")
    bghi_in = nc.dram_tensor("bghi", [128, POS * 8], dt.int16, kind="ExternalInput")
    bgm_in = nc.dram_tensor("bgm", [128, POS * 2], dt.uint8, kind="ExternalInput")
    out_t = nc.dram_tensor("out", [POS * 128, HEADS * C], dt.float32,
                           kind="ExternalOutput")

    with tile.TileContext(nc) as tc, ExitStack() as ctx:
        const = ctx.enter_context(tc.tile_pool(name="const", bufs=1))

        # ---- constants
        ident_bf = const.tile([P, P], dt.bfloat16)
        make_identity(nc, ident_bf[:])
        iota_row = const.tile([P, P], dt.bfloat16)
        nc.gpsimd.iota(iota_row[:], pattern=[[1, P]], base=0, channel_multiplier=0,
                       allow_small_or_imprecise_dtypes=True)
        iota2k = const.tile([P, RND * 128], dt.float16)
        nc.gpsimd.iota(iota2k[:], pattern=[[1, RND * 128]], base=0,
                       channel_multiplier=0, allow_small_or_imprecise_dtypes=True)
        ones_bf = const.tile([P, 1], dt.bfloat16)
        nc.gpsimd.memset(ones_bf[:], 1.0)
        iota_cp1 = const.tile([P, 1], dt.float32)
        nc.gpsimd.iota(iota_cp1[:], pattern=[[0, 1]], base=1, channel_multiplier=1,
                       allow_small_or_imprecise_dtypes=True)
        shiftmat = const.tile([P, P], dt.float32)
        nc.vector.tensor_scalar(out=shiftmat[:], in0=iota_row[:],
                                scalar1=iota_cp1[:], scalar2=None, op0=op.is_equal)
        w_sb = const.tile([P, HEADS * C], dt.float32)
        nc.sync.dma_start(w_sb[:], w_in.ap()[:, :])
        w_bf = const.tile([P, HEADS * C], dt.bfloat16)
        nc.vector.tensor_scalar(out=w_bf[:], in0=w_sb[:], scalar1=0.0,
                                scalar2=None, op0=op.add)
        bias_bf = const.tile([P, HEADS * C], dt.bfloat16)
        bias_f32 = const.tile([P, HEADS * C], dt.float32)
        nc.sync.dma_start(bias_f32[:], bass.AP(bias_in, 0, [[0, P], [1, HEADS * C]]))
        nc.vector.tensor_scalar(out=bias_bf[:], in0=bias_f32[:], scalar1=0.0,
                                scalar2=None, op0=op.add)

        # wa4[k, i] = sum_c W[k, h*C+c]*att[h, c]; cols: as0 as1 ad0 ad1
        wa4 = const.tile([P, 4], dt.float32)
        wa4hl = const.tile([P, 8], dt.bfloat16)   # [hi0..hi3, lo0..lo3]
        with tc.tile_pool(name="watmp", bufs=2) as tmp_pool:
            for jat, attt in enumerate((asrc_in, adst_in)):
                for hd in range(HEADS):
                    abc = tmp_pool.tile([P, C], dt.float32, tag="abc")
                    nc.sync.dma_start(abc[:], bass.AP(attt, hd * C, [[0, P], [1, C]]))
                    t = tmp_pool.tile([P, C], dt.float32, tag="t")
                    nc.vector.tensor_tensor(
                        out=t[:], in0=w_sb[:, hd * C:(hd + 1) * C],
                        in1=abc[:], op=op.mult)
                    nc.vector.tensor_reduce(
                        out=wa4[:, 2 * jat + hd:2 * jat + hd + 1], in_=t[:],
                        axis=mybir.AxisListType.X, op=op.add)
            nc.vector.tensor_scalar(out=wa4hl[:, 0:4], in0=wa4[:], scalar1=0.0,
                                    scalar2=None, op0=op.add)
            hic = tmp_pool.tile([P, 4], dt.float32, tag="hic")
            nc.vector.tensor_scalar(out=hic[:], in0=wa4hl[:, 0:4], scalar1=0.0,
                                    scalar2=None, op0=op.add)
            lo32 = tmp_pool.tile([P, 4], dt.float32, tag="lo32")
            nc.vector.tensor_tensor(out=lo32[:], in0=wa4[:], in1=hic[:],
                                    op=op.subtract)
            nc.vector.tensor_scalar(out=wa4hl[:, 4:8], in0=lo32[:], scalar1=0.0,
                                    scalar2=None, op0=op.add)

        # ---- phase A: write a_src/a_dst hi/lo into th[:, 128:136]
        ctxA = ExitStack()
        sbA = ctxA.enter_context(tc.tile_pool(name="sbA", bufs=2))
        psT = ctxA.enter_context(tc.tile_pool(name="psT", bufs=2, space="PSUM"))
        psA8 = ctxA.enter_context(tc.tile_pool(name="psA8", bufs=2, space="PSUM"))
        stgA = ctxA.enter_context(tc.tile_pool(name="stgA", bufs=2))

        for t0 in range(0, NTILE, STAGE):
            nst = min(STAGE, NTILE - t0)
            htile = sbA.tile([P, STAGE, 128], dt.bfloat16, tag="htile")
            nc.sync.dma_start(
                htile[:, :nst, :],
                bass.AP(htab, t0 * 128, [[NTILE * 128, P], [128, nst], [1, 128]]))
            tp = psT.tile([P, STAGE * 128], dt.bfloat16, tag="tp", space="PSUM")
            for gi in range(nst):
                nc.tensor.transpose(out=tp[:, gi * 128:(gi + 1) * 128],
                                    in_=htile[:, gi, :], identity=ident_bf[:])
            hT = sbA.tile([P, STAGE * 128], dt.bfloat16, tag="hT")
            nc.vector.tensor_scalar(out=hT[:, :nst * 128], in0=tp[:, :nst * 128],
                                    scalar1=0.0, scalar2=None, op0=op.add)
            a8 = psA8.tile([P, STAGE, 8], dt.float32, tag="a8", space="PSUM")
            for gi in range(nst):
                nc.tensor.matmul(out=a8[:, gi, :],
                                 lhsT=hT[:, gi * 128:(gi + 1) * 128],
                                 rhs=wa4hl[:], start=True, stop=True)
            a8s = stgA.tile([P, STAGE, 8], dt.float32, tag="a8s")
            nc.vector.tensor_scalar(out=a8s[:, :nst, :], in0=a8[:, :nst, :],
                                    scalar1=0.0, scalar2=None, op0=op.add)
            a4g = stgA.tile([P, STAGE, 4], dt.float32, tag="a4g")
            nc.vector.tensor_tensor(out=a4g[:, :nst, :], in0=a8s[:, :nst, 0:4],
                                    in1=a8s[:, :nst, 4:8], op=op.add)
            # th cols 128:136 hold raw fp32 bits of [as0 as1 ad0 ad1]
            nc.scalar.dma_start(
                bass.AP(th, (128 * t0) * 256 + 128,
                        [[256, P], [128 * 256, nst], [1, 8]]),
                a4g[:, :nst, :].bitcast(dt.bfloat16))
        ctxA.close()

        # ---- block gather: a_dst hi/lo per (pos, dst_local) + fp16 deltas
        bgp = ExitStack()
        bgpool = bgp.enter_context(tc.tile_pool(name="bgpool", bufs=1))
        lo_ap = bass.AP(th, 0, [[256, LO], [1, 256]])
        hi_ap = bass.AP(th, LO * 256, [[256, NROWS - LO], [1, 256]])
        bgidx = bgpool.tile([P, POS * 8], dt.int16, tag="bgidx")
        nc.sync.dma_start(bgidx[:], bglo_in.ap()[:, :])
        bgidx2 = bgpool.tile([P, POS * 8], dt.int16, tag="bgidx2")
        nc.sync.dma_start(bgidx2[:], bghi_in.ap()[:, :])
        bgA = bgpool.tile([P, POS, 256], dt.bfloat16, tag="bgA")
        nc.gpsimd.dma_gather(
            out_ap=bgA[:], in_ap=lo_ap, idxs_ap=bgidx[:],
            num_idxs=POS * 128, num_idxs_reg=POS * 128, elem_size=256,
            single_packet=False)
        bgB = bgpool.tile([P, POS, 256], dt.bfloat16, tag="bgB")
        nc.gpsimd.dma_gather(
            out_ap=bgB[:], in_ap=hi_ap, idxs_ap=bgidx2[:],
            num_idxs=POS * 128, num_idxs_reg=POS * 128, elem_size=256,
            single_packet=False)
        bgm = bgpool.tile([P, POS, 2], dt.uint8, tag="bgm")
        nc.sync.dma_start(bgm[:], bgm_in.ap()[:, :])
        # a_dst per block: th cols 132:136 hold fp32 bits of [ad0 ad1]; A/B merge
        adf = bgpool.tile([P, POS, 2], dt.float32, tag="adf")
        nc.vector.tensor_copy(out=adf[:], in_=bgB[:, :, 132:136].bitcast(dt.float32))
        nc.vector.copy_predicated(out=adf[:], mask=bgm[:],
                                  data=bgA[:, :, 132:136].bitcast(dt.float32))
        # shifted[d] = a_dst[d-1] via shift-matrix matmul (exact in fp32 psum)
        psBG = bgp.enter_context(tc.tile_pool(name="psBG", bufs=1, space="PSUM"))
        sh4 = psBG.tile([P, POS, 2], dt.float32, tag="sh4", space="PSUM")
        nc.tensor.matmul(out=sh4[:], lhsT=shiftmat[:], rhs=adf[:],
                         start=True, stop=True)
        shf = bgpool.tile([P, POS, 2], dt.float32, tag="shf")
        nc.vector.tensor_scalar(out=shf[:], in0=sh4[:], scalar1=0.0,
                                scalar2=None, op0=op.add)
        dlt = bgpool.tile([P, POS, 2], dt.float32, tag="dlt")
        nc.vector.tensor_tensor(out=dlt[:], in0=adf[:], in1=shf[:],
                                op=op.subtract)
        delta4 = const.tile([P, POS, 4], dt.float16)
        nc.vector.tensor_scalar(out=delta4[:, :, 0:2], in0=dlt[:],
                                scalar1=0.0, scalar2=None, op0=op.add)
        dhc = bgpool.tile([P, POS, 2], dt.float32, tag="dhc")
        nc.vector.tensor_scalar(out=dhc[:], in0=delta4[:, :, 0:2],
                                scalar1=0.0, scalar2=None, op0=op.add)
        dlo = bgpool.tile([P, POS, 2], dt.float32, tag="dlo")
        nc.vector.tensor_tensor(out=dlo[:], in0=dlt[:], in1=dhc[:],
                                op=op.subtract)
        nc.vector.tensor_scalar(out=delta4[:, :, 2:4], in0=dlo[:],
                                scalar1=0.0, scalar2=None, op0=op.add)
        bgp.close()

        # ---- phase B preloads
        wlo_sb = const.tile([P, SUM_LO * 8], dt.int16)
        nc.sync.dma_start(wlo_sb[:], wlo_in.ap()[:, :])
        whi_sb = const.tile([P, SUM_HI * 8], dt.int16)
        nc.sync.dma_start(whi_sb[:], whi_in.ap()[:, :])
        dpj_sb = const.tile([P, SUM_K], dt.float32)
        nc.sync.dma_start(dpj_sb[:], dpj_in.ap()[:, :])
        bnd_sb = const.tile([P, max(SUM_R, 1)], dt.float32)
        nc.sync.dma_start(bnd_sb[:], bnd_in.ap()[:, :])

        gh = ctx.enter_context(tc.tile_pool(name="gh", bufs=2))
        smp = ctx.enter_context(tc.tile_pool(name="smp", bufs=2))
        exp_ = ctx.enter_context(tc.tile_pool(name="exp", bufs=3))
        tsp = ctx.enter_context(tc.tile_pool(name="tsp", bufs=2))
        fin = ctx.enter_context(tc.tile_pool(name="fin", bufs=2))
        psGT = ctx.enter_context(tc.tile_pool(name="psGT", bufs=2, space="PSUM"))
        psSS = ctx.enter_context(tc.tile_pool(name="psSS", bufs=1, space="PSUM"))
        psAD = ctx.enter_context(tc.tile_pool(name="psAD", bufs=2, space="PSUM"))
        psU = ctx.enter_context(tc.tile_pool(name="psU", bufs=1, space="PSUM"))

        off_lo = [int(sum(K_LO[:j])) for j in range(POS + 1)]
        off_hi = [int(sum(K_HI[:j])) for j in range(POS + 1)]
        off_k = [int(sum(K_LO[:j]) + sum(K_HI[:j])) for j in range(POS + 1)]
        off_r = [0]
        for j in range(POS):
            off_r.append(off_r[-1] + R_LO[j] + R_HI[j])

        # software pipeline: emit logits(j) ahead of chunks(j-1) so the DVE
        # queue never head-of-line blocks on ex (Act) readiness
        pos_grp = []
        for gi, ng in enumerate(GRPS):
            pos_grp += [gi] * ng
        gh_tiles = {}
        ex_t = {}

        def emit_gather(gi):
            g0, ng = g_starts[gi], GRPS[gi]
            slo = off_lo[g0 + ng] - off_lo[g0]
            shi = off_hi[g0 + ng] - off_hi[g0]
            ghlo = gh.tile([P, GLOMAX, 256], dt.bfloat16, tag="ghlo")
            nc.gpsimd.dma_gather(
                out_ap=ghlo[:, :slo, :], in_ap=lo_ap,
                idxs_ap=wlo_sb[:, off_lo[g0] * 8:(off_lo[g0] + slo) * 8],
                num_idxs=slo * 128, num_idxs_reg=slo * 128, elem_size=256,
                single_packet=False)
            ghhi = gh.tile([P, GHIMAX, 256], dt.bfloat16, tag="ghhi")
            nc.gpsimd.dma_gather(
                out_ap=ghhi[:, :shi, :], in_ap=hi_ap,
                idxs_ap=whi_sb[:, off_hi[g0] * 8:(off_hi[g0] + shi) * 8],
                num_idxs=shi * 128, num_idxs_reg=shi * 128, elem_size=256,
                single_packet=False)
            gh_tiles[gi] = (ghlo, ghhi)

        def emit_logits(j):
            KL = int(K_LO[j])
            KH = int(K_HI[j])
            K = KL + KH
            gi = pos_grp[j]
            g0 = g_starts[gi]
            ghlo, ghhi = gh_tiles[gi]
            lbase = off_lo[j] - off_lo[g0]
            hbase = off_hi[j] - off_hi[g0]

            # staircase a_dst per slot
            adp = psAD.tile([P, KMAX, 4], dt.float32, tag="adp", space="PSUM")
            rcol = off_r[j]
            for h, KHF, base in ((0, KL, 0), (1, KH, KL)):
                R = math.ceil(KHF / RND) if KHF else 0
                for r in range(R):
                    c0 = r * RND
                    nch = min(RND, KHF - c0)
                    sm = smp.tile([P, RND * 128], dt.float16, tag="sm")
                    nc.vector.tensor_scalar(
                        out=sm[:, :nch * 128], in0=iota2k[:, :nch * 128],
                        scalar1=bnd_sb[:, rcol:rcol + 1], scalar2=None,
                        op0=op.is_ge)
                    for jj in range(nch):
                        nc.tensor.matmul(
                            out=adp[:, base + c0 + jj, :],
                            lhsT=sm[:, jj * 128:(jj + 1) * 128],
                            rhs=delta4[:, j, :], start=True, stop=True)
                    rcol += 1

            # logits -> ex  (th cols 128:132 = fp32 bits of [as0 as1])
            t1 = tsp.tile([P, KMAX, 2], dt.float32, tag="t1")
            if KL:
                nc.vector.tensor_tensor(
                    out=t1[:, :KL, :],
                    in0=ghlo[:, lbase:lbase + KL, 128:132].bitcast(dt.float32),
                    in1=adp[:, :KL, 0:2], op=op.add)
            if KH:
                nc.vector.tensor_tensor(
                    out=t1[:, KL:K, :],
                    in0=ghhi[:, hbase:hbase + KH, 128:132].bitcast(dt.float32),
                    in1=adp[:, KL:K, 0:2], op=op.add)
            tsum = tsp.tile([P, KMAX, 2], dt.float32, tag="tsum")
            nc.vector.tensor_tensor(out=tsum[:, :K, :], in0=t1[:, :K, :],
                                    in1=adp[:, :K, 2:4], op=op.add)
            u02 = tsp.tile([P, KMAX, 2], dt.float32, tag="u02")
            nc.vector.tensor_scalar(out=u02[:, :K, :], in0=tsum[:, :K, :],
                                    scalar1=NEG_SLOPE, scalar2=None,
                                    op0=op.mult)
            lrt = tsp.tile([P, KMAX, 2], dt.float32, tag="lrt")
            nc.vector.tensor_tensor(out=lrt[:, :K, :], in0=tsum[:, :K, :],
                                    in1=u02[:, :K, :], op=op.max)
            ex = tsp.tile([P, KMAX, 2], dt.float32, tag="ex")
            nc.scalar.activation(out=ex[:, :K, :], in_=lrt[:, :K, :],
                                 func=act.Exp)
            ex_t[j] = ex

        def emit_chunks(j):
            KL = int(K_LO[j])
            KH = int(K_HI[j])
            K = KL + KH
            gi = pos_grp[j]
            g0 = g_starts[gi]
            ghlo, ghhi = gh_tiles[gi]
            lbase = off_lo[j] - off_lo[g0]
            hbase = off_hi[j] - off_hi[g0]
            ex = ex_t.pop(j)

            gtt = psGT.tile([P, HEADS * C], dt.float32, tag="gtt", space="PSUM")
            ss0 = psSS.tile([P, 1], dt.float32, tag="ss0", space="PSUM")
            ss1 = psSS.tile([P, 1], dt.float32, tag="ss1", space="PSUM")
            for jc in range(K):
                if jc < KL:
                    hgc = ghlo[:, lbase + jc, 0:128]
                else:
                    hgc = ghhi[:, hbase + (jc - KL), 0:128]
                st_ = jc == 0
                sp_ = jc == K - 1
                exm = exp_.tile([P, 2 * P], dt.bfloat16, tag="exm")
                for hd in range(HEADS):
                    nc.vector.tensor_scalar(
                        out=exm[:, hd * P:(hd + 1) * P], in0=iota_row[:],
                        scalar1=dpj_sb[:, off_k[j] + jc:off_k[j] + jc + 1],
                        scalar2=ex[:, jc, hd:hd + 1],
                        op0=op.is_equal, op1=op.mult)
                nc.tensor.matmul(out=gtt[:], lhsT=hgc, rhs=exm[:],
                                 start=st_, stop=sp_)
                nc.tensor.matmul(out=ss0[:], lhsT=exm[:, 0:P],
                                 rhs=ones_bf[:], start=st_, stop=sp_)
                nc.tensor.matmul(out=ss1[:], lhsT=exm[:, P:2 * P],
                                 rhs=ones_bf[:], start=st_, stop=sp_)

            # finalize position j
            rec = fin.tile([P, 2], dt.float32, tag="rec")
            nc.vector.reciprocal(out=rec[:, 0:1], in_=ss0[:])
            nc.vector.reciprocal(out=rec[:, 1:2], in_=ss1[:])
            gs = fin.tile([P, HEADS * C], dt.bfloat16, tag="gs")
            nc.scalar.copy(out=gs[:], in_=gtt[:])
            ot = fin.tile([P, HEADS * C], dt.bfloat16, tag="ot")
            for hd in range(HEADS):
                u = psU.tile([P, C], dt.float32, tag="u", space="PSUM")
                nc.tensor.matmul(out=u[:],
                                 lhsT=gs[:, hd * P:(hd + 1) * P],
                                 rhs=w_bf[:, hd * C:(hd + 1) * C],
                                 start=True, stop=True)
                nc.scalar.mul(out=ot[:, hd * C:(hd + 1) * C],
                              in_=u[:],
                              mul=rec[:, hd:hd + 1])
            zt = fin.tile([P, HEADS * C], dt.bfloat16, tag="zt")
            nc.gpsimd.tensor_tensor(out=zt[:], in0=ot[:], in1=bias_bf[:],
                                    op=op.add)
            et = fin.tile([P, HEADS * C], dt.bfloat16, tag="et")
            nc.scalar.activation(out=et[:], in_=zt[:], func=act.Exp)
            mt = fin.tile([P, HEADS * C], dt.bfloat16, tag="mt")
            nc.vector.tensor_scalar(out=mt[:], in0=et[:], scalar1=1.0,
                                    scalar2=-1.0, op0=op.min, op1=op.add)
            rt = fin.tile([P, HEADS * C], dt.bfloat16, tag="rt")
            nc.vector.tensor_scalar(out=rt[:], in0=zt[:], scalar1=0.0,
                                    scalar2=None, op0=op.max)
            ob = fin.tile([P, HEADS * C], dt.bfloat16, tag="ob")
            nc.gpsimd.tensor_tensor(out=ob[:], in0=mt[:], in1=rt[:],
                                    op=op.add)
            obf = fin.tile([P, HEADS * C], dt.float32, tag="obf")
            nc.scalar.copy(out=obf[:], in_=ob[:])
            nc.sync.dma_start(out_t.ap()[j * P:(j + 1) * P, :], obf[:])

        emitted_gi = -1
        for j in range(POS):
            if pos_grp[j] > emitted_gi:
                emit_gather(pos_grp[j])
                emitted_gi = pos_grp[j]
            emit_logits(j)
            emit_chunks(j)

    nc.compile()
    return nc


def _get_program(K_LO, K_HI, R_LO, R_HI):
    key = (tuple(K_LO), tuple(K_HI))
    if key not in _CACHE:
        _CACHE[key] = _build(K_LO, K_HI, R_LO, R_HI)
    return _CACHE[key]


# ------------------------------------------------------------------- kernel
def kernel(h_node, edge_index, W, att_src, att_dst, bias):
    from concourse.bass_utils import run_bass_kernel_spmd

    h_node = np.asarray(h_node, dtype=np.float32)
    W = np.asarray(W, dtype=np.float32)
    att_src = np.asarray(att_src, dtype=np.float32)
    att_dst = np.asarray(att_dst, dtype=np.float32)
    bias = np.asarray(bias, dtype=np.float32).reshape(1, HEADS * C)

    pr = _prep(np.asarray(edge_index))
    nc = _get_program(pr["K_LO"], pr["K_HI"], pr["R_LO"], pr["R_HI"])

    hb = np.zeros((NROWS, 128), dtype=BF16)
    hb[:N] = h_node.astype(BF16)
    # p-major layout: row p*NTILE + t = node 128*t + p
    htab = np.ascontiguousarray(
        hb.reshape(NTILE, 128, 128).transpose(1, 0, 2)).reshape(128 * NTILE, 128)
    thh = np.zeros((NROWS, 256), dtype=BF16)
    thh[:, 0:128] = hb

    in_maps = []
    for c in range(NC_CORES):
        in_maps.append({
            "htab": htab, "th": thh, "w_in": W, "asrc_in": att_src,
            "adst_in": att_dst, "bias_in": bias,
            "wlo": pr["wlo"][c], "whi": pr["whi"][c], "dpj": pr["dpj"][c],
            "bnd": pr["bnd"][c], "bglo": pr["bglo"][c], "bghi": pr["bghi"][c],
            "bgm": pr["bgmask"][c],
        })
    res = run_bass_kernel_spmd(nc, in_maps, core_ids=list(range(NC_CORES)))
    out = np.zeros((N, HEADS * C), dtype=np.float32)
    gmap = pr["gmap"]
    for c in range(NC_CORES):
        o = res.results[c]["out"]
        for j in range(POS):
            gg = gmap[c, j]
            if gg < 0:
                continue
            lo_n = 128 * gg
            hi_n = min(128 * (gg + 1), N)
            out[lo_n:hi_n] = o[j * 128:j * 128 + (hi_n - lo_n)]
    return out


# revision 6
# speedup vs baseline: 1.1880x; 1.1000x over previous
"""GAT layer (PyG GATConv eval, 2 heads x 128, self-loops, ELU) on 8 trn2 cores.

v2 design (dst-block sharded, rank-dealt, bf16 datapath):
  - ht table [50048, 256] bf16 in DRAM: cols 0:128 = bf16(h) (host-uploaded),
    cols 128:136 = a_src/a_dst logits as bf16 hi/lo pairs (device-computed in
    phase A).  One 512B-row dma_gather per edge fetches h AND the src logits.
  - Global dst blocks (128 nodes) are dealt to (core, position) slots by edge
    count rank so per-position chunk counts are uniform across cores (SPMD).
  - Edges sorted by (core, pos, src<32768, dst_local); per (pos, half) padded
    to 128-slot chunks.  Self loops ride the edge stream.
  - Per-slot a_dst via "staircase" matmul: SM[d, slot] = (slot >= first slot of
    dst d's run), adp = SM^T @ delta(a_dst) reconstructs a_dst[dst(slot)]
    exactly (fp16 hi/lo deltas).  No one-hot broadcast machinery.
  - exm one-hot masks in bf16 (4x DVE mode); gtt/ss/U matmuls in bf16.
  - Finalize: normalize on Act engine, ELU via exp/min/max identity.
"""
import math
from contextlib import ExitStack

import numpy as np
import ml_dtypes

BF16 = ml_dtypes.bfloat16
FP16 = np.float16

HEADS = 2
C = 128
IN = 128
N = 50000
NC_CORES = 8
NTILE = math.ceil(N / 128)        # 391 tiles / global blocks
NROWS = NTILE * 128               # 50048 table rows
POS = math.ceil(NTILE / NC_CORES)  # 49 positions per core
LO = 32768                        # lo table view rows [0, 32768)
HIBASE = 17280                    # hi table view rows [17280, 50048)
GRP = 4                           # positions per dma_gather call
RND = 16                          # max chunks per staircase round
NEG_SLOPE = 0.2
STAGE = 16                        # phase-A tiles per group

_CACHE = {}


# ----------------------------------------------------------------- host prep
def _prep(edge_index):
    src = np.concatenate([edge_index[0], np.arange(N)]).astype(np.int64)
    dst = np.concatenate([edge_index[1], np.arange(N)]).astype(np.int64)
    g = dst // 128
    dloc = dst % 128
    half = (src >= LO).astype(np.int64)

    sizes_g = np.bincount(g, minlength=NTILE)
    order_g = np.argsort(-sizes_g, kind="stable")
    gmap = np.full((NC_CORES, POS), -1, dtype=np.int64)
    for j in range(POS):
        for c in range(NC_CORES):
            r = NC_CORES * j + c
            if r < NTILE:
                gmap[c, j] = order_g[r]
    core_of = np.zeros(NTILE, dtype=np.int64)
    pos_of = np.zeros(NTILE, dtype=np.int64)
    for c in range(NC_CORES):
        for j in range(POS):
            gg = gmap[c, j]
            if gg >= 0:
                core_of[gg] = c
                pos_of[gg] = j

    ecore = core_of[g]
    epos = pos_of[g]

    # choose per-position lo/hi split M_j in [HIBASE, 32768] (hi table view
    # starts at row HIBASE so hi idx = src - HIBASE stays in int16) that
    # minimizes padded chunk count max_c ceil(lo/128) + max_c ceil(hi/128)
    cnt_all = np.zeros((NC_CORES, POS), dtype=np.int64)
    np.add.at(cnt_all, (ecore, epos), 1)
    cands = np.arange(HIBASE + 128, LO + 1, 512)
    lo_cnt = np.zeros((NC_CORES, POS, len(cands)), dtype=np.int64)
    for c in range(NC_CORES):
        for j in range(POS):
            sj = np.sort(src[(ecore == c) & (epos == j)])
            lo_cnt[c, j] = np.searchsorted(sj, cands)
    cost = (np.ceil(lo_cnt / 128).max(axis=0)
            + np.ceil((cnt_all[:, :, None] - lo_cnt) / 128).max(axis=0))
    M = cands[np.argmin(cost, axis=1)]                    # [POS]
    half = (src >= M[epos]).astype(np.int64)

    key = ((ecore * POS + epos) * 2 + half) * 128 + dloc
    order = np.argsort(key, kind="stable")
    src_s = src[order]
    dloc_s = dloc[order]
    ecore_s = ecore[order]
    epos_s = epos[order]
    half_s = half[order]

    cnt = np.zeros((NC_CORES, POS, 2), dtype=np.int64)
    np.add.at(cnt, (ecore_s, epos_s, half_s), 1)
    K_LO = np.ceil(cnt[:, :, 0].max(axis=0) / 128).astype(int)  # [POS]
    K_HI = np.ceil(cnt[:, :, 1].max(axis=0) / 128).astype(int)
    K_ALL = K_LO + K_HI
    SUM_LO = int(K_LO.sum())
    SUM_HI = int(K_HI.sum())
    SUM_K = int(K_ALL.sum())
    # rounds per (pos, half)
    R_LO = [math.ceil(k / RND) if k else 0 for k in K_LO]
    R_HI = [math.ceil(k / RND) if k else 0 for k in K_HI]
    SUM_R = int(sum(R_LO) + sum(R_HI))

    # group starts (of edges) per (core, pos, half)
    starts = np.zeros(NC_CORES * POS * 2 + 1, dtype=np.int64)
    np.cumsum(np.bincount(
        (ecore_s * POS + epos_s) * 2 + half_s,
        minlength=NC_CORES * POS * 2), out=starts[1:])

    # per-core tables
    wlo = np.zeros((NC_CORES, 128, SUM_LO * 8), dtype=np.int16)
    whi = np.zeros((NC_CORES, 128, SUM_HI * 8), dtype=np.int16)
    dpj = np.full((NC_CORES, 128, SUM_K), 999.0, dtype=np.float32)
    bnd = np.zeros((NC_CORES, 128, max(SUM_R, 1)), dtype=np.float32)
    bglo = np.zeros((NC_CORES, 128, POS * 8), dtype=np.int16)
    bghi = np.zeros((NC_CORES, 128, POS * 8), dtype=np.int16)
    bgmask = np.zeros((NC_CORES, 128, POS * 2), dtype=np.uint8)

    def wrap16(idx):
        """idx [n] (n % 128 == 0) -> wrapped [128, n // 16] int16."""
        n = len(idx)
        sl = idx.reshape(n // 16, 16).T            # [16, n/16]
        return np.broadcast_to(sl[None, :, :], (8, 16, n // 16)).reshape(
            128, n // 16).astype(np.int16)

    for c in range(NC_CORES):
        off_lo = 0
        off_hi = 0
        off_k = 0
        off_r = 0
        for j in range(POS):
            gg = gmap[c, j]
            for h in range(2):
                K = int((K_LO if h == 0 else K_HI)[j])
                nt = K * 128
                if gg >= 0:
                    s0 = starts[(c * POS + j) * 2 + h]
                    s1 = starts[(c * POS + j) * 2 + h + 1]
                    srcs = src_s[s0:s1]
                    dls = dloc_s[s0:s1]
                else:
                    srcs = np.zeros(0, dtype=np.int64)
                    dls = np.zeros(0, dtype=np.int64)
                n = len(srcs)
                assert n <= nt
                idx = np.zeros(nt, dtype=np.int64)
                idx[:n] = srcs - (HIBASE if h == 1 else 0)
                w = wrap16(idx)
                # slot i -> (chunk i//128, partition i%128)
                dv = np.full(nt, 999.0, dtype=np.float32)
                dv[:n] = dls
                dcol = dv.reshape(K, 128).T if K else np.zeros((128, 0), np.float32)
                # staircase boundaries per round
                first = np.searchsorted(dls, np.arange(128), side="left")  # [128]
                R = math.ceil(K / RND) if K else 0
                for r in range(R):
                    lo_c = r * RND * 128
                    ln = min(RND * 128, nt - lo_c)
                    b = np.clip(first - lo_c, 0, ln).astype(np.float32)
                    bnd[c, :, off_r + r] = b
                if h == 0:
                    wlo[c, :, off_lo * 8:(off_lo + K) * 8] = w
                    off_lo += K
                else:
                    whi[c, :, off_hi * 8:(off_hi + K) * 8] = w
                    off_hi += K
                dpj[c, :, off_k:off_k + K] = dcol
                off_k += K
                off_r += R
            # block gather (a_dst per block)
            if gg >= 0:
                nodes = 128 * gg + np.arange(128)
                if gg < 256:
                    bglo[c, :, j * 8:(j + 1) * 8] = wrap16(nodes)
                    bgmask[c, :, j * 2:(j + 1) * 2] = 1
                else:
                    bghi[c, :, j * 8:(j + 1) * 8] = wrap16(nodes - HIBASE)
    return dict(gmap=gmap, K_LO=K_LO, K_HI=K_HI, R_LO=R_LO, R_HI=R_HI,
                SUM_LO=SUM_LO, SUM_HI=SUM_HI, SUM_K=SUM_K, SUM_R=SUM_R,
                wlo=wlo, whi=whi, dpj=dpj, bnd=bnd,
                bglo=bglo, bghi=bghi, bgmask=bgmask)


# ------------------------------------------------------------ device program
def _build(K_LO, K_HI, R_LO, R_HI):
    import concourse.bacc as bacc
    import concourse.bass as bass
    import concourse.mybir as mybir
    import concourse.tile as tile
    from concourse.masks import make_identity

    dt = mybir.dt
    op = mybir.AluOpType
    act = mybir.ActivationFunctionType
    P = 128
    SUM_LO = int(sum(K_LO))
    SUM_HI = int(sum(K_HI))
    SUM_K = SUM_LO + SUM_HI
    SUM_R = int(sum(R_LO) + sum(R_HI))
    KMAX = int(max(K_LO[j] + K_HI[j] for j in range(POS)))
    # gather groups: 4-wide, with a small tail so the last gather's compute
    # doesn't leave a long serial epilogue
    GRPS = []
    rem = POS
    while rem > 5:
        GRPS.append(GRP)
        rem -= GRP
    while rem > 0:
        GRPS.append(min(2, rem) if rem > 1 else 1)
        rem -= GRPS[-1]
    g_starts = [int(sum(GRPS[:i])) for i in range(len(GRPS))]
    GLOMAX = max(int(sum(K_LO[g0:g0 + ng])) for g0, ng in zip(g_starts, GRPS))
    GHIMAX = max(int(sum(K_HI[g0:g0 + ng])) for g0, ng in zip(g_starts, GRPS))

    nc = bacc.Bacc("TRN2", target_bir_lowering=False, debug=False,
                   num_devices=NC_CORES)
    htab = nc.dram_tensor("htab", [128 * NTILE, 128], dt.bfloat16,
                          kind="ExternalInput")       # p-major bf16 h
    th = nc.dram_tensor("th", [NROWS, 256], dt.bfloat16,
                        kind="ExternalInput")         # node-major gather table
    w_in = nc.dram_tensor("w_in", [IN, HEADS * C], dt.float32, kind="ExternalInput")
    asrc_in = nc.dram_tensor("asrc_in", [HEADS, C], dt.float32, kind="ExternalInput")
    adst_in = nc.dram_tensor("adst_in", [HEADS, C], dt.float32, kind="ExternalInput")
    bias_in = nc.dram_tensor("bias_in", [1, HEADS * C], dt.float32, kind="ExternalInput")
    wlo_in = nc.dram_tensor("wlo", [128, SUM_LO * 8], dt.int16, kind="ExternalInput")
    whi_in = nc.dram_tensor("whi", [128, SUM_HI * 8], dt.int16, kind="ExternalInput")
    dpj_in = nc.dram_tensor("dpj", [128, SUM_K], dt.float32, kind="ExternalInput")
    bnd_in = nc.dram_tensor("bnd", [128, max(SUM_R, 1)], dt.float32, kind="ExternalInput")
    bglo_in = nc.dram_tensor("bglo", [128, POS * 8], dt.int16, kind="ExternalInput")
    bghi_in = nc.dram_tensor("bghi", [128, POS * 8], dt.int16, kind="ExternalInput")
    bgm_in = nc.dram_tensor("bgm", [128, POS * 2], dt.uint8, kind="ExternalInput")
    out_t = nc.dram_tensor("out", [POS * 128, HEADS * C], dt.float32,
                           kind="ExternalOutput")

    with tile.TileContext(nc) as tc, ExitStack() as ctx:
        const = ctx.enter_context(tc.tile_pool(name="const", bufs=1))

        # ---- constants
        ident_bf = const.tile([P, P], dt.bfloat16)
        make_identity(nc, ident_bf[:])
        iota_row = const.tile([P, P], dt.bfloat16)
        nc.gpsimd.iota(iota_row[:], pattern=[[1, P]], base=0, channel_multiplier=0,
                       allow_small_or_imprecise_dtypes=True)
        iota2k = const.tile([P, RND * 128], dt.float16)
        nc.gpsimd.iota(iota2k[:], pattern=[[1, RND * 128]], base=0,
                       channel_multiplier=0, allow_small_or_imprecise_dtypes=True)
        ones_bf = const.tile([P, 1], dt.bfloat16)
        nc.gpsimd.memset(ones_bf[:], 1.0)
        iota_cp1 = const.tile([P, 1], dt.float32)
        nc.gpsimd.iota(iota_cp1[:], pattern=[[0, 1]], base=1, channel_multiplier=1,
                       allow_small_or_imprecise_dtypes=True)
        shiftmat = const.tile([P, P], dt.float32)
        nc.vector.tensor_scalar(out=shiftmat[:], in0=iota_row[:],
                                scalar1=iota_cp1[:], scalar2=None, op0=op.is_equal)
        w_sb = const.tile([P, HEADS * C], dt.float32)
        nc.sync.dma_start(w_sb[:], w_in.ap()[:, :])
        w_bf = const.tile([P, HEADS * C], dt.bfloat16)
        nc.vector.tensor_scalar(out=w_bf[:], in0=w_sb[:], scalar1=0.0,
                                scalar2=None, op0=op.add)
        bias_bf = const.tile([P, HEADS * C], dt.bfloat16)
        bias_f32 = const.tile([P, HEADS * C], dt.float32)
        nc.sync.dma_start(bias_f32[:], bass.AP(bias_in, 0, [[0, P], [1, HEADS * C]]))
        nc.vector.tensor_scalar(out=bias_bf[:], in0=bias_f32[:], scalar1=0.0,
                                scalar2=None, op0=op.add)

        # wa4[k, i] = sum_c W[k, h*C+c]*att[h, c]; cols: as0 as1 ad0 ad1
        wa4 = const.tile([P, 4], dt.float32)
        wa4hl = const.tile([P, 8], dt.bfloat16)   # [hi0..hi3, lo0..lo3]
        with tc.tile_pool(name="watmp", bufs=2) as tmp_pool:
            for jat, attt in enumerate((asrc_in, adst_in)):
                for hd in range(HEADS):
                    abc = tmp_pool.tile([P, C], dt.float32, tag="abc")
                    nc.sync.dma_start(abc[:], bass.AP(attt, hd * C, [[0, P], [1, C]]))
                    t = tmp_pool.tile([P, C], dt.float32, tag="t")
                    nc.vector.tensor_tensor(
                        out=t[:], in0=w_sb[:, hd * C:(hd + 1) * C],
                        in1=abc[:], op=op.mult)
                    nc.vector.tensor_reduce(
                        out=wa4[:, 2 * jat + hd:2 * jat + hd + 1], in_=t[:],
                        axis=mybir.AxisListType.X, op=op.add)
            nc.vector.tensor_scalar(out=wa4hl[:, 0:4], in0=wa4[:], scalar1=0.0,
                                    scalar2=None, op0=op.add)
            hic = tmp_pool.tile([P, 4], dt.float32, tag="hic")
            nc.vector.tensor_scalar(out=hic[:], in0=wa4hl[:, 0:4], scalar1=0.0,
                                    scalar2=None, op0=op.add)
            lo32 = tmp_pool.tile([P, 4], dt.float32, tag="lo32")
            nc.vector.tensor_tensor(out=lo32[:], in0=wa4[:], in1=hic[:],
                                    op=op.subtract)
            nc.vector.tensor_scalar(out=wa4hl[:, 4:8], in0=lo32[:], scalar1=0.0,
                                    scalar2=None, op0=op.add)

        # ---- phase A: write a_src/a_dst hi/lo into th[:, 128:136]
        ctxA = ExitStack()
        sbA = ctxA.enter_context(tc.tile_pool(name="sbA", bufs=2))
        psT = ctxA.enter_context(tc.tile_pool(name="psT", bufs=2, space="PSUM"))
        psA8 = ctxA.enter_context(tc.tile_pool(name="psA8", bufs=2, space="PSUM"))
        stgA = ctxA.enter_context(tc.tile_pool(name="stgA", bufs=2))

        for t0 in range(0, NTILE, STAGE):
            nst = min(STAGE, NTILE - t0)
            htile = sbA.tile([P, STAGE, 128], dt.bfloat16, tag="htile")
            nc.sync.dma_start(
                htile[:, :nst, :],
                bass.AP(htab, t0 * 128, [[NTILE * 128, P], [128, nst], [1, 128]]))
            tp = psT.tile([P, STAGE * 128], dt.bfloat16, tag="tp", space="PSUM")
            for gi in range(nst):
                nc.tensor.transpose(out=tp[:, gi * 128:(gi + 1) * 128],
                                    in_=htile[:, gi, :], identity=ident_bf[:])
            hT = sbA.tile([P, STAGE * 128], dt.bfloat16, tag="hT")
            hh = (nst // 2) * 128
            nc.vector.tensor_scalar(out=hT[:, :hh], in0=tp[:, :hh],
                                    scalar1=0.0, scalar2=None, op0=op.add)
            nc.scalar.copy(out=hT[:, hh:nst * 128], in_=tp[:, hh:nst * 128])
            a8 = psA8.tile([P, STAGE, 8], dt.float32, tag="a8", space="PSUM")
            for gi in range(nst):
                nc.tensor.matmul(out=a8[:, gi, :],
                                 lhsT=hT[:, gi * 128:(gi + 1) * 128],
                                 rhs=wa4hl[:], start=True, stop=True)
            a8s = stgA.tile([P, STAGE, 8], dt.float32, tag="a8s")
            nc.vector.tensor_scalar(out=a8s[:, :nst, :], in0=a8[:, :nst, :],
                                    scalar1=0.0, scalar2=None, op0=op.add)
            a4g = stgA.tile([P, STAGE, 4], dt.float32, tag="a4g")
            nc.vector.tensor_tensor(out=a4g[:, :nst, :], in0=a8s[:, :nst, 0:4],
                                    in1=a8s[:, :nst, 4:8], op=op.add)
            # th cols 128:136 hold raw fp32 bits of [as0 as1 ad0 ad1]
            nc.scalar.dma_start(
                bass.AP(th, (128 * t0) * 256 + 128,
                        [[256, P], [128 * 256, nst], [1, 8]]),
                a4g[:, :nst, :].bitcast(dt.bfloat16))
        ctxA.close()

        # ---- block gather: a_dst hi/lo per (pos, dst_local) + fp16 deltas
        bgp = ExitStack()
        bgpool = bgp.enter_context(tc.tile_pool(name="bgpool", bufs=1))
        lo_ap = bass.AP(th, 0, [[256, LO], [1, 256]])
        hi_ap = bass.AP(th, HIBASE * 256, [[256, NROWS - HIBASE], [1, 256]])
        bgidx = bgpool.tile([P, POS * 8], dt.int16, tag="bgidx")
        nc.sync.dma_start(bgidx[:], bglo_in.ap()[:, :])
        bgidx2 = bgpool.tile([P, POS * 8], dt.int16, tag="bgidx2")
        nc.sync.dma_start(bgidx2[:], bghi_in.ap()[:, :])
        bgA = bgpool.tile([P, POS, 256], dt.bfloat16, tag="bgA")
        nc.gpsimd.dma_gather(
            out_ap=bgA[:], in_ap=lo_ap, idxs_ap=bgidx[:],
            num_idxs=POS * 128, num_idxs_reg=POS * 128, elem_size=256,
            single_packet=False)
        bgB = bgpool.tile([P, POS, 256], dt.bfloat16, tag="bgB")
        nc.gpsimd.dma_gather(
            out_ap=bgB[:], in_ap=hi_ap, idxs_ap=bgidx2[:],
            num_idxs=POS * 128, num_idxs_reg=POS * 128, elem_size=256,
            single_packet=False)
        bgm = bgpool.tile([P, POS, 2], dt.uint8, tag="bgm")
        nc.sync.dma_start(bgm[:], bgm_in.ap()[:, :])
        # a_dst per block: th cols 132:136 hold fp32 bits of [ad0 ad1]; A/B merge
        adf = bgpool.tile([P, POS, 2], dt.float32, tag="adf")
        nc.vector.tensor_copy(out=adf[:], in_=bgB[:, :, 132:136].bitcast(dt.float32))
        nc.vector.copy_predicated(out=adf[:], mask=bgm[:],
                                  data=bgA[:, :, 132:136].bitcast(dt.float32))
        # shifted[d] = a_dst[d-1] via shift-matrix matmul (exact in fp32 psum)
        psBG = bgp.enter_context(tc.tile_pool(name="psBG", bufs=1, space="PSUM"))
        sh4 = psBG.tile([P, POS, 2], dt.float32, tag="sh4", space="PSUM")
        nc.tensor.matmul(out=sh4[:], lhsT=shiftmat[:], rhs=adf[:],
                         start=True, stop=True)
        shf = bgpool.tile([P, POS, 2], dt.float32, tag="shf")
        nc.vector.tensor_scalar(out=shf[:], in0=sh4[:], scalar1=0.0,
                                scalar2=None, op0=op.add)
        dlt = bgpool.tile([P, POS, 2], dt.float32, tag="dlt")
        nc.vector.tensor_tensor(out=dlt[:], in0=adf[:], in1=shf[:],
                                op=op.subtract)
        delta4 = const.tile([P, POS, 4], dt.float16)
        nc.vector.tensor_scalar(out=delta4[:, :, 0:2], in0=dlt[:],
                                scalar1=0.0, scalar2=None, op0=op.add)
        dhc = bgpool.tile([P, POS, 2], dt.float32, tag="dhc")
        nc.vector.tensor_scalar(out=dhc[:], in0=delta4[:, :, 0:2],
                                scalar1=0.0, scalar2=None, op0=op.add)
        dlo = bgpool.tile([P, POS, 2], dt.float32, tag="dlo")
        nc.vector.tensor_tensor(out=dlo[:], in0=dlt[:], in1=dhc[:],
                                op=op.subtract)
        nc.vector.tensor_scalar(out=delta4[:, :, 2:4], in0=dlo[:],
                                scalar1=0.0, scalar2=None, op0=op.add)
        bgp.close()

        # ---- phase B preloads
        wlo_sb = const.tile([P, SUM_LO * 8], dt.int16)
        nc.sync.dma_start(wlo_sb[:], wlo_in.ap()[:, :])
        whi_sb = const.tile([P, SUM_HI * 8], dt.int16)
        nc.sync.dma_start(whi_sb[:], whi_in.ap()[:, :])
        dpj_sb = const.tile([P, SUM_K], dt.float32)
        nc.sync.dma_start(dpj_sb[:], dpj_in.ap()[:, :])
        bnd_sb = const.tile([P, max(SUM_R, 1)], dt.float32)
        nc.sync.dma_start(bnd_sb[:], bnd_in.ap()[:, :])

        gh = ctx.enter_context(tc.tile_pool(name="gh", bufs=2))
        smp = ctx.enter_context(tc.tile_pool(name="smp", bufs=2))
        exp_ = ctx.enter_context(tc.tile_pool(name="exp", bufs=3))
        tsp = ctx.enter_context(tc.tile_pool(name="tsp", bufs=2))
        fin = ctx.enter_context(tc.tile_pool(name="fin", bufs=2))
        psGT = ctx.enter_context(tc.tile_pool(name="psGT", bufs=2, space="PSUM"))
        psSS = ctx.enter_context(tc.tile_pool(name="psSS", bufs=1, space="PSUM"))
        psAD = ctx.enter_context(tc.tile_pool(name="psAD", bufs=2, space="PSUM"))
        psU = ctx.enter_context(tc.tile_pool(name="psU", bufs=1, space="PSUM"))

        off_lo = [int(sum(K_LO[:j])) for j in range(POS + 1)]
        off_hi = [int(sum(K_HI[:j])) for j in range(POS + 1)]
        off_k = [int(sum(K_LO[:j]) + sum(K_HI[:j])) for j in range(POS + 1)]
        off_r = [0]
        for j in range(POS):
            off_r.append(off_r[-1] + R_LO[j] + R_HI[j])

        # software pipeline: emit logits(j) ahead of chunks(j-1) so the DVE
        # queue never head-of-line blocks on ex (Act) readiness
        pos_grp = []
        for gi, ng in enumerate(GRPS):
            pos_grp += [gi] * ng
        gh_tiles = {}
        ex_t = {}

        def emit_gather(gi):
            g0, ng = g_starts[gi], GRPS[gi]
            slo = off_lo[g0 + ng] - off_lo[g0]
            shi = off_hi[g0 + ng] - off_hi[g0]
            ghlo = gh.tile([P, GLOMAX, 256], dt.bfloat16, tag="ghlo")
            nc.gpsimd.dma_gather(
                out_ap=ghlo[:, :slo, :], in_ap=lo_ap,
                idxs_ap=wlo_sb[:, off_lo[g0] * 8:(off_lo[g0] + slo) * 8],
                num_idxs=slo * 128, num_idxs_reg=slo * 128, elem_size=256,
                single_packet=False)
            ghhi = gh.tile([P, GHIMAX, 256], dt.bfloat16, tag="ghhi")
            nc.gpsimd.dma_gather(
                out_ap=ghhi[:, :shi, :], in_ap=hi_ap,
                idxs_ap=whi_sb[:, off_hi[g0] * 8:(off_hi[g0] + shi) * 8],
                num_idxs=shi * 128, num_idxs_reg=shi * 128, elem_size=256,
                single_packet=False)
            gh_tiles[gi] = (ghlo, ghhi)

        def emit_logits(j):
            KL = int(K_LO[j])
            KH = int(K_HI[j])
            K = KL + KH
            gi = pos_grp[j]
            g0 = g_starts[gi]
            ghlo, ghhi = gh_tiles[gi]
            lbase = off_lo[j] - off_lo[g0]
            hbase = off_hi[j] - off_hi[g0]

            # staircase a_dst per slot
            adp = psAD.tile([P, KMAX, 4], dt.float32, tag="adp", space="PSUM")
            rcol = off_r[j]
            for h, KHF, base in ((0, KL, 0), (1, KH, KL)):
                R = math.ceil(KHF / RND) if KHF else 0
                for r in range(R):
                    c0 = r * RND
                    nch = min(RND, KHF - c0)
                    sm = smp.tile([P, RND * 128], dt.float16, tag="sm")
                    nc.vector.tensor_scalar(
                        out=sm[:, :nch * 128], in0=iota2k[:, :nch * 128],
                        scalar1=bnd_sb[:, rcol:rcol + 1], scalar2=None,
                        op0=op.is_ge)
                    for jj in range(nch):
                        nc.tensor.matmul(
                            out=adp[:, base + c0 + jj, :],
                            lhsT=sm[:, jj * 128:(jj + 1) * 128],
                            rhs=delta4[:, j, :], start=True, stop=True)
                    rcol += 1

            # logits -> ex  (th cols 128:132 = fp32 bits of [as0 as1])
            t1 = tsp.tile([P, KMAX, 2], dt.float32, tag="t1")
            if KL:
                nc.vector.tensor_tensor(
                    out=t1[:, :KL, :],
                    in0=ghlo[:, lbase:lbase + KL, 128:132].bitcast(dt.float32),
                    in1=adp[:, :KL, 0:2], op=op.add)
            if KH:
                nc.vector.tensor_tensor(
                    out=t1[:, KL:K, :],
                    in0=ghhi[:, hbase:hbase + KH, 128:132].bitcast(dt.float32),
                    in1=adp[:, KL:K, 0:2], op=op.add)
            tsum = tsp.tile([P, KMAX, 2], dt.float32, tag="tsum")
            nc.vector.tensor_tensor(out=tsum[:, :K, :], in0=t1[:, :K, :],
                                    in1=adp[:, :K, 2:4], op=op.add)
            u02 = tsp.tile([P, KMAX, 2], dt.float32, tag="u02")
            nc.vector.tensor_scalar(out=u02[:, :K, :], in0=tsum[:, :K, :],
                                    scalar1=NEG_SLOPE, scalar2=None,
                                    op0=op.mult)
            lrt = tsp.tile([P, KMAX, 2], dt.float32, tag="lrt")
            nc.vector.tensor_tensor(out=lrt[:, :K, :], in0=tsum[:, :K, :],
                                    in1=u02[:, :K, :], op=op.max)
            ex = tsp.tile([P, KMAX, 2], dt.float32, tag="ex")
            nc.scalar.activation(out=ex[:, :K, :], in_=lrt[:, :K, :],
                                 func=act.Exp)
            ex_t[j] = ex

        def emit_chunks(j):
            KL = int(K_LO[j])
            KH = int(K_HI[j])
            K = KL + KH
            gi = pos_grp[j]
            g0 = g_starts[gi]
            ghlo, ghhi = gh_tiles[gi]
            lbase = off_lo[j] - off_lo[g0]
            hbase = off_hi[j] - off_hi[g0]
            ex = ex_t.pop(j)

            gtt = psGT.tile([P, HEADS * C], dt.float32, tag="gtt", space="PSUM")
            ss0 = psSS.tile([P, 1], dt.float32, tag="ss0", space="PSUM")
            ss1 = psSS.tile([P, 1], dt.float32, tag="ss1", space="PSUM")
            for jc in range(K):
                if jc < KL:
                    hgc = ghlo[:, lbase + jc, 0:128]
                else:
                    hgc = ghhi[:, hbase + (jc - KL), 0:128]
                st_ = jc == 0
                sp_ = jc == K - 1
                exm = exp_.tile([P, 2 * P], dt.bfloat16, tag="exm")
                for hd in range(HEADS):
                    nc.vector.tensor_scalar(
                        out=exm[:, hd * P:(hd + 1) * P], in0=iota_row[:],
                        scalar1=dpj_sb[:, off_k[j] + jc:off_k[j] + jc + 1],
                        scalar2=ex[:, jc, hd:hd + 1],
                        op0=op.is_equal, op1=op.mult)
                nc.tensor.matmul(out=gtt[:], lhsT=hgc, rhs=exm[:],
                                 start=st_, stop=sp_)
                nc.tensor.matmul(out=ss0[:], lhsT=exm[:, 0:P],
                                 rhs=ones_bf[:], start=st_, stop=sp_)
                nc.tensor.matmul(out=ss1[:], lhsT=exm[:, P:2 * P],
                                 rhs=ones_bf[:], start=st_, stop=sp_)

            # finalize position j
            rec = fin.tile([P, 2], dt.float32, tag="rec")
            nc.vector.reciprocal(out=rec[:, 0:1], in_=ss0[:])
            nc.vector.reciprocal(out=rec[:, 1:2], in_=ss1[:])
            gs = fin.tile([P, HEADS * C], dt.bfloat16, tag="gs")
            nc.scalar.copy(out=gs[:], in_=gtt[:])
            ot = fin.tile([P, HEADS * C], dt.bfloat16, tag="ot")
            for hd in range(HEADS):
                u = psU.tile([P, C], dt.float32, tag="u", space="PSUM")
                nc.tensor.matmul(out=u[:],
                                 lhsT=gs[:, hd * P:(hd + 1) * P],
                                 rhs=w_bf[:, hd * C:(hd + 1) * C],
                                 start=True, stop=True)
                nc.scalar.mul(out=ot[:, hd * C:(hd + 1) * C],
                              in_=u[:],
                              mul=rec[:, hd:hd + 1])
            zt = fin.tile([P, HEADS * C], dt.bfloat16, tag="zt")
            nc.gpsimd.tensor_tensor(out=zt[:], in0=ot[:], in1=bias_bf[:],
                                    op=op.add)
            et = fin.tile([P, HEADS * C], dt.bfloat16, tag="et")
            nc.scalar.activation(out=et[:], in_=zt[:], func=act.Exp)
            mt = fin.tile([P, HEADS * C], dt.bfloat16, tag="mt")
            nc.vector.tensor_scalar(out=mt[:], in0=et[:], scalar1=1.0,
                                    scalar2=-1.0, op0=op.min, op1=op.add)
            rt = fin.tile([P, HEADS * C], dt.bfloat16, tag="rt")
            nc.scalar.activation(out=rt[:], in_=zt[:], func=act.Relu)
            ob = fin.tile([P, HEADS * C], dt.bfloat16, tag="ob")
            nc.gpsimd.tensor_tensor(out=ob[:], in0=mt[:], in1=rt[:],
                                    op=op.add)
            obf = fin.tile([P, HEADS * C], dt.float32, tag="obf")
            nc.scalar.copy(out=obf[:], in_=ob[:])
            nc.sync.dma_start(out_t.ap()[j * P:(j + 1) * P, :], obf[:])

        emitted_gi = -1
        for j in range(POS):
            if pos_grp[j] > emitted_gi:
                emit_gather(pos_grp[j])
                emitted_gi = pos_grp[j]
            emit_logits(j)
            emit_chunks(j)

    nc.compile()
    return nc


def _get_program(K_LO, K_HI, R_LO, R_HI):
    key = (tuple(K_LO), tuple(K_HI))
    if key not in _CACHE:
        _CACHE[key] = _build(K_LO, K_HI, R_LO, R_HI)
    return _CACHE[key]


# ------------------------------------------------------------------- kernel
def kernel(h_node, edge_index, W, att_src, att_dst, bias):
    from concourse.bass_utils import run_bass_kernel_spmd

    h_node = np.asarray(h_node, dtype=np.float32)
    W = np.asarray(W, dtype=np.float32)
    att_src = np.asarray(att_src, dtype=np.float32)
    att_dst = np.asarray(att_dst, dtype=np.float32)
    bias = np.asarray(bias, dtype=np.float32).reshape(1, HEADS * C)

    pr = _prep(np.asarray(edge_index))
    nc = _get_program(pr["K_LO"], pr["K_HI"], pr["R_LO"], pr["R_HI"])

    hb = np.zeros((NROWS, 128), dtype=BF16)
    hb[:N] = h_node.astype(BF16)
    # p-major layout: row p*NTILE + t = node 128*t + p
    htab = np.ascontiguousarray(
        hb.reshape(NTILE, 128, 128).transpose(1, 0, 2)).reshape(128 * NTILE, 128)
    thh = np.zeros((NROWS, 256), dtype=BF16)
    thh[:, 0:128] = hb

    in_maps = []
    for c in range(NC_CORES):
        in_maps.append({
            "htab": htab, "th": thh, "w_in": W, "asrc_in": att_src,
            "adst_in": att_dst, "bias_in": bias,
            "wlo": pr["wlo"][c], "whi": pr["whi"][c], "dpj": pr["dpj"][c],
            "bnd": pr["bnd"][c], "bglo": pr["bglo"][c], "bghi": pr["bghi"][c],
            "bgm": pr["bgmask"][c],
        })
    res = run_bass_kernel_spmd(nc, in_maps, core_ids=list(range(NC_CORES)))
    out = np.zeros((N, HEADS * C), dtype=np.float32)
    gmap = pr["gmap"]
    for c in range(NC_CORES):
        o = res.results[c]["out"]
        for j in range(POS):
            gg = gmap[c, j]
            if gg < 0:
                continue
            lo_n = 128 * gg
            hi_n = min(128 * (gg + 1), N)
            out[lo_n:hi_n] = o[j * 128:j * 128 + (hi_n - lo_n)]
    return out


# revision 7
# speedup vs baseline: 1.1959x; 1.0067x over previous
"""GAT layer (PyG GATConv eval, 2 heads x 128, self-loops, ELU) on 8 trn2 cores.

v2 design (dst-block sharded, rank-dealt, bf16 datapath):
  - ht table [50048, 256] bf16 in DRAM: cols 0:128 = bf16(h) (host-uploaded),
    cols 128:136 = a_src/a_dst logits as bf16 hi/lo pairs (device-computed in
    phase A).  One 512B-row dma_gather per edge fetches h AND the src logits.
  - Global dst blocks (128 nodes) are dealt to (core, position) slots by edge
    count rank so per-position chunk counts are uniform across cores (SPMD).
  - Edges sorted by (core, pos, src<32768, dst_local); per (pos, half) padded
    to 128-slot chunks.  Self loops ride the edge stream.
  - Per-slot a_dst via "staircase" matmul: SM[d, slot] = (slot >= first slot of
    dst d's run), adp = SM^T @ delta(a_dst) reconstructs a_dst[dst(slot)]
    exactly (fp16 hi/lo deltas).  No one-hot broadcast machinery.
  - exm one-hot masks in bf16 (4x DVE mode); gtt/ss/U matmuls in bf16.
  - Finalize: normalize on Act engine, ELU via exp/min/max identity.
"""
import math
from contextlib import ExitStack

import numpy as np
import ml_dtypes

BF16 = ml_dtypes.bfloat16
FP16 = np.float16

HEADS = 2
C = 128
IN = 128
N = 50000
NC_CORES = 8
NTILE = math.ceil(N / 128)        # 391 tiles / global blocks
NROWS = NTILE * 128               # 50048 table rows
POS = math.ceil(NTILE / NC_CORES)  # 49 positions per core
LO = 32768                        # lo table view rows [0, 32768)
HIBASE = 17280                    # hi table view rows [17280, 50048)
GRP = 4                           # positions per dma_gather call
RND = 16                          # max chunks per staircase round
NEG_SLOPE = 0.2
STAGE = 16                        # phase-A tiles per group

_CACHE = {}


# ----------------------------------------------------------------- host prep
def _prep(edge_index):
    src = np.concatenate([edge_index[0], np.arange(N)]).astype(np.int64)
    dst = np.concatenate([edge_index[1], np.arange(N)]).astype(np.int64)
    g = dst // 128
    dloc = dst % 128
    half = (src >= LO).astype(np.int64)

    sizes_g = np.bincount(g, minlength=NTILE)
    order_g = np.argsort(-sizes_g, kind="stable")
    gmap = np.full((NC_CORES, POS), -1, dtype=np.int64)
    for j in range(POS):
        for c in range(NC_CORES):
            r = NC_CORES * j + c
            if r < NTILE:
                gmap[c, j] = order_g[r]
    core_of = np.zeros(NTILE, dtype=np.int64)
    pos_of = np.zeros(NTILE, dtype=np.int64)
    for c in range(NC_CORES):
        for j in range(POS):
            gg = gmap[c, j]
            if gg >= 0:
                core_of[gg] = c
                pos_of[gg] = j

    ecore = core_of[g]
    epos = pos_of[g]

    # choose per-position lo/hi split M_j in [HIBASE, 32768] (hi table view
    # starts at row HIBASE so hi idx = src - HIBASE stays in int16) that
    # minimizes padded chunk count max_c ceil(lo/128) + max_c ceil(hi/128)
    cnt_all = np.zeros((NC_CORES, POS), dtype=np.int64)
    np.add.at(cnt_all, (ecore, epos), 1)
    cands = np.arange(HIBASE + 128, LO + 1, 512)
    lo_cnt = np.zeros((NC_CORES, POS, len(cands)), dtype=np.int64)
    for c in range(NC_CORES):
        for j in range(POS):
            sj = np.sort(src[(ecore == c) & (epos == j)])
            lo_cnt[c, j] = np.searchsorted(sj, cands)
    cost = (np.ceil(lo_cnt / 128).max(axis=0)
            + np.ceil((cnt_all[:, :, None] - lo_cnt) / 128).max(axis=0))
    M = cands[np.argmin(cost, axis=1)]                    # [POS]
    half = (src >= M[epos]).astype(np.int64)

    key = ((ecore * POS + epos) * 2 + half) * 128 + dloc
    order = np.argsort(key, kind="stable")
    src_s = src[order]
    dloc_s = dloc[order]
    ecore_s = ecore[order]
    epos_s = epos[order]
    half_s = half[order]

    cnt = np.zeros((NC_CORES, POS, 2), dtype=np.int64)
    np.add.at(cnt, (ecore_s, epos_s, half_s), 1)
    K_LO = np.ceil(cnt[:, :, 0].max(axis=0) / 128).astype(int)  # [POS]
    K_HI = np.ceil(cnt[:, :, 1].max(axis=0) / 128).astype(int)
    K_ALL = K_LO + K_HI
    SUM_LO = int(K_LO.sum())
    SUM_HI = int(K_HI.sum())
    SUM_K = int(K_ALL.sum())
    # rounds per (pos, half)
    R_LO = [math.ceil(k / RND) if k else 0 for k in K_LO]
    R_HI = [math.ceil(k / RND) if k else 0 for k in K_HI]
    SUM_R = int(sum(R_LO) + sum(R_HI))

    # group starts (of edges) per (core, pos, half)
    starts = np.zeros(NC_CORES * POS * 2 + 1, dtype=np.int64)
    np.cumsum(np.bincount(
        (ecore_s * POS + epos_s) * 2 + half_s,
        minlength=NC_CORES * POS * 2), out=starts[1:])

    # per-core tables
    wlo = np.zeros((NC_CORES, 128, SUM_LO * 8), dtype=np.int16)
    whi = np.zeros((NC_CORES, 128, SUM_HI * 8), dtype=np.int16)
    dpj = np.full((NC_CORES, 128, SUM_K), 999.0, dtype=np.float32)
    bnd = np.zeros((NC_CORES, 128, max(SUM_R, 1)), dtype=np.float32)
    bsel = np.zeros((NC_CORES, 128, 4), dtype=np.int16)  # 64 wrapped tile ids

    def wrap16(idx):
        """idx [n] (n % 128 == 0) -> wrapped [128, n // 16] int16."""
        n = len(idx)
        sl = idx.reshape(n // 16, 16).T            # [16, n/16]
        return np.broadcast_to(sl[None, :, :], (8, 16, n // 16)).reshape(
            128, n // 16).astype(np.int16)

    for c in range(NC_CORES):
        off_lo = 0
        off_hi = 0
        off_k = 0
        off_r = 0
        for j in range(POS):
            gg = gmap[c, j]
            for h in range(2):
                K = int((K_LO if h == 0 else K_HI)[j])
                nt = K * 128
                if gg >= 0:
                    s0 = starts[(c * POS + j) * 2 + h]
                    s1 = starts[(c * POS + j) * 2 + h + 1]
                    srcs = src_s[s0:s1]
                    dls = dloc_s[s0:s1]
                else:
                    srcs = np.zeros(0, dtype=np.int64)
                    dls = np.zeros(0, dtype=np.int64)
                n = len(srcs)
                assert n <= nt
                idx = np.zeros(nt, dtype=np.int64)
                idx[:n] = srcs - (HIBASE if h == 1 else 0)
                w = wrap16(idx)
                # slot i -> (chunk i//128, partition i%128)
                dv = np.full(nt, 999.0, dtype=np.float32)
                dv[:n] = dls
                dcol = dv.reshape(K, 128).T if K else np.zeros((128, 0), np.float32)
                # staircase boundaries per round
                first = np.searchsorted(dls, np.arange(128), side="left")  # [128]
                R = math.ceil(K / RND) if K else 0
                for r in range(R):
                    lo_c = r * RND * 128
                    ln = min(RND * 128, nt - lo_c)
                    b = np.clip(first - lo_c, 0, ln).astype(np.float32)
                    bnd[c, :, off_r + r] = b
                if h == 0:
                    wlo[c, :, off_lo * 8:(off_lo + K) * 8] = w
                    off_lo += K
                else:
                    whi[c, :, off_hi * 8:(off_hi + K) * 8] = w
                    off_hi += K
                dpj[c, :, off_k:off_k + K] = dcol
                off_k += K
                off_r += R
        gl = np.zeros(64, dtype=np.int64)
        gl[:POS] = np.maximum(gmap[c], 0)
        bsel[c] = wrap16(gl)
    return dict(gmap=gmap, K_LO=K_LO, K_HI=K_HI, R_LO=R_LO, R_HI=R_HI,
                SUM_LO=SUM_LO, SUM_HI=SUM_HI, SUM_K=SUM_K, SUM_R=SUM_R,
                wlo=wlo, whi=whi, dpj=dpj, bnd=bnd, bsel=bsel)


# ------------------------------------------------------------ device program
def _build(K_LO, K_HI, R_LO, R_HI):
    import concourse.bacc as bacc
    import concourse.bass as bass
    import concourse.mybir as mybir
    import concourse.tile as tile
    from concourse.masks import make_identity

    dt = mybir.dt
    op = mybir.AluOpType
    act = mybir.ActivationFunctionType
    P = 128
    SUM_LO = int(sum(K_LO))
    SUM_HI = int(sum(K_HI))
    SUM_K = SUM_LO + SUM_HI
    SUM_R = int(sum(R_LO) + sum(R_HI))
    KMAX = int(max(K_LO[j] + K_HI[j] for j in range(POS)))
    # gather groups: 4-wide, with a small tail so the last gather's compute
    # doesn't leave a long serial epilogue
    GRPS = []
    rem = POS
    while rem > 5:
        GRPS.append(GRP)
        rem -= GRP
    while rem > 0:
        GRPS.append(min(2, rem) if rem > 1 else 1)
        rem -= GRPS[-1]
    g_starts = [int(sum(GRPS[:i])) for i in range(len(GRPS))]
    GLOMAX = max(int(sum(K_LO[g0:g0 + ng])) for g0, ng in zip(g_starts, GRPS))
    GHIMAX = max(int(sum(K_HI[g0:g0 + ng])) for g0, ng in zip(g_starts, GRPS))

    nc = bacc.Bacc("TRN2", target_bir_lowering=False, debug=False,
                   num_devices=NC_CORES)
    htab = nc.dram_tensor("htab", [128 * NTILE, 128], dt.bfloat16,
                          kind="ExternalInput")       # p-major bf16 h
    th = nc.dram_tensor("th", [NROWS, 256], dt.bfloat16,
                        kind="ExternalInput")         # node-major gather table
    w_in = nc.dram_tensor("w_in", [IN, HEADS * C], dt.float32, kind="ExternalInput")
    asrc_in = nc.dram_tensor("asrc_in", [HEADS, C], dt.float32, kind="ExternalInput")
    adst_in = nc.dram_tensor("adst_in", [HEADS, C], dt.float32, kind="ExternalInput")
    bias_in = nc.dram_tensor("bias_in", [1, HEADS * C], dt.float32, kind="ExternalInput")
    wlo_in = nc.dram_tensor("wlo", [128, SUM_LO * 8], dt.int16, kind="ExternalInput")
    whi_in = nc.dram_tensor("whi", [128, SUM_HI * 8], dt.int16, kind="ExternalInput")
    dpj_in = nc.dram_tensor("dpj", [128, SUM_K], dt.float32, kind="ExternalInput")
    bnd_in = nc.dram_tensor("bnd", [128, max(SUM_R, 1)], dt.float32, kind="ExternalInput")
    bsel_in = nc.dram_tensor("bsel", [128, 4], dt.int16, kind="ExternalInput")
    out_t = nc.dram_tensor("out", [POS * 128, HEADS * C], dt.float32,
                           kind="ExternalOutput")

    with tile.TileContext(nc) as tc, ExitStack() as ctx:
        const = ctx.enter_context(tc.tile_pool(name="const", bufs=1))

        # ---- constants
        ident_bf = const.tile([P, P], dt.bfloat16)
        make_identity(nc, ident_bf[:])
        iota_row = const.tile([P, P], dt.bfloat16)
        nc.gpsimd.iota(iota_row[:], pattern=[[1, P]], base=0, channel_multiplier=0,
                       allow_small_or_imprecise_dtypes=True)
        iota2k = const.tile([P, RND * 128], dt.float16)
        nc.gpsimd.iota(iota2k[:], pattern=[[1, RND * 128]], base=0,
                       channel_multiplier=0, allow_small_or_imprecise_dtypes=True)
        ones_bf = const.tile([P, 1], dt.bfloat16)
        nc.gpsimd.memset(ones_bf[:], 1.0)
        iota_cp1 = const.tile([P, 1], dt.float32)
        nc.gpsimd.iota(iota_cp1[:], pattern=[[0, 1]], base=1, channel_multiplier=1,
                       allow_small_or_imprecise_dtypes=True)
        shiftmat = const.tile([P, P], dt.float32)
        nc.vector.tensor_scalar(out=shiftmat[:], in0=iota_row[:],
                                scalar1=iota_cp1[:], scalar2=None, op0=op.is_equal)
        w_sb = const.tile([P, HEADS * C], dt.float32)
        nc.sync.dma_start(w_sb[:], w_in.ap()[:, :])
        w_bf = const.tile([P, HEADS * C], dt.bfloat16)
        nc.vector.tensor_scalar(out=w_bf[:], in0=w_sb[:], scalar1=0.0,
                                scalar2=None, op0=op.add)
        bias_bf = const.tile([P, HEADS * C], dt.bfloat16)
        bias_f32 = const.tile([P, HEADS * C], dt.float32)
        nc.sync.dma_start(bias_f32[:], bass.AP(bias_in, 0, [[0, P], [1, HEADS * C]]))
        nc.vector.tensor_scalar(out=bias_bf[:], in0=bias_f32[:], scalar1=0.0,
                                scalar2=None, op0=op.add)

        # wa4[k, i] = sum_c W[k, h*C+c]*att[h, c]; cols: as0 as1 ad0 ad1
        wa4 = const.tile([P, 4], dt.float32)
        wa4hl = const.tile([P, 8], dt.bfloat16)   # [hi0..hi3, lo0..lo3]
        with tc.tile_pool(name="watmp", bufs=2) as tmp_pool:
            for jat, attt in enumerate((asrc_in, adst_in)):
                for hd in range(HEADS):
                    abc = tmp_pool.tile([P, C], dt.float32, tag="abc")
                    nc.sync.dma_start(abc[:], bass.AP(attt, hd * C, [[0, P], [1, C]]))
                    t = tmp_pool.tile([P, C], dt.float32, tag="t")
                    nc.vector.tensor_tensor(
                        out=t[:], in0=w_sb[:, hd * C:(hd + 1) * C],
                        in1=abc[:], op=op.mult)
                    nc.vector.tensor_reduce(
                        out=wa4[:, 2 * jat + hd:2 * jat + hd + 1], in_=t[:],
                        axis=mybir.AxisListType.X, op=op.add)
            nc.vector.tensor_scalar(out=wa4hl[:, 0:4], in0=wa4[:], scalar1=0.0,
                                    scalar2=None, op0=op.add)
            hic = tmp_pool.tile([P, 4], dt.float32, tag="hic")
            nc.vector.tensor_scalar(out=hic[:], in0=wa4hl[:, 0:4], scalar1=0.0,
                                    scalar2=None, op0=op.add)
            lo32 = tmp_pool.tile([P, 4], dt.float32, tag="lo32")
            nc.vector.tensor_tensor(out=lo32[:], in0=wa4[:], in1=hic[:],
                                    op=op.subtract)
            nc.vector.tensor_scalar(out=wa4hl[:, 4:8], in0=lo32[:], scalar1=0.0,
                                    scalar2=None, op0=op.add)

        # ---- phase A: write a_src/a_dst hi/lo into th[:, 128:136]
        adall = const.tile([P, NTILE, 2], dt.float32)
        ctxA = ExitStack()
        sbA = ctxA.enter_context(tc.tile_pool(name="sbA", bufs=2))
        psT = ctxA.enter_context(tc.tile_pool(name="psT", bufs=2, space="PSUM"))
        psA8 = ctxA.enter_context(tc.tile_pool(name="psA8", bufs=2, space="PSUM"))
        stgA = ctxA.enter_context(tc.tile_pool(name="stgA", bufs=2))

        for t0 in range(0, NTILE, STAGE):
            nst = min(STAGE, NTILE - t0)
            htile = sbA.tile([P, STAGE, 128], dt.bfloat16, tag="htile")
            nc.sync.dma_start(
                htile[:, :nst, :],
                bass.AP(htab, t0 * 128, [[NTILE * 128, P], [128, nst], [1, 128]]))
            tp = psT.tile([P, STAGE * 128], dt.bfloat16, tag="tp", space="PSUM")
            for gi in range(nst):
                nc.tensor.transpose(out=tp[:, gi * 128:(gi + 1) * 128],
                                    in_=htile[:, gi, :], identity=ident_bf[:])
            hT = sbA.tile([P, STAGE * 128], dt.bfloat16, tag="hT")
            hh = (nst // 2) * 128
            nc.vector.tensor_scalar(out=hT[:, :hh], in0=tp[:, :hh],
                                    scalar1=0.0, scalar2=None, op0=op.add)
            nc.scalar.copy(out=hT[:, hh:nst * 128], in_=tp[:, hh:nst * 128])
            a8 = psA8.tile([P, STAGE, 8], dt.float32, tag="a8", space="PSUM")
            for gi in range(nst):
                nc.tensor.matmul(out=a8[:, gi, :],
                                 lhsT=hT[:, gi * 128:(gi + 1) * 128],
                                 rhs=wa4hl[:], start=True, stop=True)
            a8s = stgA.tile([P, STAGE, 8], dt.float32, tag="a8s")
            nc.vector.tensor_scalar(out=a8s[:, :nst, :], in0=a8[:, :nst, :],
                                    scalar1=0.0, scalar2=None, op0=op.add)
            a4g = stgA.tile([P, STAGE, 4], dt.float32, tag="a4g")
            nc.vector.tensor_tensor(out=a4g[:, :nst, :], in0=a8s[:, :nst, 0:4],
                                    in1=a8s[:, :nst, 4:8], op=op.add)
            nc.vector.tensor_scalar(out=adall[:, t0:t0 + nst, :],
                                    in0=a4g[:, :nst, 2:4],
                                    scalar1=0.0, scalar2=None, op0=op.add)
            # th cols 128:136 hold raw fp32 bits of [as0 as1 ad0 ad1]
            nc.scalar.dma_start(
                bass.AP(th, (128 * t0) * 256 + 128,
                        [[256, P], [128 * 256, nst], [1, 8]]),
                a4g[:, :nst, :].bitcast(dt.bfloat16))
        ctxA.close()

        # ---- select this core's blocks' a_dst from adall (no DRAM roundtrip)
        bgp = ExitStack()
        bgpool = bgp.enter_context(tc.tile_pool(name="bgpool", bufs=1))
        lo_ap = bass.AP(th, 0, [[256, LO], [1, 256]])
        hi_ap = bass.AP(th, HIBASE * 256, [[256, NROWS - HIBASE], [1, 256]])
        bgidx = bgpool.tile([P, 4], dt.int16, tag="bgidx")
        nc.sync.dma_start(bgidx[:], bsel_in.ap()[:, :])
        adsel = bgpool.tile([P, 64, 2], dt.float32, tag="adsel")
        nc.gpsimd.ap_gather(out_ap=adsel[:], in_ap=adall[:], idxs_ap=bgidx[:],
                            channels=128, num_elems=NTILE, d=2, num_idxs=64)
        adf = bgpool.tile([P, POS, 2], dt.float32, tag="adf")
        nc.vector.tensor_copy(out=adf[:], in_=adsel[:, :POS, :])
        # shifted[d] = a_dst[d-1] via shift-matrix matmul (exact in fp32 psum)
        psBG = bgp.enter_context(tc.tile_pool(name="psBG", bufs=1, space="PSUM"))
        sh4 = psBG.tile([P, POS, 2], dt.float32, tag="sh4", space="PSUM")
        nc.tensor.matmul(out=sh4[:], lhsT=shiftmat[:], rhs=adf[:],
                         start=True, stop=True)
        shf = bgpool.tile([P, POS, 2], dt.float32, tag="shf")
        nc.vector.tensor_scalar(out=shf[:], in0=sh4[:], scalar1=0.0,
                                scalar2=None, op0=op.add)
        dlt = bgpool.tile([P, POS, 2], dt.float32, tag="dlt")
        nc.vector.tensor_tensor(out=dlt[:], in0=adf[:], in1=shf[:],
                                op=op.subtract)
        delta4 = const.tile([P, POS, 4], dt.float16)
        nc.vector.tensor_scalar(out=delta4[:, :, 0:2], in0=dlt[:],
                                scalar1=0.0, scalar2=None, op0=op.add)
        dhc = bgpool.tile([P, POS, 2], dt.float32, tag="dhc")
        nc.vector.tensor_scalar(out=dhc[:], in0=delta4[:, :, 0:2],
                                scalar1=0.0, scalar2=None, op0=op.add)
        dlo = bgpool.tile([P, POS, 2], dt.float32, tag="dlo")
        nc.vector.tensor_tensor(out=dlo[:], in0=dlt[:], in1=dhc[:],
                                op=op.subtract)
        nc.vector.tensor_scalar(out=delta4[:, :, 2:4], in0=dlo[:],
                                scalar1=0.0, scalar2=None, op0=op.add)
        bgp.close()

        # ---- phase B preloads
        wlo_sb = const.tile([P, SUM_LO * 8], dt.int16)
        nc.sync.dma_start(wlo_sb[:], wlo_in.ap()[:, :])
        whi_sb = const.tile([P, SUM_HI * 8], dt.int16)
        nc.sync.dma_start(whi_sb[:], whi_in.ap()[:, :])
        dpj_sb = const.tile([P, SUM_K], dt.float32)
        nc.sync.dma_start(dpj_sb[:], dpj_in.ap()[:, :])
        bnd_sb = const.tile([P, max(SUM_R, 1)], dt.float32)
        nc.sync.dma_start(bnd_sb[:], bnd_in.ap()[:, :])

        gh = ctx.enter_context(tc.tile_pool(name="gh", bufs=2))
        smp = ctx.enter_context(tc.tile_pool(name="smp", bufs=3))
        exp_ = ctx.enter_context(tc.tile_pool(name="exp", bufs=4))
        tsp = ctx.enter_context(tc.tile_pool(name="tsp", bufs=3))
        fin = ctx.enter_context(tc.tile_pool(name="fin", bufs=3))
        psGT = ctx.enter_context(tc.tile_pool(name="psGT", bufs=2, space="PSUM"))
        psSS = ctx.enter_context(tc.tile_pool(name="psSS", bufs=1, space="PSUM"))
        psAD = ctx.enter_context(tc.tile_pool(name="psAD", bufs=2, space="PSUM"))
        psU = ctx.enter_context(tc.tile_pool(name="psU", bufs=1, space="PSUM"))

        off_lo = [int(sum(K_LO[:j])) for j in range(POS + 1)]
        off_hi = [int(sum(K_HI[:j])) for j in range(POS + 1)]
        off_k = [int(sum(K_LO[:j]) + sum(K_HI[:j])) for j in range(POS + 1)]
        off_r = [0]
        for j in range(POS):
            off_r.append(off_r[-1] + R_LO[j] + R_HI[j])

        # software pipeline: emit logits(j) ahead of chunks(j-1) so the DVE
        # queue never head-of-line blocks on ex (Act) readiness
        pos_grp = []
        for gi, ng in enumerate(GRPS):
            pos_grp += [gi] * ng
        gh_tiles = {}
        ex_t = {}

        def emit_gather(gi):
            g0, ng = g_starts[gi], GRPS[gi]
            slo = off_lo[g0 + ng] - off_lo[g0]
            shi = off_hi[g0 + ng] - off_hi[g0]
            ghlo = gh.tile([P, GLOMAX, 256], dt.bfloat16, tag="ghlo")
            nc.gpsimd.dma_gather(
                out_ap=ghlo[:, :slo, :], in_ap=lo_ap,
                idxs_ap=wlo_sb[:, off_lo[g0] * 8:(off_lo[g0] + slo) * 8],
                num_idxs=slo * 128, num_idxs_reg=slo * 128, elem_size=256,
                single_packet=False)
            ghhi = gh.tile([P, GHIMAX, 256], dt.bfloat16, tag="ghhi")
            nc.gpsimd.dma_gather(
                out_ap=ghhi[:, :shi, :], in_ap=hi_ap,
                idxs_ap=whi_sb[:, off_hi[g0] * 8:(off_hi[g0] + shi) * 8],
                num_idxs=shi * 128, num_idxs_reg=shi * 128, elem_size=256,
                single_packet=False)
            gh_tiles[gi] = (ghlo, ghhi)

        def emit_logits(j):
            KL = int(K_LO[j])
            KH = int(K_HI[j])
            K = KL + KH
            gi = pos_grp[j]
            g0 = g_starts[gi]
            ghlo, ghhi = gh_tiles[gi]
            lbase = off_lo[j] - off_lo[g0]
            hbase = off_hi[j] - off_hi[g0]

            # staircase a_dst per slot
            adp = psAD.tile([P, KMAX, 4], dt.float32, tag="adp", space="PSUM")
            rcol = off_r[j]
            for h, KHF, base in ((0, KL, 0), (1, KH, KL)):
                R = math.ceil(KHF / RND) if KHF else 0
                for r in range(R):
                    c0 = r * RND
                    nch = min(RND, KHF - c0)
                    sm = smp.tile([P, RND * 128], dt.float16, tag="sm")
                    nc.vector.tensor_scalar(
                        out=sm[:, :nch * 128], in0=iota2k[:, :nch * 128],
                        scalar1=bnd_sb[:, rcol:rcol + 1], scalar2=None,
                        op0=op.is_ge)
                    for jj in range(nch):
                        nc.tensor.matmul(
                            out=adp[:, base + c0 + jj, :],
                            lhsT=sm[:, jj * 128:(jj + 1) * 128],
                            rhs=delta4[:, j, :], start=True, stop=True)
                    rcol += 1

            # logits -> ex  (th cols 128:132 = fp32 bits of [as0 as1])
            t1 = tsp.tile([P, KMAX, 2], dt.float32, tag="t1")
            if KL:
                nc.vector.tensor_tensor(
                    out=t1[:, :KL, :],
                    in0=ghlo[:, lbase:lbase + KL, 128:132].bitcast(dt.float32),
                    in1=adp[:, :KL, 0:2], op=op.add)
            if KH:
                nc.vector.tensor_tensor(
                    out=t1[:, KL:K, :],
                    in0=ghhi[:, hbase:hbase + KH, 128:132].bitcast(dt.float32),
                    in1=adp[:, KL:K, 0:2], op=op.add)
            tsum = tsp.tile([P, KMAX, 2], dt.float32, tag="tsum")
            nc.vector.tensor_tensor(out=tsum[:, :K, :], in0=t1[:, :K, :],
                                    in1=adp[:, :K, 2:4], op=op.add)
            u02 = tsp.tile([P, KMAX, 2], dt.float32, tag="u02")
            nc.vector.tensor_scalar(out=u02[:, :K, :], in0=tsum[:, :K, :],
                                    scalar1=NEG_SLOPE, scalar2=None,
                                    op0=op.mult)
            lrt = tsp.tile([P, KMAX, 2], dt.float32, tag="lrt")
            nc.vector.tensor_tensor(out=lrt[:, :K, :], in0=tsum[:, :K, :],
                                    in1=u02[:, :K, :], op=op.max)
            ex = tsp.tile([P, KMAX, 2], dt.float32, tag="ex")
            nc.scalar.activation(out=ex[:, :K, :], in_=lrt[:, :K, :],
                                 func=act.Exp)
            ex_t[j] = ex

        def emit_chunks(j):
            KL = int(K_LO[j])
            KH = int(K_HI[j])
            K = KL + KH
            gi = pos_grp[j]
            g0 = g_starts[gi]
            ghlo, ghhi = gh_tiles[gi]
            lbase = off_lo[j] - off_lo[g0]
            hbase = off_hi[j] - off_hi[g0]
            ex = ex_t.pop(j)

            gtt = psGT.tile([P, HEADS * C], dt.float32, tag="gtt", space="PSUM")
            ss0 = psSS.tile([P, 1], dt.float32, tag="ss0", space="PSUM")
            ss1 = psSS.tile([P, 1], dt.float32, tag="ss1", space="PSUM")
            for jc in range(K):
                if jc < KL:
                    hgc = ghlo[:, lbase + jc, 0:128]
                else:
                    hgc = ghhi[:, hbase + (jc - KL), 0:128]
                st_ = jc == 0
                sp_ = jc == K - 1
                exm = exp_.tile([P, 2 * P], dt.bfloat16, tag="exm")
                for hd in range(HEADS):
                    nc.vector.tensor_scalar(
                        out=exm[:, hd * P:(hd + 1) * P], in0=iota_row[:],
                        scalar1=dpj_sb[:, off_k[j] + jc:off_k[j] + jc + 1],
                        scalar2=ex[:, jc, hd:hd + 1],
                        op0=op.is_equal, op1=op.mult)
                nc.tensor.matmul(out=gtt[:], lhsT=hgc, rhs=exm[:],
                                 start=st_, stop=sp_)
                nc.tensor.matmul(out=ss0[:], lhsT=exm[:, 0:P],
                                 rhs=ones_bf[:], start=st_, stop=sp_)
                nc.tensor.matmul(out=ss1[:], lhsT=exm[:, P:2 * P],
                                 rhs=ones_bf[:], start=st_, stop=sp_)

            # finalize position j
            rec = fin.tile([P, 2], dt.float32, tag="rec")
            nc.vector.reciprocal(out=rec[:, 0:1], in_=ss0[:])
            nc.vector.reciprocal(out=rec[:, 1:2], in_=ss1[:])
            gs = fin.tile([P, HEADS * C], dt.bfloat16, tag="gs")
            nc.scalar.copy(out=gs[:], in_=gtt[:])
            ot = fin.tile([P, HEADS * C], dt.bfloat16, tag="ot")
            for hd in range(HEADS):
                u = psU.tile([P, C], dt.float32, tag="u", space="PSUM")
                nc.tensor.matmul(out=u[:],
                                 lhsT=gs[:, hd * P:(hd + 1) * P],
                                 rhs=w_bf[:, hd * C:(hd + 1) * C],
                                 start=True, stop=True)
                nc.scalar.mul(out=ot[:, hd * C:(hd + 1) * C],
                              in_=u[:],
                              mul=rec[:, hd:hd + 1])
            zt = fin.tile([P, HEADS * C], dt.bfloat16, tag="zt")
            nc.gpsimd.tensor_tensor(out=zt[:], in0=ot[:], in1=bias_bf[:],
                                    op=op.add)
            et = fin.tile([P, HEADS * C], dt.bfloat16, tag="et")
            nc.scalar.activation(out=et[:], in_=zt[:], func=act.Exp)
            mt = fin.tile([P, HEADS * C], dt.bfloat16, tag="mt")
            nc.vector.tensor_scalar(out=mt[:], in0=et[:], scalar1=1.0,
                                    scalar2=-1.0, op0=op.min, op1=op.add)
            rt = fin.tile([P, HEADS * C], dt.bfloat16, tag="rt")
            nc.scalar.activation(out=rt[:], in_=zt[:], func=act.Relu)
            ob = fin.tile([P, HEADS * C], dt.bfloat16, tag="ob")
            nc.gpsimd.tensor_tensor(out=ob[:], in0=mt[:], in1=rt[:],
                                    op=op.add)
            obf = fin.tile([P, HEADS * C], dt.float32, tag="obf")
            nc.scalar.copy(out=obf[:], in_=ob[:])
            nc.sync.dma_start(out_t.ap()[j * P:(j + 1) * P, :], obf[:])

        emitted_gi = -1
        for j in range(POS):
            if pos_grp[j] > emitted_gi:
                emit_gather(pos_grp[j])
                emitted_gi = pos_grp[j]
            emit_logits(j)
            emit_chunks(j)

    nc.compile()
    return nc


def _get_program(K_LO, K_HI, R_LO, R_HI):
    key = (tuple(K_LO), tuple(K_HI))
    if key not in _CACHE:
        _CACHE[key] = _build(K_LO, K_HI, R_LO, R_HI)
    return _CACHE[key]


# ------------------------------------------------------------------- kernel
def kernel(h_node, edge_index, W, att_src, att_dst, bias):
    from concourse.bass_utils import run_bass_kernel_spmd

    h_node = np.asarray(h_node, dtype=np.float32)
    W = np.asarray(W, dtype=np.float32)
    att_src = np.asarray(att_src, dtype=np.float32)
    att_dst = np.asarray(att_dst, dtype=np.float32)
    bias = np.asarray(bias, dtype=np.float32).reshape(1, HEADS * C)

    pr = _prep(np.asarray(edge_index))
    nc = _get_program(pr["K_LO"], pr["K_HI"], pr["R_LO"], pr["R_HI"])

    hb = np.zeros((NROWS, 128), dtype=BF16)
    hb[:N] = h_node.astype(BF16)
    # p-major layout: row p*NTILE + t = node 128*t + p
    htab = np.ascontiguousarray(
        hb.reshape(NTILE, 128, 128).transpose(1, 0, 2)).reshape(128 * NTILE, 128)
    thh = np.zeros((NROWS, 256), dtype=BF16)
    thh[:, 0:128] = hb

    in_maps = []
    for c in range(NC_CORES):
        in_maps.append({
            "htab": htab, "th": thh, "w_in": W, "asrc_in": att_src,
            "adst_in": att_dst, "bias_in": bias,
            "wlo": pr["wlo"][c], "whi": pr["whi"][c], "dpj": pr["dpj"][c],
            "bnd": pr["bnd"][c], "bsel": pr["bsel"][c],
        })
    res = run_bass_kernel_spmd(nc, in_maps, core_ids=list(range(NC_CORES)))
    out = np.zeros((N, HEADS * C), dtype=np.float32)
    gmap = pr["gmap"]
    for c in range(NC_CORES):
        o = res.results[c]["out"]
        for j in range(POS):
            gg = gmap[c, j]
            if gg < 0:
                continue
            lo_n = 128 * gg
            hi_n = min(128 * (gg + 1), N)
            out[lo_n:hi_n] = o[j * 128:j * 128 + (hi_n - lo_n)]
    return out


# revision 8
# speedup vs baseline: 1.2007x; 1.0040x over previous
"""GAT layer (PyG GATConv eval, 2 heads x 128, self-loops, ELU) on 8 trn2 cores.

v2 design (dst-block sharded, rank-dealt, bf16 datapath):
  - ht table [50048, 256] bf16 in DRAM: cols 0:128 = bf16(h) (host-uploaded),
    cols 128:136 = a_src/a_dst logits as bf16 hi/lo pairs (device-computed in
    phase A).  One 512B-row dma_gather per edge fetches h AND the src logits.
  - Global dst blocks (128 nodes) are dealt to (core, position) slots by edge
    count rank so per-position chunk counts are uniform across cores (SPMD).
  - Edges sorted by (core, pos, src<32768, dst_local); per (pos, half) padded
    to 128-slot chunks.  Self loops ride the edge stream.
  - Per-slot a_dst via "staircase" matmul: SM[d, slot] = (slot >= first slot of
    dst d's run), adp = SM^T @ delta(a_dst) reconstructs a_dst[dst(slot)]
    exactly (fp16 hi/lo deltas).  No one-hot broadcast machinery.
  - exm one-hot masks in bf16 (4x DVE mode); gtt/ss/U matmuls in bf16.
  - Finalize: normalize on Act engine, ELU via exp/min/max identity.
"""
import math
from contextlib import ExitStack

import numpy as np
import ml_dtypes

BF16 = ml_dtypes.bfloat16
FP16 = np.float16

HEADS = 2
C = 128
IN = 128
N = 50000
NC_CORES = 8
NTILE = math.ceil(N / 128)        # 391 tiles / global blocks
NROWS = NTILE * 128               # 50048 table rows
POS = math.ceil(NTILE / NC_CORES)  # 49 positions per core
LO = 32768                        # lo table view rows [0, 32768)
HIBASE = 17280                    # hi table view rows [17280, 50048)
GRP = 4                           # positions per dma_gather call
RND = 16                          # max chunks per staircase round
NEG_SLOPE = 0.2
STAGE = 16                        # phase-A tiles per group

_CACHE = {}


# ----------------------------------------------------------------- host prep
def _prep(edge_index):
    src = np.concatenate([edge_index[0], np.arange(N)]).astype(np.int64)
    dst = np.concatenate([edge_index[1], np.arange(N)]).astype(np.int64)
    g = dst // 128
    dloc = dst % 128
    half = (src >= LO).astype(np.int64)

    sizes_g = np.bincount(g, minlength=NTILE)
    order_g = np.argsort(-sizes_g, kind="stable")
    gmap = np.full((NC_CORES, POS), -1, dtype=np.int64)
    for j in range(POS):
        for c in range(NC_CORES):
            r = NC_CORES * j + c
            if r < NTILE:
                gmap[c, j] = order_g[r]
    core_of = np.zeros(NTILE, dtype=np.int64)
    pos_of = np.zeros(NTILE, dtype=np.int64)
    for c in range(NC_CORES):
        for j in range(POS):
            gg = gmap[c, j]
            if gg >= 0:
                core_of[gg] = c
                pos_of[gg] = j

    ecore = core_of[g]
    epos = pos_of[g]

    # choose per-position lo/hi split M_j in [HIBASE, 32768] (hi table view
    # starts at row HIBASE so hi idx = src - HIBASE stays in int16) that
    # minimizes padded chunk count max_c ceil(lo/128) + max_c ceil(hi/128)
    cnt_all = np.zeros((NC_CORES, POS), dtype=np.int64)
    np.add.at(cnt_all, (ecore, epos), 1)
    cands = np.arange(HIBASE + 128, LO + 1, 512)
    lo_cnt = np.zeros((NC_CORES, POS, len(cands)), dtype=np.int64)
    for c in range(NC_CORES):
        for j in range(POS):
            sj = np.sort(src[(ecore == c) & (epos == j)])
            lo_cnt[c, j] = np.searchsorted(sj, cands)
    cost = (np.ceil(lo_cnt / 128).max(axis=0)
            + np.ceil((cnt_all[:, :, None] - lo_cnt) / 128).max(axis=0))
    M = cands[np.argmin(cost, axis=1)]                    # [POS]
    half = (src >= M[epos]).astype(np.int64)

    key = ((ecore * POS + epos) * 2 + half) * 128 + dloc
    order = np.argsort(key, kind="stable")
    src_s = src[order]
    dloc_s = dloc[order]
    ecore_s = ecore[order]
    epos_s = epos[order]
    half_s = half[order]

    cnt = np.zeros((NC_CORES, POS, 2), dtype=np.int64)
    np.add.at(cnt, (ecore_s, epos_s, half_s), 1)
    K_LO = np.ceil(cnt[:, :, 0].max(axis=0) / 128).astype(int)  # [POS]
    K_HI = np.ceil(cnt[:, :, 1].max(axis=0) / 128).astype(int)
    K_ALL = K_LO + K_HI
    SUM_LO = int(K_LO.sum())
    SUM_HI = int(K_HI.sum())
    SUM_K = int(K_ALL.sum())
    # rounds per (pos, half)
    R_LO = [math.ceil(k / RND) if k else 0 for k in K_LO]
    R_HI = [math.ceil(k / RND) if k else 0 for k in K_HI]
    SUM_R = int(sum(R_LO) + sum(R_HI))

    # group starts (of edges) per (core, pos, half)
    starts = np.zeros(NC_CORES * POS * 2 + 1, dtype=np.int64)
    np.cumsum(np.bincount(
        (ecore_s * POS + epos_s) * 2 + half_s,
        minlength=NC_CORES * POS * 2), out=starts[1:])

    # per-core tables
    wlo = np.zeros((NC_CORES, 128, SUM_LO * 8), dtype=np.int16)
    whi = np.zeros((NC_CORES, 128, SUM_HI * 8), dtype=np.int16)
    dpj = np.full((NC_CORES, 128, SUM_K), 999.0, dtype=np.float32)
    bnd = np.zeros((NC_CORES, 128, max(SUM_R, 1)), dtype=np.float32)
    bsel = np.zeros((NC_CORES, 128, 4), dtype=np.int16)  # 64 wrapped tile ids

    def wrap16(idx):
        """idx [n] (n % 128 == 0) -> wrapped [128, n // 16] int16."""
        n = len(idx)
        sl = idx.reshape(n // 16, 16).T            # [16, n/16]
        return np.broadcast_to(sl[None, :, :], (8, 16, n // 16)).reshape(
            128, n // 16).astype(np.int16)

    for c in range(NC_CORES):
        off_lo = 0
        off_hi = 0
        off_k = 0
        off_r = 0
        for j in range(POS):
            gg = gmap[c, j]
            for h in range(2):
                K = int((K_LO if h == 0 else K_HI)[j])
                nt = K * 128
                if gg >= 0:
                    s0 = starts[(c * POS + j) * 2 + h]
                    s1 = starts[(c * POS + j) * 2 + h + 1]
                    srcs = src_s[s0:s1]
                    dls = dloc_s[s0:s1]
                else:
                    srcs = np.zeros(0, dtype=np.int64)
                    dls = np.zeros(0, dtype=np.int64)
                n = len(srcs)
                assert n <= nt
                idx = np.zeros(nt, dtype=np.int64)
                idx[:n] = srcs - (HIBASE if h == 1 else 0)
                w = wrap16(idx)
                # slot i -> (chunk i//128, partition i%128)
                dv = np.full(nt, 999.0, dtype=np.float32)
                dv[:n] = dls
                dcol = dv.reshape(K, 128).T if K else np.zeros((128, 0), np.float32)
                # staircase boundaries per round
                first = np.searchsorted(dls, np.arange(128), side="left")  # [128]
                R = math.ceil(K / RND) if K else 0
                for r in range(R):
                    lo_c = r * RND * 128
                    ln = min(RND * 128, nt - lo_c)
                    b = np.clip(first - lo_c, 0, ln).astype(np.float32)
                    bnd[c, :, off_r + r] = b
                if h == 0:
                    wlo[c, :, off_lo * 8:(off_lo + K) * 8] = w
                    off_lo += K
                else:
                    whi[c, :, off_hi * 8:(off_hi + K) * 8] = w
                    off_hi += K
                dpj[c, :, off_k:off_k + K] = dcol
                off_k += K
                off_r += R
        gl = np.zeros(64, dtype=np.int64)
        gl[:POS] = np.maximum(gmap[c], 0)
        bsel[c] = wrap16(gl)
    return dict(gmap=gmap, K_LO=K_LO, K_HI=K_HI, R_LO=R_LO, R_HI=R_HI,
                SUM_LO=SUM_LO, SUM_HI=SUM_HI, SUM_K=SUM_K, SUM_R=SUM_R,
                wlo=wlo, whi=whi, dpj=dpj, bnd=bnd, bsel=bsel)


# ------------------------------------------------------------ device program
def _build(K_LO, K_HI, R_LO, R_HI):
    import concourse.bacc as bacc
    import concourse.bass as bass
    import concourse.mybir as mybir
    import concourse.tile as tile
    from concourse.masks import make_identity

    dt = mybir.dt
    op = mybir.AluOpType
    act = mybir.ActivationFunctionType
    P = 128
    SUM_LO = int(sum(K_LO))
    SUM_HI = int(sum(K_HI))
    SUM_K = SUM_LO + SUM_HI
    SUM_R = int(sum(R_LO) + sum(R_HI))
    KMAX = int(max(K_LO[j] + K_HI[j] for j in range(POS)))
    # gather groups: 4-wide, with a small tail so the last gather's compute
    # doesn't leave a long serial epilogue
    GRPS = [1, 1, 2]
    rem = POS - 4
    while rem > 5:
        GRPS.append(GRP)
        rem -= GRP
    while rem > 0:
        GRPS.append(min(2, rem) if rem > 1 else 1)
        rem -= GRPS[-1]
    g_starts = [int(sum(GRPS[:i])) for i in range(len(GRPS))]
    GLOMAX = max(int(sum(K_LO[g0:g0 + ng])) for g0, ng in zip(g_starts, GRPS))
    GHIMAX = max(int(sum(K_HI[g0:g0 + ng])) for g0, ng in zip(g_starts, GRPS))

    nc = bacc.Bacc("TRN2", target_bir_lowering=False, debug=False,
                   num_devices=NC_CORES)
    htab = nc.dram_tensor("htab", [128 * NTILE, 128], dt.bfloat16,
                          kind="ExternalInput")       # p-major bf16 h
    th = nc.dram_tensor("th", [NROWS, 256], dt.bfloat16,
                        kind="ExternalInput")         # node-major gather table
    w_in = nc.dram_tensor("w_in", [IN, HEADS * C], dt.float32, kind="ExternalInput")
    asrc_in = nc.dram_tensor("asrc_in", [HEADS, C], dt.float32, kind="ExternalInput")
    adst_in = nc.dram_tensor("adst_in", [HEADS, C], dt.float32, kind="ExternalInput")
    bias_in = nc.dram_tensor("bias_in", [1, HEADS * C], dt.float32, kind="ExternalInput")
    wlo_in = nc.dram_tensor("wlo", [128, SUM_LO * 8], dt.int16, kind="ExternalInput")
    whi_in = nc.dram_tensor("whi", [128, SUM_HI * 8], dt.int16, kind="ExternalInput")
    dpj_in = nc.dram_tensor("dpj", [128, SUM_K], dt.float32, kind="ExternalInput")
    bnd_in = nc.dram_tensor("bnd", [128, max(SUM_R, 1)], dt.float32, kind="ExternalInput")
    bsel_in = nc.dram_tensor("bsel", [128, 4], dt.int16, kind="ExternalInput")
    out_t = nc.dram_tensor("out", [POS * 128, HEADS * C], dt.float32,
                           kind="ExternalOutput")

    with tile.TileContext(nc) as tc, ExitStack() as ctx:
        const = ctx.enter_context(tc.tile_pool(name="const", bufs=1))

        # ---- constants
        ident_bf = const.tile([P, P], dt.bfloat16)
        make_identity(nc, ident_bf[:])
        iota_row = const.tile([P, P], dt.bfloat16)
        nc.gpsimd.iota(iota_row[:], pattern=[[1, P]], base=0, channel_multiplier=0,
                       allow_small_or_imprecise_dtypes=True)
        iota2k = const.tile([P, RND * 128], dt.float16)
        nc.gpsimd.iota(iota2k[:], pattern=[[1, RND * 128]], base=0,
                       channel_multiplier=0, allow_small_or_imprecise_dtypes=True)
        ones_bf = const.tile([P, 1], dt.bfloat16)
        nc.gpsimd.memset(ones_bf[:], 1.0)
        iota_cp1 = const.tile([P, 1], dt.float32)
        nc.gpsimd.iota(iota_cp1[:], pattern=[[0, 1]], base=1, channel_multiplier=1,
                       allow_small_or_imprecise_dtypes=True)
        shiftmat = const.tile([P, P], dt.float32)
        nc.vector.tensor_scalar(out=shiftmat[:], in0=iota_row[:],
                                scalar1=iota_cp1[:], scalar2=None, op0=op.is_equal)
        w_sb = const.tile([P, HEADS * C], dt.float32)
        nc.sync.dma_start(w_sb[:], w_in.ap()[:, :])
        w_bf = const.tile([P, HEADS * C], dt.bfloat16)
        nc.vector.tensor_scalar(out=w_bf[:], in0=w_sb[:], scalar1=0.0,
                                scalar2=None, op0=op.add)
        bias_bf = const.tile([P, HEADS * C], dt.bfloat16)
        bias_f32 = const.tile([P, HEADS * C], dt.float32)
        nc.sync.dma_start(bias_f32[:], bass.AP(bias_in, 0, [[0, P], [1, HEADS * C]]))
        nc.vector.tensor_scalar(out=bias_bf[:], in0=bias_f32[:], scalar1=0.0,
                                scalar2=None, op0=op.add)

        # wa4[k, i] = sum_c W[k, h*C+c]*att[h, c]; cols: as0 as1 ad0 ad1
        wa4 = const.tile([P, 4], dt.float32)
        wa4hl = const.tile([P, 8], dt.bfloat16)   # [hi0..hi3, lo0..lo3]
        with tc.tile_pool(name="watmp", bufs=2) as tmp_pool:
            for jat, attt in enumerate((asrc_in, adst_in)):
                for hd in range(HEADS):
                    abc = tmp_pool.tile([P, C], dt.float32, tag="abc")
                    nc.sync.dma_start(abc[:], bass.AP(attt, hd * C, [[0, P], [1, C]]))
                    t = tmp_pool.tile([P, C], dt.float32, tag="t")
                    nc.vector.tensor_tensor(
                        out=t[:], in0=w_sb[:, hd * C:(hd + 1) * C],
                        in1=abc[:], op=op.mult)
                    nc.vector.tensor_reduce(
                        out=wa4[:, 2 * jat + hd:2 * jat + hd + 1], in_=t[:],
                        axis=mybir.AxisListType.X, op=op.add)
            nc.vector.tensor_scalar(out=wa4hl[:, 0:4], in0=wa4[:], scalar1=0.0,
                                    scalar2=None, op0=op.add)
            hic = tmp_pool.tile([P, 4], dt.float32, tag="hic")
            nc.vector.tensor_scalar(out=hic[:], in0=wa4hl[:, 0:4], scalar1=0.0,
                                    scalar2=None, op0=op.add)
            lo32 = tmp_pool.tile([P, 4], dt.float32, tag="lo32")
            nc.vector.tensor_tensor(out=lo32[:], in0=wa4[:], in1=hic[:],
                                    op=op.subtract)
            nc.vector.tensor_scalar(out=wa4hl[:, 4:8], in0=lo32[:], scalar1=0.0,
                                    scalar2=None, op0=op.add)

        # ---- phase A: write a_src/a_dst hi/lo into th[:, 128:136]
        adall = const.tile([P, NTILE, 2], dt.float32)
        ctxA = ExitStack()
        sbA = ctxA.enter_context(tc.tile_pool(name="sbA", bufs=2))
        psT = ctxA.enter_context(tc.tile_pool(name="psT", bufs=2, space="PSUM"))
        psA8 = ctxA.enter_context(tc.tile_pool(name="psA8", bufs=2, space="PSUM"))
        stgA = ctxA.enter_context(tc.tile_pool(name="stgA", bufs=2))

        for t0 in range(0, NTILE, STAGE):
            nst = min(STAGE, NTILE - t0)
            htile = sbA.tile([P, STAGE, 128], dt.bfloat16, tag="htile")
            nc.sync.dma_start(
                htile[:, :nst, :],
                bass.AP(htab, t0 * 128, [[NTILE * 128, P], [128, nst], [1, 128]]))
            tp = psT.tile([P, STAGE * 128], dt.bfloat16, tag="tp", space="PSUM")
            for gi in range(nst):
                nc.tensor.transpose(out=tp[:, gi * 128:(gi + 1) * 128],
                                    in_=htile[:, gi, :], identity=ident_bf[:])
            hT = sbA.tile([P, STAGE * 128], dt.bfloat16, tag="hT")
            hh = (nst // 2) * 128
            nc.vector.tensor_scalar(out=hT[:, :hh], in0=tp[:, :hh],
                                    scalar1=0.0, scalar2=None, op0=op.add)
            nc.scalar.copy(out=hT[:, hh:nst * 128], in_=tp[:, hh:nst * 128])
            a8 = psA8.tile([P, STAGE, 8], dt.float32, tag="a8", space="PSUM")
            for gi in range(nst):
                nc.tensor.matmul(out=a8[:, gi, :],
                                 lhsT=hT[:, gi * 128:(gi + 1) * 128],
                                 rhs=wa4hl[:], start=True, stop=True)
            a8s = stgA.tile([P, STAGE, 8], dt.float32, tag="a8s")
            nc.vector.tensor_scalar(out=a8s[:, :nst, :], in0=a8[:, :nst, :],
                                    scalar1=0.0, scalar2=None, op0=op.add)
            a4g = stgA.tile([P, STAGE, 4], dt.float32, tag="a4g")
            nc.vector.tensor_tensor(out=a4g[:, :nst, :], in0=a8s[:, :nst, 0:4],
                                    in1=a8s[:, :nst, 4:8], op=op.add)
            nc.vector.tensor_scalar(out=adall[:, t0:t0 + nst, :],
                                    in0=a4g[:, :nst, 2:4],
                                    scalar1=0.0, scalar2=None, op0=op.add)
            # th cols 128:136 hold raw fp32 bits of [as0 as1 ad0 ad1]
            nc.scalar.dma_start(
                bass.AP(th, (128 * t0) * 256 + 128,
                        [[256, P], [128 * 256, nst], [1, 8]]),
                a4g[:, :nst, :].bitcast(dt.bfloat16))
        ctxA.close()

        # ---- select this core's blocks' a_dst from adall (no DRAM roundtrip)
        bgp = ExitStack()
        bgpool = bgp.enter_context(tc.tile_pool(name="bgpool", bufs=1))
        lo_ap = bass.AP(th, 0, [[256, LO], [1, 256]])
        hi_ap = bass.AP(th, HIBASE * 256, [[256, NROWS - HIBASE], [1, 256]])
        bgidx = bgpool.tile([P, 4], dt.int16, tag="bgidx")
        nc.sync.dma_start(bgidx[:], bsel_in.ap()[:, :])
        adsel = bgpool.tile([P, 64, 2], dt.float32, tag="adsel")
        nc.gpsimd.ap_gather(out_ap=adsel[:], in_ap=adall[:], idxs_ap=bgidx[:],
                            channels=128, num_elems=NTILE, d=2, num_idxs=64)
        adf = bgpool.tile([P, POS, 2], dt.float32, tag="adf")
        nc.vector.tensor_copy(out=adf[:], in_=adsel[:, :POS, :])
        # shifted[d] = a_dst[d-1] via shift-matrix matmul (exact in fp32 psum)
        psBG = bgp.enter_context(tc.tile_pool(name="psBG", bufs=1, space="PSUM"))
        sh4 = psBG.tile([P, POS, 2], dt.float32, tag="sh4", space="PSUM")
        nc.tensor.matmul(out=sh4[:], lhsT=shiftmat[:], rhs=adf[:],
                         start=True, stop=True)
        shf = bgpool.tile([P, POS, 2], dt.float32, tag="shf")
        nc.vector.tensor_scalar(out=shf[:], in0=sh4[:], scalar1=0.0,
                                scalar2=None, op0=op.add)
        dlt = bgpool.tile([P, POS, 2], dt.float32, tag="dlt")
        nc.vector.tensor_tensor(out=dlt[:], in0=adf[:], in1=shf[:],
                                op=op.subtract)
        delta4 = const.tile([P, POS, 4], dt.float16)
        nc.vector.tensor_scalar(out=delta4[:, :, 0:2], in0=dlt[:],
                                scalar1=0.0, scalar2=None, op0=op.add)
        dhc = bgpool.tile([P, POS, 2], dt.float32, tag="dhc")
        nc.vector.tensor_scalar(out=dhc[:], in0=delta4[:, :, 0:2],
                                scalar1=0.0, scalar2=None, op0=op.add)
        dlo = bgpool.tile([P, POS, 2], dt.float32, tag="dlo")
        nc.vector.tensor_tensor(out=dlo[:], in0=dlt[:], in1=dhc[:],
                                op=op.subtract)
        nc.vector.tensor_scalar(out=delta4[:, :, 2:4], in0=dlo[:],
                                scalar1=0.0, scalar2=None, op0=op.add)
        bgp.close()

        # ---- phase B preloads
        wlo_sb = const.tile([P, SUM_LO * 8], dt.int16)
        nc.sync.dma_start(wlo_sb[:], wlo_in.ap()[:, :])
        whi_sb = const.tile([P, SUM_HI * 8], dt.int16)
        nc.sync.dma_start(whi_sb[:], whi_in.ap()[:, :])
        dpj_sb = const.tile([P, SUM_K], dt.float32)
        nc.sync.dma_start(dpj_sb[:], dpj_in.ap()[:, :])
        bnd_sb = const.tile([P, max(SUM_R, 1)], dt.float32)
        nc.sync.dma_start(bnd_sb[:], bnd_in.ap()[:, :])

        gh = ctx.enter_context(tc.tile_pool(name="gh", bufs=2))
        smp = ctx.enter_context(tc.tile_pool(name="smp", bufs=3))
        exp_ = ctx.enter_context(tc.tile_pool(name="exp", bufs=4))
        tsp = ctx.enter_context(tc.tile_pool(name="tsp", bufs=3))
        fin = ctx.enter_context(tc.tile_pool(name="fin", bufs=3))
        psGT = ctx.enter_context(tc.tile_pool(name="psGT", bufs=2, space="PSUM"))
        psSS = ctx.enter_context(tc.tile_pool(name="psSS", bufs=1, space="PSUM"))
        psAD = ctx.enter_context(tc.tile_pool(name="psAD", bufs=2, space="PSUM"))
        psU = ctx.enter_context(tc.tile_pool(name="psU", bufs=1, space="PSUM"))

        off_lo = [int(sum(K_LO[:j])) for j in range(POS + 1)]
        off_hi = [int(sum(K_HI[:j])) for j in range(POS + 1)]
        off_k = [int(sum(K_LO[:j]) + sum(K_HI[:j])) for j in range(POS + 1)]
        off_r = [0]
        for j in range(POS):
            off_r.append(off_r[-1] + R_LO[j] + R_HI[j])

        # software pipeline: emit logits(j) ahead of chunks(j-1) so the DVE
        # queue never head-of-line blocks on ex (Act) readiness
        pos_grp = []
        for gi, ng in enumerate(GRPS):
            pos_grp += [gi] * ng
        gh_tiles = {}
        ex_t = {}

        def emit_gather(gi):
            g0, ng = g_starts[gi], GRPS[gi]
            slo = off_lo[g0 + ng] - off_lo[g0]
            shi = off_hi[g0 + ng] - off_hi[g0]
            ghlo = gh.tile([P, GLOMAX, 256], dt.bfloat16, tag="ghlo")
            nc.gpsimd.dma_gather(
                out_ap=ghlo[:, :slo, :], in_ap=lo_ap,
                idxs_ap=wlo_sb[:, off_lo[g0] * 8:(off_lo[g0] + slo) * 8],
                num_idxs=slo * 128, num_idxs_reg=slo * 128, elem_size=256,
                single_packet=False)
            ghhi = gh.tile([P, GHIMAX, 256], dt.bfloat16, tag="ghhi")
            nc.gpsimd.dma_gather(
                out_ap=ghhi[:, :shi, :], in_ap=hi_ap,
                idxs_ap=whi_sb[:, off_hi[g0] * 8:(off_hi[g0] + shi) * 8],
                num_idxs=shi * 128, num_idxs_reg=shi * 128, elem_size=256,
                single_packet=False)
            gh_tiles[gi] = (ghlo, ghhi)

        def emit_logits(j):
            KL = int(K_LO[j])
            KH = int(K_HI[j])
            K = KL + KH
            gi = pos_grp[j]
            g0 = g_starts[gi]
            ghlo, ghhi = gh_tiles[gi]
            lbase = off_lo[j] - off_lo[g0]
            hbase = off_hi[j] - off_hi[g0]

            # staircase a_dst per slot
            adp = psAD.tile([P, KMAX, 4], dt.float32, tag="adp", space="PSUM")
            rcol = off_r[j]
            for h, KHF, base in ((0, KL, 0), (1, KH, KL)):
                R = math.ceil(KHF / RND) if KHF else 0
                for r in range(R):
                    c0 = r * RND
                    nch = min(RND, KHF - c0)
                    sm = smp.tile([P, RND * 128], dt.float16, tag="sm")
                    nc.vector.tensor_scalar(
                        out=sm[:, :nch * 128], in0=iota2k[:, :nch * 128],
                        scalar1=bnd_sb[:, rcol:rcol + 1], scalar2=None,
                        op0=op.is_ge)
                    for jj in range(nch):
                        nc.tensor.matmul(
                            out=adp[:, base + c0 + jj, :],
                            lhsT=sm[:, jj * 128:(jj + 1) * 128],
                            rhs=delta4[:, j, :], start=True, stop=True)
                    rcol += 1

            # logits -> ex  (th cols 128:132 = fp32 bits of [as0 as1])
            t1 = tsp.tile([P, KMAX, 2], dt.float32, tag="t1")
            if KL:
                nc.vector.tensor_tensor(
                    out=t1[:, :KL, :],
                    in0=ghlo[:, lbase:lbase + KL, 128:132].bitcast(dt.float32),
                    in1=adp[:, :KL, 0:2], op=op.add)
            if KH:
                nc.vector.tensor_tensor(
                    out=t1[:, KL:K, :],
                    in0=ghhi[:, hbase:hbase + KH, 128:132].bitcast(dt.float32),
                    in1=adp[:, KL:K, 0:2], op=op.add)
            tsum = tsp.tile([P, KMAX, 2], dt.float32, tag="tsum")
            nc.vector.tensor_tensor(out=tsum[:, :K, :], in0=t1[:, :K, :],
                                    in1=adp[:, :K, 2:4], op=op.add)
            u02 = tsp.tile([P, KMAX, 2], dt.float32, tag="u02")
            nc.vector.tensor_scalar(out=u02[:, :K, :], in0=tsum[:, :K, :],
                                    scalar1=NEG_SLOPE, scalar2=None,
                                    op0=op.mult)
            lrt = tsp.tile([P, KMAX, 2], dt.float32, tag="lrt")
            nc.vector.tensor_tensor(out=lrt[:, :K, :], in0=tsum[:, :K, :],
                                    in1=u02[:, :K, :], op=op.max)
            ex = tsp.tile([P, KMAX, 2], dt.float32, tag="ex")
            nc.scalar.activation(out=ex[:, :K, :], in_=lrt[:, :K, :],
                                 func=act.Exp)
            ex_t[j] = ex

        def emit_chunks(j):
            KL = int(K_LO[j])
            KH = int(K_HI[j])
            K = KL + KH
            gi = pos_grp[j]
            g0 = g_starts[gi]
            ghlo, ghhi = gh_tiles[gi]
            lbase = off_lo[j] - off_lo[g0]
            hbase = off_hi[j] - off_hi[g0]
            ex = ex_t.pop(j)

            gtt = psGT.tile([P, HEADS * C], dt.float32, tag="gtt", space="PSUM")
            ss0 = psSS.tile([P, 1], dt.float32, tag="ss0", space="PSUM")
            ss1 = psSS.tile([P, 1], dt.float32, tag="ss1", space="PSUM")
            for jc in range(K):
                if jc < KL:
                    hgc = ghlo[:, lbase + jc, 0:128]
                else:
                    hgc = ghhi[:, hbase + (jc - KL), 0:128]
                st_ = jc == 0
                sp_ = jc == K - 1
                exm = exp_.tile([P, 2 * P], dt.bfloat16, tag="exm")
                for hd in range(HEADS):
                    nc.vector.tensor_scalar(
                        out=exm[:, hd * P:(hd + 1) * P], in0=iota_row[:],
                        scalar1=dpj_sb[:, off_k[j] + jc:off_k[j] + jc + 1],
                        scalar2=ex[:, jc, hd:hd + 1],
                        op0=op.is_equal, op1=op.mult)
                nc.tensor.matmul(out=gtt[:], lhsT=hgc, rhs=exm[:],
                                 start=st_, stop=sp_)
                nc.tensor.matmul(out=ss0[:], lhsT=exm[:, 0:P],
                                 rhs=ones_bf[:], start=st_, stop=sp_)
                nc.tensor.matmul(out=ss1[:], lhsT=exm[:, P:2 * P],
                                 rhs=ones_bf[:], start=st_, stop=sp_)

            # finalize position j
            rec = fin.tile([P, 2], dt.float32, tag="rec")
            nc.vector.reciprocal(out=rec[:, 0:1], in_=ss0[:])
            nc.vector.reciprocal(out=rec[:, 1:2], in_=ss1[:])
            gs = fin.tile([P, HEADS * C], dt.bfloat16, tag="gs")
            nc.scalar.copy(out=gs[:], in_=gtt[:])
            ot = fin.tile([P, HEADS * C], dt.bfloat16, tag="ot")
            for hd in range(HEADS):
                u = psU.tile([P, C], dt.float32, tag="u", space="PSUM")
                nc.tensor.matmul(out=u[:],
                                 lhsT=gs[:, hd * P:(hd + 1) * P],
                                 rhs=w_bf[:, hd * C:(hd + 1) * C],
                                 start=True, stop=True)
                nc.scalar.mul(out=ot[:, hd * C:(hd + 1) * C],
                              in_=u[:],
                              mul=rec[:, hd:hd + 1])
            zt = fin.tile([P, HEADS * C], dt.bfloat16, tag="zt")
            nc.gpsimd.tensor_tensor(out=zt[:], in0=ot[:], in1=bias_bf[:],
                                    op=op.add)
            et = fin.tile([P, HEADS * C], dt.bfloat16, tag="et")
            nc.scalar.activation(out=et[:], in_=zt[:], func=act.Exp)
            mt = fin.tile([P, HEADS * C], dt.bfloat16, tag="mt")
            nc.vector.tensor_scalar(out=mt[:], in0=et[:], scalar1=1.0,
                                    scalar2=-1.0, op0=op.min, op1=op.add)
            rt = fin.tile([P, HEADS * C], dt.bfloat16, tag="rt")
            nc.scalar.activation(out=rt[:], in_=zt[:], func=act.Relu)
            ob = fin.tile([P, HEADS * C], dt.bfloat16, tag="ob")
            nc.gpsimd.tensor_tensor(out=ob[:], in0=mt[:], in1=rt[:],
                                    op=op.add)
            obf = fin.tile([P, HEADS * C], dt.float32, tag="obf")
            nc.scalar.copy(out=obf[:], in_=ob[:])
            nc.sync.dma_start(out_t.ap()[j * P:(j + 1) * P, :], obf[:])

        emitted_gi = -1
        for j in range(POS):
            if pos_grp[j] > emitted_gi:
                emit_gather(pos_grp[j])
                emitted_gi = pos_grp[j]
            emit_logits(j)
            emit_chunks(j)

    nc.compile()
    return nc


def _get_program(K_LO, K_HI, R_LO, R_HI):
    key = (tuple(K_LO), tuple(K_HI))
    if key not in _CACHE:
        _CACHE[key] = _build(K_LO, K_HI, R_LO, R_HI)
    return _CACHE[key]


# ------------------------------------------------------------------- kernel
def kernel(h_node, edge_index, W, att_src, att_dst, bias):
    from concourse.bass_utils import run_bass_kernel_spmd

    h_node = np.asarray(h_node, dtype=np.float32)
    W = np.asarray(W, dtype=np.float32)
    att_src = np.asarray(att_src, dtype=np.float32)
    att_dst = np.asarray(att_dst, dtype=np.float32)
    bias = np.asarray(bias, dtype=np.float32).reshape(1, HEADS * C)

    pr = _prep(np.asarray(edge_index))
    nc = _get_program(pr["K_LO"], pr["K_HI"], pr["R_LO"], pr["R_HI"])

    hb = np.zeros((NROWS, 128), dtype=BF16)
    hb[:N] = h_node.astype(BF16)
    # p-major layout: row p*NTILE + t = node 128*t + p
    htab = np.ascontiguousarray(
        hb.reshape(NTILE, 128, 128).transpose(1, 0, 2)).reshape(128 * NTILE, 128)
    thh = np.zeros((NROWS, 256), dtype=BF16)
    thh[:, 0:128] = hb

    in_maps = []
    for c in range(NC_CORES):
        in_maps.append({
            "htab": htab, "th": thh, "w_in": W, "asrc_in": att_src,
            "adst_in": att_dst, "bias_in": bias,
            "wlo": pr["wlo"][c], "whi": pr["whi"][c], "dpj": pr["dpj"][c],
            "bnd": pr["bnd"][c], "bsel": pr["bsel"][c],
        })
    res = run_bass_kernel_spmd(nc, in_maps, core_ids=list(range(NC_CORES)))
    out = np.zeros((N, HEADS * C), dtype=np.float32)
    gmap = pr["gmap"]
    for c in range(NC_CORES):
        o = res.results[c]["out"]
        for j in range(POS):
            gg = gmap[c, j]
            if gg < 0:
                continue
            lo_n = 128 * gg
            hi_n = min(128 * (gg + 1), N)
            out[lo_n:hi_n] = o[j * 128:j * 128 + (hi_n - lo_n)]
    return out


# revision 9
# speedup vs baseline: 1.2076x; 1.0057x over previous
"""GAT layer (PyG GATConv eval, 2 heads x 128, self-loops, ELU) on 8 trn2 cores.

v2 design (dst-block sharded, rank-dealt, bf16 datapath):
  - ht table [50048, 256] bf16 in DRAM: cols 0:128 = bf16(h) (host-uploaded),
    cols 128:136 = a_src/a_dst logits as bf16 hi/lo pairs (device-computed in
    phase A).  One 512B-row dma_gather per edge fetches h AND the src logits.
  - Global dst blocks (128 nodes) are dealt to (core, position) slots by edge
    count rank so per-position chunk counts are uniform across cores (SPMD).
  - Edges sorted by (core, pos, src<32768, dst_local); per (pos, half) padded
    to 128-slot chunks.  Self loops ride the edge stream.
  - Per-slot a_dst via "staircase" matmul: SM[d, slot] = (slot >= first slot of
    dst d's run), adp = SM^T @ delta(a_dst) reconstructs a_dst[dst(slot)]
    exactly (fp16 hi/lo deltas).  No one-hot broadcast machinery.
  - exm one-hot masks in bf16 (4x DVE mode); gtt/ss/U matmuls in bf16.
  - Finalize: normalize on Act engine, ELU via exp/min/max identity.
"""
import math
from contextlib import ExitStack

import numpy as np
import ml_dtypes

BF16 = ml_dtypes.bfloat16
FP16 = np.float16

HEADS = 2
C = 128
IN = 128
N = 50000
NC_CORES = 8
NTILE = math.ceil(N / 128)        # 391 tiles / global blocks
NROWS = NTILE * 128               # 50048 table rows
POS = math.ceil(NTILE / NC_CORES)  # 49 positions per core
LO = 32768                        # lo table view rows [0, 32768)
HIBASE = 17280                    # hi table view rows [17280, 50048)
GRP = 4                           # positions per dma_gather call
RND = 16                          # max chunks per staircase round
NEG_SLOPE = 0.2
STAGE = 16                        # phase-A tiles per group

_CACHE = {}


# ----------------------------------------------------------------- host prep
def _prep(edge_index):
    src = np.concatenate([edge_index[0], np.arange(N)]).astype(np.int64)
    dst = np.concatenate([edge_index[1], np.arange(N)]).astype(np.int64)
    g = dst // 128
    dloc = dst % 128
    half = (src >= LO).astype(np.int64)

    sizes_g = np.bincount(g, minlength=NTILE)
    order_g = np.argsort(-sizes_g, kind="stable")
    gmap = np.full((NC_CORES, POS), -1, dtype=np.int64)
    for j in range(POS):
        for c in range(NC_CORES):
            r = NC_CORES * j + c
            if r < NTILE:
                gmap[c, j] = order_g[r]
    core_of = np.zeros(NTILE, dtype=np.int64)
    pos_of = np.zeros(NTILE, dtype=np.int64)
    for c in range(NC_CORES):
        for j in range(POS):
            gg = gmap[c, j]
            if gg >= 0:
                core_of[gg] = c
                pos_of[gg] = j

    ecore = core_of[g]
    epos = pos_of[g]

    # choose per-position lo/hi split M_j in [HIBASE, 32768] (hi table view
    # starts at row HIBASE so hi idx = src - HIBASE stays in int16) that
    # minimizes padded chunk count max_c ceil(lo/128) + max_c ceil(hi/128)
    cnt_all = np.zeros((NC_CORES, POS), dtype=np.int64)
    np.add.at(cnt_all, (ecore, epos), 1)
    cands = np.arange(HIBASE + 128, LO + 1, 512)
    lo_cnt = np.zeros((NC_CORES, POS, len(cands)), dtype=np.int64)
    for c in range(NC_CORES):
        for j in range(POS):
            sj = np.sort(src[(ecore == c) & (epos == j)])
            lo_cnt[c, j] = np.searchsorted(sj, cands)
    cost = (np.ceil(lo_cnt / 128).max(axis=0)
            + np.ceil((cnt_all[:, :, None] - lo_cnt) / 128).max(axis=0))
    M = cands[np.argmin(cost, axis=1)]                    # [POS]
    half = (src >= M[epos]).astype(np.int64)

    key = ((ecore * POS + epos) * 2 + half) * 128 + dloc
    order = np.argsort(key, kind="stable")
    src_s = src[order]
    dloc_s = dloc[order]
    ecore_s = ecore[order]
    epos_s = epos[order]
    half_s = half[order]

    cnt = np.zeros((NC_CORES, POS, 2), dtype=np.int64)
    np.add.at(cnt, (ecore_s, epos_s, half_s), 1)
    K_LO = np.ceil(cnt[:, :, 0].max(axis=0) / 128).astype(int)  # [POS]
    K_HI = np.ceil(cnt[:, :, 1].max(axis=0) / 128).astype(int)
    K_ALL = K_LO + K_HI
    SUM_LO = int(K_LO.sum())
    SUM_HI = int(K_HI.sum())
    SUM_K = int(K_ALL.sum())
    # rounds per (pos, half)
    R_LO = [math.ceil(k / RND) if k else 0 for k in K_LO]
    R_HI = [math.ceil(k / RND) if k else 0 for k in K_HI]
    SUM_R = int(sum(R_LO) + sum(R_HI))

    # group starts (of edges) per (core, pos, half)
    starts = np.zeros(NC_CORES * POS * 2 + 1, dtype=np.int64)
    np.cumsum(np.bincount(
        (ecore_s * POS + epos_s) * 2 + half_s,
        minlength=NC_CORES * POS * 2), out=starts[1:])

    # per-core tables
    wlo = np.zeros((NC_CORES, 128, SUM_LO * 8), dtype=np.int16)
    whi = np.zeros((NC_CORES, 128, SUM_HI * 8), dtype=np.int16)
    dpj = np.full((NC_CORES, 128, SUM_K), 999.0, dtype=np.float32)
    bnd = np.zeros((NC_CORES, 128, max(SUM_R, 1)), dtype=np.float32)
    bsel = np.zeros((NC_CORES, 128, 4), dtype=np.int16)  # 64 wrapped tile ids

    def wrap16(idx):
        """idx [n] (n % 128 == 0) -> wrapped [128, n // 16] int16."""
        n = len(idx)
        sl = idx.reshape(n // 16, 16).T            # [16, n/16]
        return np.broadcast_to(sl[None, :, :], (8, 16, n // 16)).reshape(
            128, n // 16).astype(np.int16)

    for c in range(NC_CORES):
        off_lo = 0
        off_hi = 0
        off_k = 0
        off_r = 0
        for j in range(POS):
            gg = gmap[c, j]
            for h in range(2):
                K = int((K_LO if h == 0 else K_HI)[j])
                nt = K * 128
                if gg >= 0:
                    s0 = starts[(c * POS + j) * 2 + h]
                    s1 = starts[(c * POS + j) * 2 + h + 1]
                    srcs = src_s[s0:s1]
                    dls = dloc_s[s0:s1]
                else:
                    srcs = np.zeros(0, dtype=np.int64)
                    dls = np.zeros(0, dtype=np.int64)
                n = len(srcs)
                assert n <= nt
                idx = np.zeros(nt, dtype=np.int64)
                idx[:n] = srcs - (HIBASE if h == 1 else 0)
                w = wrap16(idx)
                # slot i -> (chunk i//128, partition i%128)
                dv = np.full(nt, 999.0, dtype=np.float32)
                dv[:n] = dls
                dcol = dv.reshape(K, 128).T if K else np.zeros((128, 0), np.float32)
                # staircase boundaries per round
                first = np.searchsorted(dls, np.arange(128), side="left")  # [128]
                R = math.ceil(K / RND) if K else 0
                for r in range(R):
                    lo_c = r * RND * 128
                    ln = min(RND * 128, nt - lo_c)
                    b = np.clip(first - lo_c, 0, ln).astype(np.float32)
                    bnd[c, :, off_r + r] = b
                if h == 0:
                    wlo[c, :, off_lo * 8:(off_lo + K) * 8] = w
                    off_lo += K
                else:
                    whi[c, :, off_hi * 8:(off_hi + K) * 8] = w
                    off_hi += K
                dpj[c, :, off_k:off_k + K] = dcol
                off_k += K
                off_r += R
        gl = np.zeros(64, dtype=np.int64)
        gl[:POS] = np.maximum(gmap[c], 0)
        bsel[c] = wrap16(gl)
    return dict(gmap=gmap, K_LO=K_LO, K_HI=K_HI, R_LO=R_LO, R_HI=R_HI,
                SUM_LO=SUM_LO, SUM_HI=SUM_HI, SUM_K=SUM_K, SUM_R=SUM_R,
                wlo=wlo, whi=whi, dpj=dpj, bnd=bnd, bsel=bsel)


# ------------------------------------------------------------ device program
def _build(K_LO, K_HI, R_LO, R_HI):
    import concourse.bacc as bacc
    import concourse.bass as bass
    import concourse.mybir as mybir
    import concourse.tile as tile
    from concourse.masks import make_identity

    dt = mybir.dt
    op = mybir.AluOpType
    act = mybir.ActivationFunctionType
    P = 128
    SUM_LO = int(sum(K_LO))
    SUM_HI = int(sum(K_HI))
    SUM_K = SUM_LO + SUM_HI
    SUM_R = int(sum(R_LO) + sum(R_HI))
    KMAX = int(max(K_LO[j] + K_HI[j] for j in range(POS)))
    # gather groups: 4-wide, with a small tail so the last gather's compute
    # doesn't leave a long serial epilogue
    GRPS = [1, 1, 2]
    rem = POS - 4
    while rem > 5:
        GRPS.append(GRP)
        rem -= GRP
    while rem > 0:
        GRPS.append(min(2, rem) if rem > 1 else 1)
        rem -= GRPS[-1]
    g_starts = [int(sum(GRPS[:i])) for i in range(len(GRPS))]
    GLOMAX = max(int(sum(K_LO[g0:g0 + ng])) for g0, ng in zip(g_starts, GRPS))
    GHIMAX = max(int(sum(K_HI[g0:g0 + ng])) for g0, ng in zip(g_starts, GRPS))

    nc = bacc.Bacc("TRN2", target_bir_lowering=False, debug=False,
                   num_devices=NC_CORES)
    htab = nc.dram_tensor("htab", [128 * NTILE, 128], dt.bfloat16,
                          kind="ExternalInput")       # p-major bf16 h
    th = nc.dram_tensor("th", [NROWS, 256], dt.bfloat16,
                        kind="ExternalInput")         # node-major gather table
    w_in = nc.dram_tensor("w_in", [IN, HEADS * C], dt.float32, kind="ExternalInput")
    asrc_in = nc.dram_tensor("asrc_in", [HEADS, C], dt.float32, kind="ExternalInput")
    adst_in = nc.dram_tensor("adst_in", [HEADS, C], dt.float32, kind="ExternalInput")
    bias_in = nc.dram_tensor("bias_in", [1, HEADS * C], dt.float32, kind="ExternalInput")
    wlo_in = nc.dram_tensor("wlo", [128, SUM_LO * 8], dt.int16, kind="ExternalInput")
    whi_in = nc.dram_tensor("whi", [128, SUM_HI * 8], dt.int16, kind="ExternalInput")
    dpj_in = nc.dram_tensor("dpj", [128, SUM_K], dt.float32, kind="ExternalInput")
    bnd_in = nc.dram_tensor("bnd", [128, max(SUM_R, 1)], dt.float32, kind="ExternalInput")
    bsel_in = nc.dram_tensor("bsel", [128, 4], dt.int16, kind="ExternalInput")
    out_t = nc.dram_tensor("out", [POS * 128, HEADS * C], dt.float32,
                           kind="ExternalOutput")

    with tile.TileContext(nc) as tc, ExitStack() as ctx:
        const = ctx.enter_context(tc.tile_pool(name="const", bufs=1))

        # ---- constants
        ident_bf = const.tile([P, P], dt.bfloat16)
        make_identity(nc, ident_bf[:])
        iota_row = const.tile([P, P], dt.bfloat16)
        nc.gpsimd.iota(iota_row[:], pattern=[[1, P]], base=0, channel_multiplier=0,
                       allow_small_or_imprecise_dtypes=True)
        iota2k = const.tile([P, RND * 128], dt.float16)
        nc.gpsimd.iota(iota2k[:], pattern=[[1, RND * 128]], base=0,
                       channel_multiplier=0, allow_small_or_imprecise_dtypes=True)
        ones_bf = const.tile([P, 1], dt.bfloat16)
        nc.gpsimd.memset(ones_bf[:], 1.0)
        iota_cp1 = const.tile([P, 1], dt.float32)
        nc.gpsimd.iota(iota_cp1[:], pattern=[[0, 1]], base=1, channel_multiplier=1,
                       allow_small_or_imprecise_dtypes=True)
        shiftmat = const.tile([P, P], dt.float32)
        nc.vector.tensor_scalar(out=shiftmat[:], in0=iota_row[:],
                                scalar1=iota_cp1[:], scalar2=None, op0=op.is_equal)
        w_sb = const.tile([P, HEADS * C], dt.float32)
        nc.sync.dma_start(w_sb[:], w_in.ap()[:, :])
        w_bf = const.tile([P, HEADS * C], dt.bfloat16)
        nc.vector.tensor_scalar(out=w_bf[:], in0=w_sb[:], scalar1=0.0,
                                scalar2=None, op0=op.add)
        bias_bf = const.tile([P, HEADS * C], dt.bfloat16)
        bias_f32 = const.tile([P, HEADS * C], dt.float32)
        nc.sync.dma_start(bias_f32[:], bass.AP(bias_in, 0, [[0, P], [1, HEADS * C]]))
        nc.vector.tensor_scalar(out=bias_bf[:], in0=bias_f32[:], scalar1=0.0,
                                scalar2=None, op0=op.add)

        # wa4[k, i] = sum_c W[k, h*C+c]*att[h, c]; cols: as0 as1 ad0 ad1
        wa4 = const.tile([P, 4], dt.float32)
        wa4hl = const.tile([P, 8], dt.bfloat16)   # [hi0..hi3, lo0..lo3]
        with tc.tile_pool(name="watmp", bufs=2) as tmp_pool:
            for jat, attt in enumerate((asrc_in, adst_in)):
                abc = tmp_pool.tile([P, HEADS * C], dt.float32, tag="abc")
                nc.sync.dma_start(abc[:], bass.AP(attt, 0, [[0, P], [1, HEADS * C]]))
                t = tmp_pool.tile([P, HEADS * C], dt.float32, tag="t")
                nc.vector.tensor_tensor(out=t[:], in0=w_sb[:], in1=abc[:],
                                        op=op.mult)
                for hd in range(HEADS):
                    nc.vector.tensor_reduce(
                        out=wa4[:, 2 * jat + hd:2 * jat + hd + 1],
                        in_=t[:, hd * C:(hd + 1) * C],
                        axis=mybir.AxisListType.X, op=op.add)
            nc.vector.tensor_scalar(out=wa4hl[:, 0:4], in0=wa4[:], scalar1=0.0,
                                    scalar2=None, op0=op.add)
            hic = tmp_pool.tile([P, 4], dt.float32, tag="hic")
            nc.vector.tensor_scalar(out=hic[:], in0=wa4hl[:, 0:4], scalar1=0.0,
                                    scalar2=None, op0=op.add)
            lo32 = tmp_pool.tile([P, 4], dt.float32, tag="lo32")
            nc.vector.tensor_tensor(out=lo32[:], in0=wa4[:], in1=hic[:],
                                    op=op.subtract)
            nc.vector.tensor_scalar(out=wa4hl[:, 4:8], in0=lo32[:], scalar1=0.0,
                                    scalar2=None, op0=op.add)

        # ---- phase A: write a_src/a_dst hi/lo into th[:, 128:136]
        adall = const.tile([P, NTILE, 2], dt.float32)
        ctxA = ExitStack()
        sbA = ctxA.enter_context(tc.tile_pool(name="sbA", bufs=2))
        psT = ctxA.enter_context(tc.tile_pool(name="psT", bufs=2, space="PSUM"))
        psA8 = ctxA.enter_context(tc.tile_pool(name="psA8", bufs=2, space="PSUM"))
        stgA = ctxA.enter_context(tc.tile_pool(name="stgA", bufs=2))

        for t0 in range(0, NTILE, STAGE):
            nst = min(STAGE, NTILE - t0)
            htile = sbA.tile([P, STAGE, 128], dt.bfloat16, tag="htile")
            nc.sync.dma_start(
                htile[:, :nst, :],
                bass.AP(htab, t0 * 128, [[NTILE * 128, P], [128, nst], [1, 128]]))
            tp = psT.tile([P, STAGE * 128], dt.bfloat16, tag="tp", space="PSUM")
            for gi in range(nst):
                nc.tensor.transpose(out=tp[:, gi * 128:(gi + 1) * 128],
                                    in_=htile[:, gi, :], identity=ident_bf[:])
            hT = sbA.tile([P, STAGE * 128], dt.bfloat16, tag="hT")
            hh = (nst // 2) * 128
            nc.vector.tensor_scalar(out=hT[:, :hh], in0=tp[:, :hh],
                                    scalar1=0.0, scalar2=None, op0=op.add)
            nc.scalar.copy(out=hT[:, hh:nst * 128], in_=tp[:, hh:nst * 128])
            a8 = psA8.tile([P, STAGE, 8], dt.float32, tag="a8", space="PSUM")
            for gi in range(nst):
                nc.tensor.matmul(out=a8[:, gi, :],
                                 lhsT=hT[:, gi * 128:(gi + 1) * 128],
                                 rhs=wa4hl[:], start=True, stop=True)
            a8s = stgA.tile([P, STAGE, 8], dt.float32, tag="a8s")
            nc.vector.tensor_scalar(out=a8s[:, :nst, :], in0=a8[:, :nst, :],
                                    scalar1=0.0, scalar2=None, op0=op.add)
            a4g = stgA.tile([P, STAGE, 4], dt.float32, tag="a4g")
            nc.vector.tensor_tensor(out=a4g[:, :nst, :], in0=a8s[:, :nst, 0:4],
                                    in1=a8s[:, :nst, 4:8], op=op.add)
            nc.vector.tensor_scalar(out=adall[:, t0:t0 + nst, :],
                                    in0=a4g[:, :nst, 2:4],
                                    scalar1=0.0, scalar2=None, op0=op.add)
            # th cols 128:136 hold raw fp32 bits of [as0 as1 ad0 ad1]
            nc.scalar.dma_start(
                bass.AP(th, (128 * t0) * 256 + 128,
                        [[256, P], [128 * 256, nst], [1, 8]]),
                a4g[:, :nst, :].bitcast(dt.bfloat16))
        ctxA.close()

        # ---- select this core's blocks' a_dst from adall (no DRAM roundtrip)
        bgp = ExitStack()
        bgpool = bgp.enter_context(tc.tile_pool(name="bgpool", bufs=1))
        lo_ap = bass.AP(th, 0, [[256, LO], [1, 256]])
        hi_ap = bass.AP(th, HIBASE * 256, [[256, NROWS - HIBASE], [1, 256]])
        bgidx = bgpool.tile([P, 4], dt.int16, tag="bgidx")
        nc.sync.dma_start(bgidx[:], bsel_in.ap()[:, :])
        adsel = bgpool.tile([P, 64, 2], dt.float32, tag="adsel")
        nc.gpsimd.ap_gather(out_ap=adsel[:], in_ap=adall[:], idxs_ap=bgidx[:],
                            channels=128, num_elems=NTILE, d=2, num_idxs=64)
        adf = bgpool.tile([P, POS, 2], dt.float32, tag="adf")
        nc.vector.tensor_copy(out=adf[:], in_=adsel[:, :POS, :])
        # shifted[d] = a_dst[d-1] via shift-matrix matmul (exact in fp32 psum)
        psBG = bgp.enter_context(tc.tile_pool(name="psBG", bufs=1, space="PSUM"))
        sh4 = psBG.tile([P, POS, 2], dt.float32, tag="sh4", space="PSUM")
        nc.tensor.matmul(out=sh4[:], lhsT=shiftmat[:], rhs=adf[:],
                         start=True, stop=True)
        shf = bgpool.tile([P, POS, 2], dt.float32, tag="shf")
        nc.vector.tensor_scalar(out=shf[:], in0=sh4[:], scalar1=0.0,
                                scalar2=None, op0=op.add)
        dlt = bgpool.tile([P, POS, 2], dt.float32, tag="dlt")
        nc.vector.tensor_tensor(out=dlt[:], in0=adf[:], in1=shf[:],
                                op=op.subtract)
        delta4 = const.tile([P, POS, 4], dt.float16)
        nc.vector.tensor_scalar(out=delta4[:, :, 0:2], in0=dlt[:],
                                scalar1=0.0, scalar2=None, op0=op.add)
        dhc = bgpool.tile([P, POS, 2], dt.float32, tag="dhc")
        nc.vector.tensor_scalar(out=dhc[:], in0=delta4[:, :, 0:2],
                                scalar1=0.0, scalar2=None, op0=op.add)
        dlo = bgpool.tile([P, POS, 2], dt.float32, tag="dlo")
        nc.vector.tensor_tensor(out=dlo[:], in0=dlt[:], in1=dhc[:],
                                op=op.subtract)
        nc.vector.tensor_scalar(out=delta4[:, :, 2:4], in0=dlo[:],
                                scalar1=0.0, scalar2=None, op0=op.add)
        bgp.close()

        # ---- phase B preloads
        wlo_sb = const.tile([P, SUM_LO * 8], dt.int16)
        nc.sync.dma_start(wlo_sb[:], wlo_in.ap()[:, :])
        whi_sb = const.tile([P, SUM_HI * 8], dt.int16)
        nc.sync.dma_start(whi_sb[:], whi_in.ap()[:, :])
        dpj_sb = const.tile([P, SUM_K], dt.float32)
        nc.sync.dma_start(dpj_sb[:], dpj_in.ap()[:, :])
        bnd_sb = const.tile([P, max(SUM_R, 1)], dt.float32)
        nc.sync.dma_start(bnd_sb[:], bnd_in.ap()[:, :])

        gh = ctx.enter_context(tc.tile_pool(name="gh", bufs=2))
        smp = ctx.enter_context(tc.tile_pool(name="smp", bufs=3))
        exp_ = ctx.enter_context(tc.tile_pool(name="exp", bufs=4))
        tsp = ctx.enter_context(tc.tile_pool(name="tsp", bufs=3))
        fin = ctx.enter_context(tc.tile_pool(name="fin", bufs=3))
        psGT = ctx.enter_context(tc.tile_pool(name="psGT", bufs=2, space="PSUM"))
        psSS = ctx.enter_context(tc.tile_pool(name="psSS", bufs=1, space="PSUM"))
        psAD = ctx.enter_context(tc.tile_pool(name="psAD", bufs=2, space="PSUM"))
        psU = ctx.enter_context(tc.tile_pool(name="psU", bufs=1, space="PSUM"))

        off_lo = [int(sum(K_LO[:j])) for j in range(POS + 1)]
        off_hi = [int(sum(K_HI[:j])) for j in range(POS + 1)]
        off_k = [int(sum(K_LO[:j]) + sum(K_HI[:j])) for j in range(POS + 1)]
        off_r = [0]
        for j in range(POS):
            off_r.append(off_r[-1] + R_LO[j] + R_HI[j])

        # software pipeline: emit logits(j) ahead of chunks(j-1) so the DVE
        # queue never head-of-line blocks on ex (Act) readiness
        pos_grp = []
        for gi, ng in enumerate(GRPS):
            pos_grp += [gi] * ng
        gh_tiles = {}
        ex_t = {}

        def emit_gather(gi):
            g0, ng = g_starts[gi], GRPS[gi]
            slo = off_lo[g0 + ng] - off_lo[g0]
            shi = off_hi[g0 + ng] - off_hi[g0]
            ghlo = gh.tile([P, GLOMAX, 256], dt.bfloat16, tag="ghlo")
            nc.gpsimd.dma_gather(
                out_ap=ghlo[:, :slo, :], in_ap=lo_ap,
                idxs_ap=wlo_sb[:, off_lo[g0] * 8:(off_lo[g0] + slo) * 8],
                num_idxs=slo * 128, num_idxs_reg=slo * 128, elem_size=256,
                single_packet=False)
            ghhi = gh.tile([P, GHIMAX, 256], dt.bfloat16, tag="ghhi")
            nc.gpsimd.dma_gather(
                out_ap=ghhi[:, :shi, :], in_ap=hi_ap,
                idxs_ap=whi_sb[:, off_hi[g0] * 8:(off_hi[g0] + shi) * 8],
                num_idxs=shi * 128, num_idxs_reg=shi * 128, elem_size=256,
                single_packet=False)
            gh_tiles[gi] = (ghlo, ghhi)

        def emit_logits(j):
            KL = int(K_LO[j])
            KH = int(K_HI[j])
            K = KL + KH
            gi = pos_grp[j]
            g0 = g_starts[gi]
            ghlo, ghhi = gh_tiles[gi]
            lbase = off_lo[j] - off_lo[g0]
            hbase = off_hi[j] - off_hi[g0]

            # staircase a_dst per slot
            adp = psAD.tile([P, KMAX, 4], dt.float32, tag="adp", space="PSUM")
            rcol = off_r[j]
            for h, KHF, base in ((0, KL, 0), (1, KH, KL)):
                R = math.ceil(KHF / RND) if KHF else 0
                for r in range(R):
                    c0 = r * RND
                    nch = min(RND, KHF - c0)
                    sm = smp.tile([P, RND * 128], dt.float16, tag="sm")
                    nc.vector.tensor_scalar(
                        out=sm[:, :nch * 128], in0=iota2k[:, :nch * 128],
                        scalar1=bnd_sb[:, rcol:rcol + 1], scalar2=None,
                        op0=op.is_ge)
                    for jj in range(nch):
                        nc.tensor.matmul(
                            out=adp[:, base + c0 + jj, :],
                            lhsT=sm[:, jj * 128:(jj + 1) * 128],
                            rhs=delta4[:, j, :], start=True, stop=True)
                    rcol += 1

            # logits -> ex  (th cols 128:132 = fp32 bits of [as0 as1])
            t1 = tsp.tile([P, KMAX, 2], dt.float32, tag="t1")
            if KL:
                nc.vector.tensor_tensor(
                    out=t1[:, :KL, :],
                    in0=ghlo[:, lbase:lbase + KL, 128:132].bitcast(dt.float32),
                    in1=adp[:, :KL, 0:2], op=op.add)
            if KH:
                nc.vector.tensor_tensor(
                    out=t1[:, KL:K, :],
                    in0=ghhi[:, hbase:hbase + KH, 128:132].bitcast(dt.float32),
                    in1=adp[:, KL:K, 0:2], op=op.add)
            tsum = tsp.tile([P, KMAX, 2], dt.float32, tag="tsum")
            nc.vector.tensor_tensor(out=tsum[:, :K, :], in0=t1[:, :K, :],
                                    in1=adp[:, :K, 2:4], op=op.add)
            u02 = tsp.tile([P, KMAX, 2], dt.float32, tag="u02")
            nc.vector.tensor_scalar(out=u02[:, :K, :], in0=tsum[:, :K, :],
                                    scalar1=NEG_SLOPE, scalar2=None,
                                    op0=op.mult)
            lrt = tsp.tile([P, KMAX, 2], dt.float32, tag="lrt")
            nc.vector.tensor_tensor(out=lrt[:, :K, :], in0=tsum[:, :K, :],
                                    in1=u02[:, :K, :], op=op.max)
            ex = tsp.tile([P, KMAX, 2], dt.float32, tag="ex")
            nc.scalar.activation(out=ex[:, :K, :], in_=lrt[:, :K, :],
                                 func=act.Exp)
            ex_t[j] = ex

        def emit_chunks(j):
            KL = int(K_LO[j])
            KH = int(K_HI[j])
            K = KL + KH
            gi = pos_grp[j]
            g0 = g_starts[gi]
            ghlo, ghhi = gh_tiles[gi]
            lbase = off_lo[j] - off_lo[g0]
            hbase = off_hi[j] - off_hi[g0]
            ex = ex_t.pop(j)

            gtt = psGT.tile([P, HEADS * C], dt.float32, tag="gtt", space="PSUM")
            ss0 = psSS.tile([P, 1], dt.float32, tag="ss0", space="PSUM")
            ss1 = psSS.tile([P, 1], dt.float32, tag="ss1", space="PSUM")
            for jc in range(K):
                if jc < KL:
                    hgc = ghlo[:, lbase + jc, 0:128]
                else:
                    hgc = ghhi[:, hbase + (jc - KL), 0:128]
                st_ = jc == 0
                sp_ = jc == K - 1
                exm = exp_.tile([P, 2 * P], dt.bfloat16, tag="exm")
                for hd in range(HEADS):
                    nc.vector.tensor_scalar(
                        out=exm[:, hd * P:(hd + 1) * P], in0=iota_row[:],
                        scalar1=dpj_sb[:, off_k[j] + jc:off_k[j] + jc + 1],
                        scalar2=ex[:, jc, hd:hd + 1],
                        op0=op.is_equal, op1=op.mult)
                nc.tensor.matmul(out=gtt[:], lhsT=hgc, rhs=exm[:],
                                 start=st_, stop=sp_)
                nc.tensor.matmul(out=ss0[:], lhsT=exm[:, 0:P],
                                 rhs=ones_bf[:], start=st_, stop=sp_)
                nc.tensor.matmul(out=ss1[:], lhsT=exm[:, P:2 * P],
                                 rhs=ones_bf[:], start=st_, stop=sp_)

            # finalize position j
            rec = fin.tile([P, 2], dt.float32, tag="rec")
            nc.vector.reciprocal(out=rec[:, 0:1], in_=ss0[:])
            nc.vector.reciprocal(out=rec[:, 1:2], in_=ss1[:])
            gs = fin.tile([P, HEADS * C], dt.bfloat16, tag="gs")
            nc.scalar.copy(out=gs[:], in_=gtt[:])
            ot = fin.tile([P, HEADS * C], dt.bfloat16, tag="ot")
            for hd in range(HEADS):
                u = psU.tile([P, C], dt.float32, tag="u", space="PSUM")
                nc.tensor.matmul(out=u[:],
                                 lhsT=gs[:, hd * P:(hd + 1) * P],
                                 rhs=w_bf[:, hd * C:(hd + 1) * C],
                                 start=True, stop=True)
                nc.scalar.mul(out=ot[:, hd * C:(hd + 1) * C],
                              in_=u[:],
                              mul=rec[:, hd:hd + 1])
            zt = fin.tile([P, HEADS * C], dt.bfloat16, tag="zt")
            nc.gpsimd.tensor_tensor(out=zt[:], in0=ot[:], in1=bias_bf[:],
                                    op=op.add)
            et = fin.tile([P, HEADS * C], dt.bfloat16, tag="et")
            nc.scalar.activation(out=et[:], in_=zt[:], func=act.Exp)
            mt = fin.tile([P, HEADS * C], dt.bfloat16, tag="mt")
            nc.vector.tensor_scalar(out=mt[:], in0=et[:], scalar1=1.0,
                                    scalar2=-1.0, op0=op.min, op1=op.add)
            rt = fin.tile([P, HEADS * C], dt.bfloat16, tag="rt")
            nc.scalar.activation(out=rt[:], in_=zt[:], func=act.Relu)
            ob = fin.tile([P, HEADS * C], dt.bfloat16, tag="ob")
            nc.gpsimd.tensor_tensor(out=ob[:], in0=mt[:], in1=rt[:],
                                    op=op.add)
            obf = fin.tile([P, HEADS * C], dt.float32, tag="obf")
            nc.scalar.copy(out=obf[:], in_=ob[:])
            nc.sync.dma_start(out_t.ap()[j * P:(j + 1) * P, :], obf[:])

        emitted_gi = -1
        for j in range(POS):
            if pos_grp[j] > emitted_gi:
                emit_gather(pos_grp[j])
                emitted_gi = pos_grp[j]
            emit_logits(j)
            emit_chunks(j)

    nc.compile()
    return nc


def _get_program(K_LO, K_HI, R_LO, R_HI):
    key = (tuple(K_LO), tuple(K_HI))
    if key not in _CACHE:
        _CACHE[key] = _build(K_LO, K_HI, R_LO, R_HI)
    return _CACHE[key]


# ------------------------------------------------------------------- kernel
def kernel(h_node, edge_index, W, att_src, att_dst, bias):
    from concourse.bass_utils import run_bass_kernel_spmd

    h_node = np.asarray(h_node, dtype=np.float32)
    W = np.asarray(W, dtype=np.float32)
    att_src = np.asarray(att_src, dtype=np.float32)
    att_dst = np.asarray(att_dst, dtype=np.float32)
    bias = np.asarray(bias, dtype=np.float32).reshape(1, HEADS * C)

    pr = _prep(np.asarray(edge_index))
    nc = _get_program(pr["K_LO"], pr["K_HI"], pr["R_LO"], pr["R_HI"])

    hb = np.zeros((NROWS, 128), dtype=BF16)
    hb[:N] = h_node.astype(BF16)
    # p-major layout: row p*NTILE + t = node 128*t + p
    htab = np.ascontiguousarray(
        hb.reshape(NTILE, 128, 128).transpose(1, 0, 2)).reshape(128 * NTILE, 128)
    thh = np.zeros((NROWS, 256), dtype=BF16)
    thh[:, 0:128] = hb

    in_maps = []
    for c in range(NC_CORES):
        in_maps.append({
            "htab": htab, "th": thh, "w_in": W, "asrc_in": att_src,
            "adst_in": att_dst, "bias_in": bias,
            "wlo": pr["wlo"][c], "whi": pr["whi"][c], "dpj": pr["dpj"][c],
            "bnd": pr["bnd"][c], "bsel": pr["bsel"][c],
        })
    res = run_bass_kernel_spmd(nc, in_maps, core_ids=list(range(NC_CORES)))
    out = np.zeros((N, HEADS * C), dtype=np.float32)
    gmap = pr["gmap"]
    for c in range(NC_CORES):
        o = res.results[c]["out"]
        for j in range(POS):
            gg = gmap[c, j]
            if gg < 0:
                continue
            lo_n = 128 * gg
            hi_n = min(128 * (gg + 1), N)
            out[lo_n:hi_n] = o[j * 128:j * 128 + (hi_n - lo_n)]
    return out


# revision 10
# speedup vs baseline: 1.2301x; 1.0187x over previous
"""GAT layer (PyG GATConv eval, 2 heads x 128, self-loops, ELU) on 8 trn2 cores.

v2 design (dst-block sharded, rank-dealt, bf16 datapath):
  - ht table [50048, 256] bf16 in DRAM: cols 0:128 = bf16(h) (host-uploaded),
    cols 128:136 = a_src/a_dst logits as bf16 hi/lo pairs (device-computed in
    phase A).  One 512B-row dma_gather per edge fetches h AND the src logits.
  - Global dst blocks (128 nodes) are dealt to (core, position) slots by edge
    count rank so per-position chunk counts are uniform across cores (SPMD).
  - Edges sorted by (core, pos, src<32768, dst_local); per (pos, half) padded
    to 128-slot chunks.  Self loops ride the edge stream.
  - Per-slot a_dst via "staircase" matmul: SM[d, slot] = (slot >= first slot of
    dst d's run), adp = SM^T @ delta(a_dst) reconstructs a_dst[dst(slot)]
    exactly (fp16 hi/lo deltas).  No one-hot broadcast machinery.
  - exm one-hot masks in bf16 (4x DVE mode); gtt/ss/U matmuls in bf16.
  - Finalize: normalize on Act engine, ELU via exp/min/max identity.
"""
import math
from contextlib import ExitStack

import numpy as np
import ml_dtypes

BF16 = ml_dtypes.bfloat16
FP16 = np.float16

HEADS = 2
C = 128
IN = 128
N = 50000
NC_CORES = 8
NTILE = math.ceil(N / 128)        # 391 tiles / global blocks
NROWS = NTILE * 128               # 50048 table rows
POS = math.ceil(NTILE / NC_CORES)  # 49 positions per core
LO = 32768                        # lo table view rows [0, 32768)
HIBASE = 17280                    # hi table view rows [17280, 50048)
GRP = 4                           # positions per dma_gather call
RND = 16                          # max chunks per staircase round
NEG_SLOPE = 0.2
STAGE = 16                        # phase-A tiles per group

_CACHE = {}


# ----------------------------------------------------------------- host prep
def _prep(edge_index):
    src = np.concatenate([edge_index[0], np.arange(N)]).astype(np.int64)
    dst = np.concatenate([edge_index[1], np.arange(N)]).astype(np.int64)
    g = dst // 128
    dloc = dst % 128
    half = (src >= LO).astype(np.int64)

    sizes_g = np.bincount(g, minlength=NTILE)
    order_g = np.argsort(-sizes_g, kind="stable")
    gmap = np.full((NC_CORES, POS), -1, dtype=np.int64)
    for j in range(POS):
        for c in range(NC_CORES):
            r = NC_CORES * j + c
            if r < NTILE:
                gmap[c, j] = order_g[r]
    core_of = np.zeros(NTILE, dtype=np.int64)
    pos_of = np.zeros(NTILE, dtype=np.int64)
    for c in range(NC_CORES):
        for j in range(POS):
            gg = gmap[c, j]
            if gg >= 0:
                core_of[gg] = c
                pos_of[gg] = j

    ecore = core_of[g]
    epos = pos_of[g]

    # choose per-position lo/hi split M_j in [HIBASE, 32768] (hi table view
    # starts at row HIBASE so hi idx = src - HIBASE stays in int16) that
    # minimizes padded chunk count max_c ceil(lo/128) + max_c ceil(hi/128)
    cnt_all = np.zeros((NC_CORES, POS), dtype=np.int64)
    np.add.at(cnt_all, (ecore, epos), 1)
    cands = np.arange(HIBASE + 128, LO + 1, 512)
    lo_cnt = np.zeros((NC_CORES, POS, len(cands)), dtype=np.int64)
    for c in range(NC_CORES):
        for j in range(POS):
            sj = np.sort(src[(ecore == c) & (epos == j)])
            lo_cnt[c, j] = np.searchsorted(sj, cands)
    cost = (np.ceil(lo_cnt / 128).max(axis=0)
            + np.ceil((cnt_all[:, :, None] - lo_cnt) / 128).max(axis=0))
    M = cands[np.argmin(cost, axis=1)]                    # [POS]
    half = (src >= M[epos]).astype(np.int64)

    key = ((ecore * POS + epos) * 2 + half) * 128 + dloc
    order = np.argsort(key, kind="stable")
    src_s = src[order]
    dloc_s = dloc[order]
    ecore_s = ecore[order]
    epos_s = epos[order]
    half_s = half[order]

    cnt = np.zeros((NC_CORES, POS, 2), dtype=np.int64)
    np.add.at(cnt, (ecore_s, epos_s, half_s), 1)
    K_LO = np.ceil(cnt[:, :, 0].max(axis=0) / 128).astype(int)  # [POS]
    K_HI = np.ceil(cnt[:, :, 1].max(axis=0) / 128).astype(int)
    K_ALL = K_LO + K_HI
    SUM_LO = int(K_LO.sum())
    SUM_HI = int(K_HI.sum())
    SUM_K = int(K_ALL.sum())
    # rounds per (pos, half)
    R_LO = [math.ceil(k / RND) if k else 0 for k in K_LO]
    R_HI = [math.ceil(k / RND) if k else 0 for k in K_HI]
    SUM_R = int(sum(R_LO) + sum(R_HI))

    # group starts (of edges) per (core, pos, half)
    starts = np.zeros(NC_CORES * POS * 2 + 1, dtype=np.int64)
    np.cumsum(np.bincount(
        (ecore_s * POS + epos_s) * 2 + half_s,
        minlength=NC_CORES * POS * 2), out=starts[1:])

    # per-core tables
    wlo = np.zeros((NC_CORES, 128, SUM_LO * 8), dtype=np.int16)
    whi = np.zeros((NC_CORES, 128, SUM_HI * 8), dtype=np.int16)
    dpj = np.full((NC_CORES, 128, SUM_K), 999.0, dtype=np.float32)
    bnd = np.zeros((NC_CORES, 128, max(SUM_R, 1)), dtype=np.float32)
    bsel = np.zeros((NC_CORES, 128, 4), dtype=np.int16)  # 64 wrapped tile ids

    def wrap16(idx):
        """idx [n] (n % 128 == 0) -> wrapped [128, n // 16] int16."""
        n = len(idx)
        sl = idx.reshape(n // 16, 16).T            # [16, n/16]
        return np.broadcast_to(sl[None, :, :], (8, 16, n // 16)).reshape(
            128, n // 16).astype(np.int16)

    for c in range(NC_CORES):
        off_lo = 0
        off_hi = 0
        off_k = 0
        off_r = 0
        for j in range(POS):
            gg = gmap[c, j]
            for h in range(2):
                K = int((K_LO if h == 0 else K_HI)[j])
                nt = K * 128
                if gg >= 0:
                    s0 = starts[(c * POS + j) * 2 + h]
                    s1 = starts[(c * POS + j) * 2 + h + 1]
                    srcs = src_s[s0:s1]
                    dls = dloc_s[s0:s1]
                else:
                    srcs = np.zeros(0, dtype=np.int64)
                    dls = np.zeros(0, dtype=np.int64)
                n = len(srcs)
                assert n <= nt
                idx = np.zeros(nt, dtype=np.int64)
                idx[:n] = srcs - (HIBASE if h == 1 else 0)
                w = wrap16(idx)
                # slot i -> (chunk i//128, partition i%128)
                dv = np.full(nt, 999.0, dtype=np.float32)
                dv[:n] = dls
                dcol = dv.reshape(K, 128).T if K else np.zeros((128, 0), np.float32)
                # staircase boundaries per round
                first = np.searchsorted(dls, np.arange(128), side="left")  # [128]
                R = math.ceil(K / RND) if K else 0
                for r in range(R):
                    lo_c = r * RND * 128
                    ln = min(RND * 128, nt - lo_c)
                    b = np.clip(first - lo_c, 0, ln).astype(np.float32)
                    bnd[c, :, off_r + r] = b
                if h == 0:
                    wlo[c, :, off_lo * 8:(off_lo + K) * 8] = w
                    off_lo += K
                else:
                    whi[c, :, off_hi * 8:(off_hi + K) * 8] = w
                    off_hi += K
                dpj[c, :, off_k:off_k + K] = dcol
                off_k += K
                off_r += R
        gl = np.zeros(64, dtype=np.int64)
        gl[:POS] = np.maximum(gmap[c], 0)
        bsel[c] = wrap16(gl)
    return dict(gmap=gmap, K_LO=K_LO, K_HI=K_HI, R_LO=R_LO, R_HI=R_HI,
                SUM_LO=SUM_LO, SUM_HI=SUM_HI, SUM_K=SUM_K, SUM_R=SUM_R,
                wlo=wlo, whi=whi, dpj=dpj, bnd=bnd, bsel=bsel)


# ------------------------------------------------------------ device program
def _build(K_LO, K_HI, R_LO, R_HI):
    import concourse.bacc as bacc
    import concourse.bass as bass
    import concourse.mybir as mybir
    import concourse.tile as tile
    from concourse.masks import make_identity

    dt = mybir.dt
    op = mybir.AluOpType
    act = mybir.ActivationFunctionType
    P = 128
    SUM_LO = int(sum(K_LO))
    SUM_HI = int(sum(K_HI))
    SUM_K = SUM_LO + SUM_HI
    SUM_R = int(sum(R_LO) + sum(R_HI))
    KMAX = int(max(K_LO[j] + K_HI[j] for j in range(POS)))
    # gather groups: 4-wide, with a small tail so the last gather's compute
    # doesn't leave a long serial epilogue
    GRPS = [1, 1, 2]
    rem = POS - 4
    while rem > 5:
        GRPS.append(GRP)
        rem -= GRP
    while rem > 0:
        GRPS.append(min(2, rem) if rem > 1 else 1)
        rem -= GRPS[-1]
    g_starts = [int(sum(GRPS[:i])) for i in range(len(GRPS))]
    GLOMAX = max(int(sum(K_LO[g0:g0 + ng])) for g0, ng in zip(g_starts, GRPS))
    GHIMAX = max(int(sum(K_HI[g0:g0 + ng])) for g0, ng in zip(g_starts, GRPS))

    nc = bacc.Bacc("TRN2", target_bir_lowering=False, debug=False,
                   num_devices=NC_CORES)
    htab = nc.dram_tensor("htab", [128 * NTILE, 128], dt.bfloat16,
                          kind="ExternalInput")       # p-major bf16 h
    th = nc.dram_tensor("th", [NROWS, 256], dt.bfloat16,
                        kind="ExternalInput")         # node-major gather table
    w_in = nc.dram_tensor("w_in", [IN, HEADS * C], dt.float32, kind="ExternalInput")
    asrc_in = nc.dram_tensor("asrc_in", [HEADS, C], dt.float32, kind="ExternalInput")
    adst_in = nc.dram_tensor("adst_in", [HEADS, C], dt.float32, kind="ExternalInput")
    bias_in = nc.dram_tensor("bias_in", [1, HEADS * C], dt.float32, kind="ExternalInput")
    wlo_in = nc.dram_tensor("wlo", [128, SUM_LO * 8], dt.int16, kind="ExternalInput")
    whi_in = nc.dram_tensor("whi", [128, SUM_HI * 8], dt.int16, kind="ExternalInput")
    dpj_in = nc.dram_tensor("dpj", [128, SUM_K], dt.float32, kind="ExternalInput")
    bnd_in = nc.dram_tensor("bnd", [128, max(SUM_R, 1)], dt.float32, kind="ExternalInput")
    bsel_in = nc.dram_tensor("bsel", [128, 4], dt.int16, kind="ExternalInput")
    out_t = nc.dram_tensor("out", [POS * 128, HEADS * C], dt.float32,
                           kind="ExternalOutput")

    with tile.TileContext(nc) as tc, ExitStack() as ctx:
        const = ctx.enter_context(tc.tile_pool(name="const", bufs=1))

        # ---- constants
        ident_bf = const.tile([P, P], dt.bfloat16)
        make_identity(nc, ident_bf[:])
        iota_row = const.tile([P, P], dt.bfloat16)
        nc.gpsimd.iota(iota_row[:], pattern=[[1, P]], base=0, channel_multiplier=0,
                       allow_small_or_imprecise_dtypes=True)
        iota2k = const.tile([P, RND * 128], dt.float16)
        nc.gpsimd.iota(iota2k[:], pattern=[[1, RND * 128]], base=0,
                       channel_multiplier=0, allow_small_or_imprecise_dtypes=True)
        ones_bf = const.tile([P, 1], dt.bfloat16)
        nc.gpsimd.memset(ones_bf[:], 1.0)
        iota_cp1 = const.tile([P, 1], dt.float32)
        nc.gpsimd.iota(iota_cp1[:], pattern=[[0, 1]], base=1, channel_multiplier=1,
                       allow_small_or_imprecise_dtypes=True)
        shiftmat = const.tile([P, P], dt.float32)
        nc.vector.tensor_scalar(out=shiftmat[:], in0=iota_row[:],
                                scalar1=iota_cp1[:], scalar2=None, op0=op.is_equal)
        w_sb = const.tile([P, HEADS * C], dt.float32)
        nc.sync.dma_start(w_sb[:], w_in.ap()[:, :])
        w_bf = const.tile([P, HEADS * C], dt.bfloat16)
        nc.vector.tensor_scalar(out=w_bf[:], in0=w_sb[:], scalar1=0.0,
                                scalar2=None, op0=op.add)
        bias_bf = const.tile([P, HEADS * C], dt.bfloat16)
        bias_f32 = const.tile([P, HEADS * C], dt.float32)
        nc.sync.dma_start(bias_f32[:], bass.AP(bias_in, 0, [[0, P], [1, HEADS * C]]))
        nc.vector.tensor_scalar(out=bias_bf[:], in0=bias_f32[:], scalar1=0.0,
                                scalar2=None, op0=op.add)

        # wa4[k, i] = sum_c W[k, h*C+c]*att[h, c]; cols: as0 as1 ad0 ad1
        wa4 = const.tile([P, 4], dt.float32)
        wa4hl = const.tile([P, 8], dt.bfloat16)   # [hi0..hi3, lo0..lo3]
        with tc.tile_pool(name="watmp", bufs=2) as tmp_pool:
            for jat, attt in enumerate((asrc_in, adst_in)):
                abc = tmp_pool.tile([P, HEADS * C], dt.float32, tag="abc")
                nc.sync.dma_start(abc[:], bass.AP(attt, 0, [[0, P], [1, HEADS * C]]))
                t = tmp_pool.tile([P, HEADS * C], dt.float32, tag="t")
                nc.vector.tensor_tensor(out=t[:], in0=w_sb[:], in1=abc[:],
                                        op=op.mult)
                for hd in range(HEADS):
                    nc.vector.tensor_reduce(
                        out=wa4[:, 2 * jat + hd:2 * jat + hd + 1],
                        in_=t[:, hd * C:(hd + 1) * C],
                        axis=mybir.AxisListType.X, op=op.add)
            nc.vector.tensor_scalar(out=wa4hl[:, 0:4], in0=wa4[:], scalar1=0.0,
                                    scalar2=None, op0=op.add)
            hic = tmp_pool.tile([P, 4], dt.float32, tag="hic")
            nc.vector.tensor_scalar(out=hic[:], in0=wa4hl[:, 0:4], scalar1=0.0,
                                    scalar2=None, op0=op.add)
            lo32 = tmp_pool.tile([P, 4], dt.float32, tag="lo32")
            nc.vector.tensor_tensor(out=lo32[:], in0=wa4[:], in1=hic[:],
                                    op=op.subtract)
            nc.vector.tensor_scalar(out=wa4hl[:, 4:8], in0=lo32[:], scalar1=0.0,
                                    scalar2=None, op0=op.add)

        # ---- phase A: write a_src/a_dst hi/lo into th[:, 128:136]
        adall = const.tile([P, NTILE, 2], dt.float32)
        ctxA = ExitStack()
        sbA = ctxA.enter_context(tc.tile_pool(name="sbA", bufs=2))
        psT = ctxA.enter_context(tc.tile_pool(name="psT", bufs=2, space="PSUM"))
        psA8 = ctxA.enter_context(tc.tile_pool(name="psA8", bufs=2, space="PSUM"))
        stgA = ctxA.enter_context(tc.tile_pool(name="stgA", bufs=2))

        for t0 in range(0, NTILE, STAGE):
            nst = min(STAGE, NTILE - t0)
            htile = sbA.tile([P, STAGE, 128], dt.bfloat16, tag="htile")
            nc.sync.dma_start(
                htile[:, :nst, :],
                bass.AP(htab, t0 * 128, [[NTILE * 128, P], [128, nst], [1, 128]]))
            tp = psT.tile([P, STAGE * 128], dt.bfloat16, tag="tp", space="PSUM")
            for gi in range(nst):
                nc.tensor.transpose(out=tp[:, gi * 128:(gi + 1) * 128],
                                    in_=htile[:, gi, :], identity=ident_bf[:])
            hT = sbA.tile([P, STAGE * 128], dt.bfloat16, tag="hT")
            hh = (nst // 2) * 128
            nc.vector.tensor_scalar(out=hT[:, :hh], in0=tp[:, :hh],
                                    scalar1=0.0, scalar2=None, op0=op.add)
            if nst * 128 > hh:
                nc.scalar.copy(out=hT[:, hh:nst * 128], in_=tp[:, hh:nst * 128])
            a8 = psA8.tile([P, STAGE, 8], dt.float32, tag="a8", space="PSUM")
            for gi in range(nst):
                nc.tensor.matmul(out=a8[:, gi, :],
                                 lhsT=hT[:, gi * 128:(gi + 1) * 128],
                                 rhs=wa4hl[:], start=True, stop=True)
            a8s = stgA.tile([P, STAGE, 8], dt.float32, tag="a8s")
            nc.vector.tensor_scalar(out=a8s[:, :nst, :], in0=a8[:, :nst, :],
                                    scalar1=0.0, scalar2=None, op0=op.add)
            a4g = stgA.tile([P, STAGE, 4], dt.float32, tag="a4g")
            nc.vector.tensor_tensor(out=a4g[:, :nst, :], in0=a8s[:, :nst, 0:4],
                                    in1=a8s[:, :nst, 4:8], op=op.add)
            nc.vector.tensor_scalar(out=adall[:, t0:t0 + nst, :],
                                    in0=a4g[:, :nst, 2:4],
                                    scalar1=0.0, scalar2=None, op0=op.add)
            # th cols 128:136 hold raw fp32 bits of [as0 as1 ad0 ad1]
            nc.scalar.dma_start(
                bass.AP(th, (128 * t0) * 256 + 128,
                        [[256, P], [128 * 256, nst], [1, 8]]),
                a4g[:, :nst, :].bitcast(dt.bfloat16))
        ctxA.close()

        # ---- select this core's blocks' a_dst from adall (no DRAM roundtrip)
        bgp = ExitStack()
        bgpool = bgp.enter_context(tc.tile_pool(name="bgpool", bufs=1))
        lo_ap = bass.AP(th, 0, [[256, LO], [1, 256]])
        hi_ap = bass.AP(th, HIBASE * 256, [[256, NROWS - HIBASE], [1, 256]])
        bgidx = bgpool.tile([P, 4], dt.int16, tag="bgidx")
        nc.sync.dma_start(bgidx[:], bsel_in.ap()[:, :])
        adsel = bgpool.tile([P, 64, 2], dt.float32, tag="adsel")
        nc.gpsimd.ap_gather(out_ap=adsel[:], in_ap=adall[:], idxs_ap=bgidx[:],
                            channels=128, num_elems=NTILE, d=2, num_idxs=64)
        adf = bgpool.tile([P, POS, 2], dt.float32, tag="adf")
        nc.vector.tensor_copy(out=adf[:], in_=adsel[:, :POS, :])
        # shifted[d] = a_dst[d-1] via shift-matrix matmul (exact in fp32 psum)
        psBG = bgp.enter_context(tc.tile_pool(name="psBG", bufs=1, space="PSUM"))
        sh4 = psBG.tile([P, POS, 2], dt.float32, tag="sh4", space="PSUM")
        nc.tensor.matmul(out=sh4[:], lhsT=shiftmat[:], rhs=adf[:],
                         start=True, stop=True)
        shf = bgpool.tile([P, POS, 2], dt.float32, tag="shf")
        nc.vector.tensor_scalar(out=shf[:], in0=sh4[:], scalar1=0.0,
                                scalar2=None, op0=op.add)
        dlt = bgpool.tile([P, POS, 2], dt.float32, tag="dlt")
        nc.vector.tensor_tensor(out=dlt[:], in0=adf[:], in1=shf[:],
                                op=op.subtract)
        delta4 = const.tile([P, POS, 4], dt.float16)
        nc.vector.tensor_scalar(out=delta4[:, :, 0:2], in0=dlt[:],
                                scalar1=0.0, scalar2=None, op0=op.add)
        dhc = bgpool.tile([P, POS, 2], dt.float32, tag="dhc")
        nc.vector.tensor_scalar(out=dhc[:], in0=delta4[:, :, 0:2],
                                scalar1=0.0, scalar2=None, op0=op.add)
        dlo = bgpool.tile([P, POS, 2], dt.float32, tag="dlo")
        nc.vector.tensor_tensor(out=dlo[:], in0=dlt[:], in1=dhc[:],
                                op=op.subtract)
        nc.vector.tensor_scalar(out=delta4[:, :, 2:4], in0=dlo[:],
                                scalar1=0.0, scalar2=None, op0=op.add)
        bgp.close()

        # ---- phase B preloads
        wlo_sb = const.tile([P, SUM_LO * 8], dt.int16)
        nc.sync.dma_start(wlo_sb[:], wlo_in.ap()[:, :])
        whi_sb = const.tile([P, SUM_HI * 8], dt.int16)
        nc.sync.dma_start(whi_sb[:], whi_in.ap()[:, :])
        dpj_sb = const.tile([P, SUM_K], dt.float32)
        nc.sync.dma_start(dpj_sb[:], dpj_in.ap()[:, :])
        bnd_sb = const.tile([P, max(SUM_R, 1)], dt.float32)
        nc.sync.dma_start(bnd_sb[:], bnd_in.ap()[:, :])

        gh = ctx.enter_context(tc.tile_pool(name="gh", bufs=2))
        smp = ctx.enter_context(tc.tile_pool(name="smp", bufs=3))
        exp_ = ctx.enter_context(tc.tile_pool(name="exp", bufs=4))
        tsp = ctx.enter_context(tc.tile_pool(name="tsp", bufs=3))
        fin = ctx.enter_context(tc.tile_pool(name="fin", bufs=3))
        psGT = ctx.enter_context(tc.tile_pool(name="psGT", bufs=2, space="PSUM"))
        psSS = ctx.enter_context(tc.tile_pool(name="psSS", bufs=1, space="PSUM"))
        psAD = ctx.enter_context(tc.tile_pool(name="psAD", bufs=2, space="PSUM"))
        psU = ctx.enter_context(tc.tile_pool(name="psU", bufs=2, space="PSUM"))

        off_lo = [int(sum(K_LO[:j])) for j in range(POS + 1)]
        off_hi = [int(sum(K_HI[:j])) for j in range(POS + 1)]
        off_k = [int(sum(K_LO[:j]) + sum(K_HI[:j])) for j in range(POS + 1)]
        off_r = [0]
        for j in range(POS):
            off_r.append(off_r[-1] + R_LO[j] + R_HI[j])

        # software pipeline: emit logits(j) ahead of chunks(j-1) so the DVE
        # queue never head-of-line blocks on ex (Act) readiness
        pos_grp = []
        for gi, ng in enumerate(GRPS):
            pos_grp += [gi] * ng
        gh_tiles = {}
        ex_t = {}

        def emit_gather(gi):
            g0, ng = g_starts[gi], GRPS[gi]
            slo = off_lo[g0 + ng] - off_lo[g0]
            shi = off_hi[g0 + ng] - off_hi[g0]
            ghlo = gh.tile([P, GLOMAX, 256], dt.bfloat16, tag="ghlo")
            nc.gpsimd.dma_gather(
                out_ap=ghlo[:, :slo, :], in_ap=lo_ap,
                idxs_ap=wlo_sb[:, off_lo[g0] * 8:(off_lo[g0] + slo) * 8],
                num_idxs=slo * 128, num_idxs_reg=slo * 128, elem_size=256,
                single_packet=False)
            ghhi = gh.tile([P, GHIMAX, 256], dt.bfloat16, tag="ghhi")
            nc.gpsimd.dma_gather(
                out_ap=ghhi[:, :shi, :], in_ap=hi_ap,
                idxs_ap=whi_sb[:, off_hi[g0] * 8:(off_hi[g0] + shi) * 8],
                num_idxs=shi * 128, num_idxs_reg=shi * 128, elem_size=256,
                single_packet=False)
            gh_tiles[gi] = (ghlo, ghhi)

        def emit_logits(j):
            KL = int(K_LO[j])
            KH = int(K_HI[j])
            K = KL + KH
            gi = pos_grp[j]
            g0 = g_starts[gi]
            ghlo, ghhi = gh_tiles[gi]
            lbase = off_lo[j] - off_lo[g0]
            hbase = off_hi[j] - off_hi[g0]

            # staircase a_dst per slot
            adp = psAD.tile([P, KMAX, 4], dt.float32, tag="adp", space="PSUM")
            rcol = off_r[j]
            for h, KHF, base in ((0, KL, 0), (1, KH, KL)):
                R = math.ceil(KHF / RND) if KHF else 0
                for r in range(R):
                    c0 = r * RND
                    nch = min(RND, KHF - c0)
                    sm = smp.tile([P, RND * 128], dt.float16, tag="sm")
                    nc.vector.tensor_scalar(
                        out=sm[:, :nch * 128], in0=iota2k[:, :nch * 128],
                        scalar1=bnd_sb[:, rcol:rcol + 1], scalar2=None,
                        op0=op.is_ge)
                    for jj in range(nch):
                        nc.tensor.matmul(
                            out=adp[:, base + c0 + jj, :],
                            lhsT=sm[:, jj * 128:(jj + 1) * 128],
                            rhs=delta4[:, j, :], start=True, stop=True)
                    rcol += 1

            # logits -> ex  (th cols 128:132 = fp32 bits of [as0 as1])
            t1 = tsp.tile([P, KMAX, 2], dt.float32, tag="t1")
            if KL:
                nc.vector.tensor_tensor(
                    out=t1[:, :KL, :],
                    in0=ghlo[:, lbase:lbase + KL, 128:132].bitcast(dt.float32),
                    in1=adp[:, :KL, 0:2], op=op.add)
            if KH:
                nc.vector.tensor_tensor(
                    out=t1[:, KL:K, :],
                    in0=ghhi[:, hbase:hbase + KH, 128:132].bitcast(dt.float32),
                    in1=adp[:, KL:K, 0:2], op=op.add)
            tsum = tsp.tile([P, KMAX, 2], dt.float32, tag="tsum")
            nc.vector.tensor_tensor(out=tsum[:, :K, :], in0=t1[:, :K, :],
                                    in1=adp[:, :K, 2:4], op=op.add)
            u02 = tsp.tile([P, KMAX, 2], dt.float32, tag="u02")
            nc.vector.tensor_scalar(out=u02[:, :K, :], in0=tsum[:, :K, :],
                                    scalar1=NEG_SLOPE, scalar2=None,
                                    op0=op.mult)
            lrt = tsp.tile([P, KMAX, 2], dt.float32, tag="lrt")
            nc.vector.tensor_tensor(out=lrt[:, :K, :], in0=tsum[:, :K, :],
                                    in1=u02[:, :K, :], op=op.max)
            ex = tsp.tile([P, KMAX, 2], dt.float32, tag="ex")
            nc.scalar.activation(out=ex[:, :K, :], in_=lrt[:, :K, :],
                                 func=act.Exp)
            ex_t[j] = ex

        def emit_chunks(j):
            KL = int(K_LO[j])
            KH = int(K_HI[j])
            K = KL + KH
            gi = pos_grp[j]
            g0 = g_starts[gi]
            ghlo, ghhi = gh_tiles[gi]
            lbase = off_lo[j] - off_lo[g0]
            hbase = off_hi[j] - off_hi[g0]
            ex = ex_t.pop(j)

            gtt = psGT.tile([P, HEADS * C], dt.float32, tag="gtt", space="PSUM")
            ss0 = psSS.tile([P, 1], dt.float32, tag="ss0", space="PSUM")
            ss1 = psSS.tile([P, 1], dt.float32, tag="ss1", space="PSUM")
            for jc in range(K):
                if jc < KL:
                    hgc = ghlo[:, lbase + jc, 0:128]
                else:
                    hgc = ghhi[:, hbase + (jc - KL), 0:128]
                st_ = jc == 0
                sp_ = jc == K - 1
                exm = exp_.tile([P, 2 * P], dt.bfloat16, tag="exm")
                for hd in range(HEADS):
                    nc.vector.tensor_scalar(
                        out=exm[:, hd * P:(hd + 1) * P], in0=iota_row[:],
                        scalar1=dpj_sb[:, off_k[j] + jc:off_k[j] + jc + 1],
                        scalar2=ex[:, jc, hd:hd + 1],
                        op0=op.is_equal, op1=op.mult)
                nc.tensor.matmul(out=gtt[:], lhsT=hgc, rhs=exm[:],
                                 start=st_, stop=sp_)
                nc.tensor.matmul(out=ss0[:], lhsT=exm[:, 0:P],
                                 rhs=ones_bf[:], start=st_, stop=sp_)
                nc.tensor.matmul(out=ss1[:], lhsT=exm[:, P:2 * P],
                                 rhs=ones_bf[:], start=st_, stop=sp_)

            # finalize position j
            rec = fin.tile([P, 2], dt.float32, tag="rec")
            nc.vector.reciprocal(out=rec[:, 0:1], in_=ss0[:])
            nc.vector.reciprocal(out=rec[:, 1:2], in_=ss1[:])
            gs = fin.tile([P, HEADS * C], dt.bfloat16, tag="gs")
            nc.scalar.copy(out=gs[:], in_=gtt[:])
            ot = fin.tile([P, HEADS * C], dt.bfloat16, tag="ot")
            for hd in range(HEADS):
                u = psU.tile([P, C], dt.float32, tag="u", space="PSUM")
                nc.tensor.matmul(out=u[:],
                                 lhsT=gs[:, hd * P:(hd + 1) * P],
                                 rhs=w_bf[:, hd * C:(hd + 1) * C],
                                 start=True, stop=True)
                nc.scalar.mul(out=ot[:, hd * C:(hd + 1) * C],
                              in_=u[:],
                              mul=rec[:, hd:hd + 1])
            zt = fin.tile([P, HEADS * C], dt.bfloat16, tag="zt")
            nc.gpsimd.tensor_tensor(out=zt[:], in0=ot[:], in1=bias_bf[:],
                                    op=op.add)
            et = fin.tile([P, HEADS * C], dt.bfloat16, tag="et")
            nc.scalar.activation(out=et[:], in_=zt[:], func=act.Exp)
            mt = fin.tile([P, HEADS * C], dt.bfloat16, tag="mt")
            nc.vector.tensor_scalar(out=mt[:], in0=et[:], scalar1=1.0,
                                    scalar2=-1.0, op0=op.min, op1=op.add)
            rt = fin.tile([P, HEADS * C], dt.bfloat16, tag="rt")
            nc.scalar.activation(out=rt[:], in_=zt[:], func=act.Relu)
            ob = fin.tile([P, HEADS * C], dt.bfloat16, tag="ob")
            nc.gpsimd.tensor_tensor(out=ob[:], in0=mt[:], in1=rt[:],
                                    op=op.add)
            obf = fin.tile([P, HEADS * C], dt.float32, tag="obf")
            nc.scalar.copy(out=obf[:], in_=ob[:])
            nc.sync.dma_start(out_t.ap()[j * P:(j + 1) * P, :], obf[:])

        emitted_gi = -1
        for j in range(POS):
            if pos_grp[j] > emitted_gi:
                emit_gather(pos_grp[j])
                emitted_gi = pos_grp[j]
            emit_logits(j)
            emit_chunks(j)

    nc.compile()
    return nc


def _get_program(K_LO, K_HI, R_LO, R_HI):
    key = (tuple(K_LO), tuple(K_HI))
    if key not in _CACHE:
        _CACHE[key] = _build(K_LO, K_HI, R_LO, R_HI)
    return _CACHE[key]


# ------------------------------------------------------------------- kernel
def kernel(h_node, edge_index, W, att_src, att_dst, bias):
    from concourse.bass_utils import run_bass_kernel_spmd

    h_node = np.asarray(h_node, dtype=np.float32)
    W = np.asarray(W, dtype=np.float32)
    att_src = np.asarray(att_src, dtype=np.float32)
    att_dst = np.asarray(att_dst, dtype=np.float32)
    bias = np.asarray(bias, dtype=np.float32).reshape(1, HEADS * C)

    pr = _prep(np.asarray(edge_index))
    nc = _get_program(pr["K_LO"], pr["K_HI"], pr["R_LO"], pr["R_HI"])

    hb = np.zeros((NROWS, 128), dtype=BF16)
    hb[:N] = h_node.astype(BF16)
    # p-major layout: row p*NTILE + t = node 128*t + p
    htab = np.ascontiguousarray(
        hb.reshape(NTILE, 128, 128).transpose(1, 0, 2)).reshape(128 * NTILE, 128)
    thh = np.zeros((NROWS, 256), dtype=BF16)
    thh[:, 0:128] = hb

    in_maps = []
    for c in range(NC_CORES):
        in_maps.append({
            "htab": htab, "th": thh, "w_in": W, "asrc_in": att_src,
            "adst_in": att_dst, "bias_in": bias,
            "wlo": pr["wlo"][c], "whi": pr["whi"][c], "dpj": pr["dpj"][c],
            "bnd": pr["bnd"][c], "bsel": pr["bsel"][c],
        })
    res = run_bass_kernel_spmd(nc, in_maps, core_ids=list(range(NC_CORES)))
    out = np.zeros((N, HEADS * C), dtype=np.float32)
    gmap = pr["gmap"]
    for c in range(NC_CORES):
        o = res.results[c]["out"]
        for j in range(POS):
            gg = gmap[c, j]
            if gg < 0:
                continue
            lo_n = 128 * gg
            hi_n = min(128 * (gg + 1), N)
            out[lo_n:hi_n] = o[j * 128:j * 128 + (hi_n - lo_n)]
    return out
